# revision 46
# baseline (speedup 1.0000x reference)
"""Trainium2 Bass kernel for nn_ChartQualityEvaluator (bf16 rewrite).

Data parallel: 32 samples -> 8 cores x 4 samples. Feature-major activations
[128 part, 2 blocks, 4*500 cols], bf16 residual stream, all matmuls bf16
(1 cyc/row vs 4 for fp32). Host folds LN gain/bias into adjacent weight
matrices, q/k biases into a per-key exp-bias column (softmax shift
invariance), and the v bias into Wo's bias. Softmax esum rides a DVE
tree-add + 1-row matmuls; normalization is batched per 4 heads.
"""
import math
import sys

import numpy as np

_TRN = "/opt/trn_rl_repo"
if _TRN not in sys.path:
    sys.path.insert(0, _TRN)

import ml_dtypes

BF16 = ml_dtypes.bfloat16

D = 256
H = 8
NLAYERS = 6
HALF = 128
S = 500
NEV = 256
NCORES = 8
NS = 4  # samples per core
B = 32
EPS = 1e-5
INV2PI = float(np.float32(1.0 / (2.0 * math.pi)))
TWOPI = 2.0 * math.pi
SC32 = 1.0 / math.sqrt(32.0)


def _host_prep(inp):
    f = np.float32
    f8 = np.float64
    out = {}

    def t2(v):  # [256] -> [128,2] fp32
        return np.ascontiguousarray(np.asarray(v).reshape(2, 128).T.astype(f))

    def b(a):  # to bf16
        return np.ascontiguousarray(np.asarray(a).astype(f).astype(BF16))

    # ---------------- frontend ----------------
    out["wmelT"] = b(np.asarray(inp["mel_W"]).T)
    out["melb"] = np.ascontiguousarray(np.asarray(inp["mel_b"]).reshape(16, 1).astype(f))
    out["w1t"] = b(np.asarray(inp["conv1_w"]).transpose(1, 2, 0))
    out["c1b"] = np.ascontiguousarray(np.asarray(inp["conv1_b"]).reshape(128, 1).astype(f))
    out["gng"] = np.ascontiguousarray(np.asarray(inp["gn_g"]).reshape(128, 1).astype(f))
    out["gnb"] = np.ascontiguousarray(np.asarray(inp["gn_b"]).reshape(128, 1).astype(f))
    out["w2t"] = b(np.asarray(inp["conv2_w"]).transpose(1, 2, 0))
    out["c2b"] = t2(inp["conv2_b"])
    out["cng"] = t2(inp["cn_g"])

    freq = np.exp(np.arange(HALF, dtype=f) * f(-math.log(10000.0) / (HALF - 1)))
    e32 = (np.arange(S, dtype=f)[None, :] * freq[:, None]).astype(f)
    e64 = e32.astype(np.float64)
    pos_fm = np.concatenate([np.sin(e64), np.cos(e64)], axis=0)  # [256,500]
    out["posT"] = np.ascontiguousarray(
        pos_fm.reshape(2, 128, S).transpose(1, 0, 2).astype(f))
    out["freqv"] = np.ascontiguousarray(freq.reshape(128, 1))

    out["epW1T"] = np.ascontiguousarray(
        np.asarray(inp["ep_W1"]).T.reshape(6, 128, 256).transpose(1, 0, 2).astype(f).astype(BF16))
    out["epb1"] = t2(inp["ep_b1"])
    out["epW2T"] = np.ascontiguousarray(
        np.asarray(inp["ep_W2"]).T.reshape(2, 128, 256).transpose(1, 0, 2).astype(f).astype(BF16))
    out["epb2row"] = np.ascontiguousarray(
        np.tile(np.asarray(inp["ep_b2"]).astype(f)[None, :], (128, 1)))

    def wT(w, kc, m, dt=BF16):  # w [m, k] -> [128, kc, m]
        return np.ascontiguousarray(
            np.asarray(w).astype(f).T.reshape(kc, 128, m).transpose(1, 0, 2).astype(dt))

    # ---------------- transformer with LN folding ----------------
    wqkvT, woT, boW, w1TT, b1v, w2TT, b2W = [], [], [], [], [], [], []
    for i in range(NLAYERS):
        g1 = np.asarray(inp["tl_ln1g"][i]).astype(f8)
        b1 = np.asarray(inp["tl_ln1b"][i]).astype(f8)
        g2 = np.asarray(inp["tl_ln2g"][i]).astype(f8)
        b2 = np.asarray(inp["tl_ln2b"][i]).astype(f8)
        Wqkv = np.asarray(inp["tl_Wqkv"][i]).astype(f8)   # [768, 256]
        bqkv = np.asarray(inp["tl_bqkv"][i]).astype(f8)   # [768]
        Wo = np.asarray(inp["tl_Wo"][i]).astype(f8)       # [256, 256]
        bo = np.asarray(inp["tl_bo"][i]).astype(f8)
        W1 = np.asarray(inp["tl_W1"][i]).astype(f8)       # [1024, 256]
        b1f = np.asarray(inp["tl_b1"][i]).astype(f8)
        W2 = np.asarray(inp["tl_W2"][i]).astype(f8)       # [256, 1024]
        b2f = np.asarray(inp["tl_b2"][i]).astype(f8)

        Wq, Wk, Wv = Wqkv[0:256], Wqkv[256:512], Wqkv[512:768]
        Wq_f = Wq * g1[None, :]
        Wk_f = Wk * g1[None, :] * SC32
        Wv_f = Wv * g1[None, :]
        bq_eff = Wq @ b1 + bqkv[0:256]          # survives as per-key exp bias
        bv_eff = Wv @ b1 + bqkv[512:768]        # folds into Wo bias
        # wc_h: c_h[k] = bq_eff_h . k''_h[:,k] = (Wk_f[h].T @ bq_eff[h]) . zhat
        WC = np.stack([Wk_f[32 * h:32 * h + 32].T @ bq_eff[32 * h:32 * h + 32]
                       for h in range(8)])      # [8, 256]
        W_f = np.concatenate([Wq_f, Wk_f, Wv_f, WC], axis=0)  # [776, 256]
        wqkvT.append(wT(W_f, 2, 776))
        bo_eff = bo + Wo @ bv_eff
        woT.append(wT(Wo, 2, 256))
        boW.append(np.ascontiguousarray(bo_eff.astype(f).reshape(1, 256).astype(BF16)))
        W1_f = W1 * g2[None, :]
        b1_eff = W1 @ b2 + b1f
        w1TT.append(wT(W1_f, 2, 1024))
        b1v.append(np.ascontiguousarray(
            b1_eff.astype(f).reshape(8, 128).T.astype(f)))     # [128, 8] fp32
        w2TT.append(wT(W2, 8, 256))
        b2W.append(np.ascontiguousarray(b2f.astype(f).reshape(1, 256).astype(BF16)))
    out["wqkvT"] = np.stack(wqkvT)
    out["woT"] = np.stack(woT)
    out["boW"] = np.stack(boW)
    out["w1TT"] = np.stack(w1TT)
    out["b1v"] = np.stack(b1v)
    out["w2TT"] = np.stack(w2TT)
    out["b2W"] = np.stack(b2W)

    indT = np.zeros((16, 4, 128), np.float32)
    for qb in range(4):
        for p in range(128):
            indT[4 * (p // 32) + qb, qb, p] = 1.0
    out["indTc"] = np.ascontiguousarray(indT.astype(BF16))

    # ---------------- pooling head ----------------
    out["poolq"] = np.ascontiguousarray(
        np.asarray(inp["pool_q"]).astype(f).reshape(2, 128).T)
    c1 = (np.asarray(inp["oh_W"])[0] * np.asarray(inp["on_g"])).astype(f)
    out["c1v"] = t2(c1)
    out["sc1"] = float(c1.astype(np.float64).sum())
    out["c2s"] = float((np.asarray(inp["oh_W"])[0].astype(np.float64)
                        * np.asarray(inp["on_b"]).astype(np.float64)).sum()
                       + float(np.asarray(inp["oh_b"])[0]))

    # ---------------- per-sample host precompute ----------------
    events = np.asarray(inp["events"]).astype(np.int64)
    mask = np.asarray(inp["event_mask"])
    star = np.asarray(inp["star_rating"]).astype(f)
    nb = events.shape[0]
    diff = np.maximum(events[:, 1:] - events[:, :-1], 1)
    g = np.concatenate([diff[:, :1], diff], axis=1)
    gap_ms = (g * 5).astype(f)
    g_f = np.maximum(g.astype(f), f(1.0))
    r = np.clip(g_f[:, 1:] / g_f[:, :-1], f(0.1), f(10.0)).astype(f)
    ones = np.ones((nb, 1), f)
    rb50 = np.trunc(np.concatenate([ones, r], axis=1) * f(50.0)).astype(f)
    ra50 = np.trunc(np.concatenate([r, ones], axis=1) * f(50.0)).astype(f)
    out["_evrows"] = np.ascontiguousarray(np.stack([rb50, ra50, gap_ms], axis=1))
    tp = np.clip(events // 4, 0, S - 1).astype(f)
    keep = (1.0 - mask.astype(f)).astype(f)
    out["_tposv"] = np.ascontiguousarray(tp.reshape(nb, 2, 128).transpose(0, 2, 1))
    out["_keepv"] = np.ascontiguousarray(keep.reshape(nb, 2, 128).transpose(0, 2, 1))
    bucket = np.clip((star / f(0.5)).astype(np.int32), 0, 19)
    sb = (np.asarray(inp["cn_b"])[None, :] + np.asarray(inp["star_table"])[bucket]).astype(f)
    out["_starbias"] = np.ascontiguousarray(sb.reshape(nb, 2, 128).transpose(0, 2, 1))
    out["_mel"] = np.asarray(inp["mel"]).astype(f).astype(BF16)
    return out


CONST_KEYS = ["wmelT", "melb", "w1t", "c1b", "gng", "gnb", "w2t", "c2b", "cng",
              "posT", "freqv", "epW1T", "epb1", "epW2T", "epb2row",
              "wqkvT", "woT", "boW", "w1TT", "b1v", "w2TT", "b2W",
              "poolq", "c1v", "indTc"]


def _build(nl_run=NLAYERS, ns_run=NS, debug=False, sc1=0.0, c2s=0.0):
    import concourse.bacc as bacc
    import concourse.tile as tile
    from concourse import mybir
    from concourse.masks import make_identity

    class _Bacc(bacc.Bacc):
        # Restrict activation-table choice so ln/exp (and everything the
        # kernel needs besides gelu/sin) resolve to one shared table --
        # avoids a table reload on every LayerNorm. Table ids stay
        # positional, so unused entries are blanked rather than removed.
        _KEEP = {"natural_log_exp_and_others", "trig_and_small",
                 "gelu_and_others", "sqrt_and_others"}

        def insert_act_table_loads(self):
            from concourse.hw_specs import get_activation_tables
            import bass_rust as _bass_rust
            has_activation = any(
                isinstance(i, mybir.InstActivation)
                for b in self.main_func.blocks
                for i in b.instructions
            )
            if not has_activation:
                return
            tables = [
                (name, funcs if name in self._KEEP else set())
                for name, funcs in get_activation_tables(self.m.arch).items()
            ]
            _bass_rust.insert_act_table_loads(self, tables)

    f32 = mybir.dt.float32
    bf16 = mybir.dt.bfloat16
    f32r = mybir.dt.float32r
    f8 = mybir.dt.float8e4
    A = mybir.AluOpType
    AF = mybir.ActivationFunctionType
    AX = mybir.AxisListType
    DR = mybir.MatmulPerfMode.DoubleRow

    def r32(ap):
        # fp32 matmul streams at 4 cyc/row; f32r at 1 (ap>=256). Same bytes.
        return ap.bitcast(f32r)

    nc = _Bacc(None)
    P = {}
    shapes = dict(
        mel4=([ns_run, 80, 2000], bf16),
        evrows=([ns_run, 3, 256], f32),
        tposv=([ns_run, 128, 2], f32),
        keepv=([ns_run, 128, 2], f32),
        starbias=([ns_run, 128, 2], f32),
        wmelT=([80, 16], bf16), melb=([16, 1], f32),
        w1t=([16, 7, 128], bf16), c1b=([128, 1], f32),
        gng=([128, 1], f32), gnb=([128, 1], f32),
        w2t=([128, 7, 256], bf16), c2b=([128, 2], f32),
        cng=([128, 2], f32), posT=([128, 2, 500], f32), freqv=([128, 1], f32),
        epW1T=([128, 6, 256], bf16), epb1=([128, 2], f32),
        epW2T=([128, 2, 256], bf16), epb2row=([128, 256], f32),
        wqkvT=([NLAYERS, 128, 2, 776], bf16),
        woT=([NLAYERS, 128, 2, 256], bf16), boW=([NLAYERS, 1, 256], bf16),
        w1TT=([NLAYERS, 128, 2, 1024], bf16), b1v=([NLAYERS, 128, 8], f32),
        w2TT=([NLAYERS, 128, 8, 256], bf16), b2W=([NLAYERS, 1, 256], bf16),
        poolq=([128, 2], f32), c1v=([128, 2], f32),
        indTc=([16, 4, 128], bf16),
    )
    for k, (sh, dt) in shapes.items():
        P[k] = nc.declare_dram_parameter(k, sh, dt, isOutput=False)
    Y = nc.declare_dram_parameter("y", [ns_run, 1], f32, isOutput=True)
    DBG = None
    if debug:
        DBG = nc.declare_dram_parameter("dbg_x0", [128, 2, 2000], f32, isOutput=True)

    with tile.TileContext(nc) as tc:
        sing = tc.alloc_tile_pool(name="sing", bufs=1)
        sc = tc.alloc_tile_pool(name="sc", bufs=2)
        bigx = tc.alloc_tile_pool(name="bigx", bufs=1)
        wpool = tc.alloc_tile_pool(name="wpool", bufs=1)
        # PSUM pools: pmA rotating 1-bank x3, pmB 2-bank, pmP 1-bank,
        # pmS 1-bank (stats+esum), pmT small bf16 transpose staging
        pmA = tc.alloc_tile_pool(name="pmA", bufs=3, space="PSUM")
        pmB = tc.alloc_tile_pool(name="pmB", bufs=1, space="PSUM")
        pmP = tc.alloc_tile_pool(name="pmP", bufs=1, space="PSUM")
        pmS = tc.alloc_tile_pool(name="pmS", bufs=1, space="PSUM")

        fc = tc.alloc_tile_pool(name="fc", bufs=1)
        _FRONT = ["wmelT", "melb", "w1t", "c1b", "gng", "gnb", "w2t", "c2b",
                  "cng", "posT", "freqv", "epW1T", "epb1", "epW2T", "epb2row"]
        C = {}
        for k in _FRONT + ["poolq", "c1v", "indTc"]:
            pool = fc if k in _FRONT else sing
            C[k] = pool.tile(shapes[k][0], shapes[k][1], tag=k, name="c_" + k)
            nc.sync.dma_start(out=C[k][:], in_=P[k][:])
        # transformer weights stream through 2 SBUF slots (DMA is nearly
        # idle); slot for layer i+2 refills while layer i+1 runs
        WSPECS = [("wqkvT", [128, 2, 776], bf16),
                  ("woT", [128, 2, 256], bf16),
                  ("boW", [1, 256], bf16),
                  ("w1TT", [128, 2, 1024], bf16),
                  ("b1v", [128, 8], f32),
                  ("w2TT", [128, 8, 256], bf16),
                  ("b2W", [1, 256], bf16)]

        def load_layer_weights(slot, i):
            W = {}
            for k, sh, dt in WSPECS:
                W[k] = wpool.tile(sh, dt, tag=f"w{slot}_{k}",
                                  name=f"w{slot}_{k}")
                nc.sync.dma_start(out=W[k][:], in_=P[k][i])
            return W

        LWslot = [load_layer_weights(0, 0)]
        if nl_run > 1:
            LWslot.append(load_layer_weights(1, 1))

        identF = sing.tile([128, 128], f32, tag="identF")
        make_identity(nc, identF[:])
        onesPb = sing.tile([128, 1], bf16, tag="onesPb")      # plain ones bf16
        nc.vector.memset(onesPb[:], 1.0)
        onesP8 = sing.tile([128, 1], f8, tag="onesP8")        # ones fp8e4
        nc.vector.memset(onesP8[:], 1.0)
        onesMb = sing.tile([128, 1], bf16, tag="onesMb")      # 1/256 (stats lhsT)
        nc.vector.memset(onesMb[:], 1.0 / 256.0)
        onesMf = sing.tile([128, 1], f32, tag="onesMf")       # 1/256 fp32
        nc.vector.memset(onesMf[:], 1.0 / 256.0)
        ones1b = sing.tile([1, 128], bf16, tag="ones1b")
        nc.vector.memset(ones1b[:], 1.0)
        ones1f = sing.tile([1, 128], f32, tag="ones1f")
        nc.vector.memset(ones1f[:], 1.0)
        ones500b = sing.tile([1, 500], bf16, tag="ones500b")
        nc.vector.memset(ones500b[:], 1.0)
        onesPf = sing.tile([128, 1], f32, tag="onesPf")
        nc.vector.memset(onesPf[:], 1.0)
        zerov = fc.tile([128, 1], f32, tag="zerov")
        nc.vector.memset(zerov[:], 0.0)
        epsv = sing.tile([128, 1], f32, tag="epsv")
        nc.vector.memset(epsv[:], EPS)
        iotaB = fc.tile([128, 500], f32, tag="iotaB")
        nc.gpsimd.iota(iotaB[:], pattern=[[1, 500]], base=0, channel_multiplier=0,
                       allow_small_or_imprecise_dtypes=True)

        x = bigx.tile([128, 2, 2000], f32, tag="x_fm")

        def s2(t):  # step-2 view of [p, n] -> [p, n//2]
            return t.rearrange("p (t s) -> p s t", s=2)[:, 0, :]

        # ================= front end =================
        fr = tc.alloc_tile_pool(name="fr", bufs=2)
        for s in range(ns_run):
            cs = s * 500
            melp = fr.tile([80, 2006], bf16, tag="melp")
            nc.vector.memset(melp[:, 0:3], 0.0)
            nc.vector.memset(melp[:, 2003:2006], 0.0)
            nc.sync.dma_start(out=melp[:, 3:2003], in_=P["mel4"][s])
            xmelp = fr.tile([16, 2006], bf16, tag="xmelp")
            nc.vector.memset(xmelp[:, 0:3], 0.0)
            nc.vector.memset(xmelp[:, 2003:2006], 0.0)
            for nch in range(4):
                pcm = pmA.tile([128, 512], f32, tag="pmA")
                nc.tensor.matmul(out=pcm[:16, 0:500], lhsT=C["wmelT"][:],
                                 rhs=melp[:, 3 + nch * 500: 3 + nch * 500 + 500],
                                 start=True, stop=True)
                nc.scalar.activation(out=xmelp[:, 3 + nch * 500: 3 + nch * 500 + 500],
                                     in_=pcm[:16, 0:500], func=AF.Identity,
                                     bias=C["melb"][:, 0:1])
            pc1 = pmB.tile([128, 2, 512], f32, tag="pmB")
            for half in range(2):
                for k in range(7):
                    nc.tensor.matmul(
                        out=pc1[:, half, 0:500], lhsT=C["w1t"][:, k, :],
                        rhs=s2(xmelp[:, k + half * 1000: k + half * 1000 + 1000]),
                        start=(k == 0), stop=(k == 6))
            h1g = fr.tile([128, 2, 500], bf16, tag="h1g")
            stg = fr.tile([128, 2], f32, tag="stg")
            nc.scalar.activation(out=h1g[:], in_=pc1[:, :, 0:500], func=AF.Gelu,
                                 bias=C["c1b"][:, 0:1], accum_out=stg[:, 0:1])
            sqf = fr.tile([128, 2, 500], bf16, tag="sqf")
            nc.scalar.activation(out=sqf[:], in_=h1g[:], func=AF.Square,
                                 accum_out=stg[:, 1:2])
            pg = pmS.tile([128, 512], f32, tag="pmS")
            nc.tensor.matmul(out=pg[:1, 0:2], lhsT=onesPf[:], rhs=stg[:],
                             start=True, stop=True)
            sn = sc.tile([1, 8], f32, tag="sn")
            nc.vector.tensor_scalar(out=sn[:, 0:2], in0=pg[:1, 0:2],
                                    scalar1=1.0 / 128000.0, scalar2=None, op0=A.mult)
            nc.vector.tensor_tensor(out=sn[:, 2:3], in0=sn[:, 0:1], in1=sn[:, 0:1],
                                    op=A.mult)
            nc.vector.tensor_tensor(out=sn[:, 3:4], in0=sn[:, 1:2], in1=sn[:, 2:3],
                                    op=A.subtract)
            nc.scalar.activation(out=sn[:, 4:5], in_=sn[:, 3:4], func=AF.Ln,
                                 bias=epsv[0:1, :])
            nc.scalar.activation(out=sn[:, 1:2], in_=sn[:, 4:5], func=AF.Exp,
                                 scale=-0.5)
            pgb = pmA.tile([128, 512], f32, tag="pmA")
            nc.tensor.matmul(out=pgb[:, 0:2], lhsT=ones1f[:], rhs=sn[:, 0:2],
                             start=True, stop=True)
            sv = sc.tile([128, 2], f32, tag="sv")
            nc.vector.tensor_tensor(out=sv[:, 0:1], in0=pgb[:, 1:2], in1=C["gng"][:],
                                    op=A.mult)
            nc.vector.tensor_tensor(out=sv[:, 1:2], in0=pgb[:, 0:1], in1=sv[:, 0:1],
                                    op=A.mult)
            nc.vector.tensor_tensor(out=sv[:, 1:2], in0=C["gnb"][:], in1=sv[:, 1:2],
                                    op=A.subtract)
            x2p = fr.tile([128, 1006], bf16, tag="x2p")
            nc.vector.memset(x2p[:, 0:3], 0.0)
            nc.vector.memset(x2p[:, 1003:1006], 0.0)
            nc.scalar.activation(out=x2p[:, 3:1003],
                                 in_=h1g.rearrange("p a b -> p (a b)"),
                                 func=AF.Identity, scale=sv[:, 0:1], bias=sv[:, 1:2])
            pc2 = pmB.tile([128, 2, 512], f32, tag="pmB")
            for mb in range(2):
                for k in range(7):
                    nc.tensor.matmul(out=pc2[:, mb, 0:500],
                                     lhsT=C["w2t"][:, k, mb * 128:(mb + 1) * 128],
                                     rhs=s2(x2p[:, k:k + 1000]),
                                     start=(k == 0), stop=(k == 6))
            for mb in range(2):
                nc.scalar.activation(out=x[:, mb, cs:cs + 500], in_=pc2[:, mb, 0:500],
                                     func=AF.Gelu, bias=C["c2b"][:, mb:mb + 1])
            # CN layernorm (stats in bf16) + cng scale + starbias + pos
            sbv = fr.tile([128, 2], f32, tag="sbv")
            nc.sync.dma_start(out=sbv[:], in_=P["starbias"][s])
            sq2 = fr.tile([128, 2, 500], bf16, tag="sqf")
            nc.vector.tensor_tensor(out=sq2[:], in0=x[:, :, cs:cs + 500],
                                    in1=x[:, :, cs:cs + 500], op=A.mult)
            pstt = pmS.tile([128, 512], f32, tag="pmS")
            for blk in range(2):
                nc.tensor.matmul(out=pstt[0:1, 0:500], lhsT=onesMf[:],
                                 rhs=x[:, blk, cs:cs + 500],
                                 start=(blk == 0), stop=(blk == 1))
            for blk in range(2):
                nc.tensor.matmul(out=pstt[32:33, 0:500], lhsT=onesMb[:],
                                 rhs=sq2[:, blk, :], start=(blk == 0), stop=(blk == 1))
            ru = sc.tile([1, 2, 500], bf16, tag="ru")
            tmp = sc.tile([1, 2, 500], f32, tag="tmp1")
            nc.scalar.activation(out=tmp[:1, 0, :], in_=pstt[0:1, 0:500],
                                 func=AF.Square)
            nc.vector.tensor_tensor(out=tmp[:1, 1, :], in0=pstt[32:33, 0:500],
                                    in1=tmp[:1, 0, :], op=A.subtract)
            nc.scalar.activation(out=tmp[:1, 0, :], in_=tmp[:1, 1, :], func=AF.Ln,
                                 bias=epsv[0:1, :])
            nc.scalar.activation(out=ru[:1, 0, :], in_=tmp[:1, 0, :],
                                 func=AF.Exp, scale=-0.5)
            nc.vector.tensor_tensor(out=ru[:1, 1, :], in0=pstt[0:1, 0:500],
                                    in1=ru[:1, 0, :], op=A.mult)
            pbc = pmB.tile([128, 2, 512], f32, tag="pmB")
            for jj in range(2):
                nc.tensor.matmul(out=pbc[:, jj, 0:500], lhsT=ones1b[:],
                                 rhs=ru[:1, jj, :], start=True, stop=True)
            for blk in range(2):
                nc.vector.tensor_tensor(out=x[:, blk, cs:cs + 500],
                                        in0=x[:, blk, cs:cs + 500],
                                        in1=pbc[:, 0, 0:500], op=A.mult)
                nc.vector.tensor_tensor(out=x[:, blk, cs:cs + 500],
                                        in0=x[:, blk, cs:cs + 500],
                                        in1=pbc[:, 1, 0:500], op=A.subtract)
                nc.scalar.activation(out=x[:, blk, cs:cs + 500],
                                     in_=x[:, blk, cs:cs + 500], func=AF.Identity,
                                     scale=C["cng"][:, blk:blk + 1],
                                     bias=sbv[:, blk:blk + 1])
            nc.vector.tensor_tensor(out=x[:, :, cs:cs + 500], in0=x[:, :, cs:cs + 500],
                                    in1=C["posT"][:], op=A.add)

            # events
            evr = fr.tile([1, 3, 256], f32, tag="evr")
            nc.sync.dma_start(out=evr[:], in_=P["evrows"][s])
            tpv = fr.tile([128, 2], f32, tag="tpv")
            nc.sync.dma_start(out=tpv[:], in_=P["tposv"][s])
            kpv = fr.tile([128, 2], f32, tag="kpv")
            nc.sync.dma_start(out=kpv[:], in_=P["keepv"][s])
            comb = fr.tile([128, 6, 256], bf16, tag="comb")
            for vr in range(3):
                pb = pmA.tile([128, 512], f32, tag="pmA")
                nc.tensor.matmul(out=pb[:, 0:256], lhsT=ones1f[:],
                                 rhs=evr[:1, vr, :], start=True, stop=True)
                arg = fr.tile([128, 256], f32, tag="arg")
                nc.scalar.activation(out=arg[:], in_=pb[:, 0:256], func=AF.Copy,
                                     scale=C["freqv"][:])
                nc.vector.tensor_scalar(out=arg[:], in0=arg[:], scalar1=INV2PI,
                                        scalar2=None, op0=A.mult)
                w1_ = fr.tile([128, 256], f32, tag="w1_")
                ti_ = fr.tile([128, 256], mybir.dt.int32, tag="ti_")
                tf_ = fr.tile([128, 256], f32, tag="tf_")
                nc.vector.tensor_copy(ti_[:], arg[:])
                nc.vector.tensor_copy(tf_[:], ti_[:])
                nc.vector.tensor_tensor(out=w1_[:], in0=arg[:], in1=tf_[:],
                                        op=A.subtract)
                nc.scalar.activation(out=comb[:, 2 * vr, :], in_=w1_[:], func=AF.Sin,
                                     scale=TWOPI, bias=zerov[:])
                nc.vector.tensor_scalar(out=arg[:], in0=arg[:], scalar1=0.25,
                                        scalar2=None, op0=A.add)
                nc.vector.tensor_copy(ti_[:], arg[:])
                nc.vector.tensor_copy(tf_[:], ti_[:])
                nc.vector.tensor_tensor(out=w1_[:], in0=arg[:], in1=tf_[:],
                                        op=A.subtract)
                nc.scalar.activation(out=comb[:, 2 * vr + 1, :], in_=w1_[:],
                                     func=AF.Sin, scale=TWOPI, bias=zerov[:])
            hmid = fr.tile([128, 2, 256], bf16, tag="hmid")
            for mb in range(2):
                ph = pmA.tile([128, 512], f32, tag="pmA")
                for kc in range(6):
                    nc.tensor.matmul(out=ph[:, 0:256],
                                     lhsT=C["epW1T"][:, kc, mb * 128:(mb + 1) * 128],
                                     rhs=comb[:, kc, :], start=(kc == 0),
                                     stop=(kc == 5))
                nc.scalar.activation(out=hmid[:, mb, :], in_=ph[:, 0:256],
                                     func=AF.Gelu, bias=C["epb1"][:, mb:mb + 1])
            evt = fr.tile([128, 2, 256], bf16, tag="evt")
            for ec in range(2):
                pe = pmA.tile([128, 512], f32, tag="pmA")
                for kc in range(2):
                    nc.tensor.matmul(out=pe[:, 0:256],
                                     lhsT=hmid[:, kc, ec * 128:(ec + 1) * 128],
                                     rhs=C["epW2T"][:, kc, :], start=(kc == 0),
                                     stop=(kc == 1))
                nc.vector.tensor_tensor(out=evt[:, ec, :], in0=pe[:, 0:256],
                                        in1=C["epb2row"][:], op=A.add)
                nc.vector.tensor_scalar(out=evt[:, ec, :], in0=evt[:, ec, :],
                                        scalar1=kpv[:, ec:ec + 1], scalar2=None,
                                        op0=A.mult)
            oh = fr.tile([128, 2, 500], bf16, tag="oh")
            for ec in range(2):
                nc.vector.tensor_scalar(out=oh[:, ec, :], in0=iotaB[:],
                                        scalar1=tpv[:, ec:ec + 1], scalar2=None,
                                        op0=A.is_equal)
            for mb in range(2):
                px = pmA.tile([128, 512], f32, tag="pmA")
                for ec in range(2):
                    nc.tensor.matmul(out=px[:, 0:500],
                                     lhsT=evt[:, ec, mb * 128:(mb + 1) * 128],
                                     rhs=oh[:, ec, :], start=(ec == 0), stop=(ec == 1))
                nc.vector.tensor_tensor(out=x[:, mb, cs:cs + 500],
                                        in0=x[:, mb, cs:cs + 500], in1=px[:, 0:500],
                                        op=A.add)
        fr.release()
        fc.release()
        big = tc.alloc_tile_pool(name="big", bufs=1)
        scr = tc.alloc_tile_pool(name="scr", bufs=1)
        scr2 = tc.alloc_tile_pool(name="scr2", bufs=1)

        if debug:
            nc.sync.dma_start(out=DBG[:], in_=x[:])

        # ================= transformer =================
        def emit_ln_stats(s):
            # Per-column mean / E[x^2] of x(s) into rows 0/32 of a pmB tile
            # (same tile later reused for the r/u broadcast).
            co = s * 500
            sq = scr2.tile([128, 2, 500], bf16, tag="sq")
            nc.vector.tensor_tensor(out=sq[:], in0=x[:, :, co:co + 500],
                                    in1=x[:, :, co:co + 500], op=A.mult)
            pbt = pmB.tile([128, 2, 512], f32, tag="pmB")
            for blk in range(2):
                nc.tensor.matmul(out=pbt[0:1, 0, 0:500], lhsT=onesMf[:],
                                 rhs=x[:, blk, co:co + 500],
                                 start=(blk == 0), stop=(blk == 1))
            for blk in range(2):
                nc.tensor.matmul(out=pbt[32:33, 0, 0:500], lhsT=onesMb[:],
                                 rhs=sq[:, blk, :], start=(blk == 0),
                                 stop=(blk == 1))
            return pbt

        def emit_ln_finish(xn, s, pbt):
            # 1/sigma = exp(-0.5*ln(var)) keeps ACT on the exp/ln table.
            co = s * 500
            ru = sc.tile([1, 2, 500], bf16, tag="ru")
            tmp = sc.tile([1, 2, 500], f32, tag="tmp1")
            nc.scalar.activation(out=tmp[:1, 0, :], in_=pbt[0:1, 0, 0:500],
                                 func=AF.Square)
            nc.vector.tensor_tensor(out=tmp[:1, 1, :], in0=pbt[32:33, 0, 0:500],
                                    in1=tmp[:1, 0, :], op=A.subtract)
            nc.scalar.activation(out=tmp[:1, 0, :], in_=tmp[:1, 1, :],
                                 func=AF.Ln, bias=epsv[0:1, :])
            nc.scalar.activation(out=ru[:1, 0, :], in_=tmp[:1, 0, :],
                                 func=AF.Exp, scale=-0.5)
            nc.vector.tensor_tensor(out=ru[:1, 1, :], in0=pbt[0:1, 0, 0:500],
                                    in1=ru[:1, 0, :], op=A.mult)
            for jj in range(2):
                nc.tensor.matmul(out=pbt[:, jj, 0:500], lhsT=ones1b[:],
                                 rhs=ru[:1, jj, :], start=True, stop=True)
            for blk in range(2):
                nc.vector.tensor_tensor(out=xn[:, blk, co:co + 500],
                                        in0=x[:, blk, co:co + 500],
                                        in1=pbt[:, 0, 0:500], op=A.mult)
                nc.vector.tensor_tensor(out=xn[:, blk, co:co + 500],
                                        in0=xn[:, blk, co:co + 500],
                                        in1=pbt[:, 1, 0:500], op=A.subtract)

        def emit_ln(xn, s):
            emit_ln_finish(xn, s, emit_ln_stats(s))

        def emit_ln_stats_sb(s, st4):
            # Stage (mean, var) at partition 0 so the sqrt/recip finish can
            # run after the whole gelu phase (one table swap per phase).
            # Square is in every act table, so no load here.
            pbt = emit_ln_stats(s)
            m2 = sc.tile([1, 512], f32, tag="m2sc")
            nc.vector.tensor_copy(st4[0:1, 0, s, :], pbt[0:1, 0, 0:500])
            nc.scalar.activation(out=m2[:1, 0:500], in_=pbt[0:1, 0, 0:500],
                                 func=AF.Square)
            nc.vector.tensor_tensor(out=st4[0:1, 1, s, :],
                                    in0=pbt[32:33, 0, 0:500],
                                    in1=m2[:1, 0:500], op=A.subtract)

        def emit_ln_ru4(st4):
            # Batched r/u for all 4 samples (Sqrt + DVE recip: one table load
            # at the phase boundary). Reading the full st4 var plane makes
            # this depend on sample 3's stats, keeping table-based ACT work
            # off the gelu phase.
            ru4 = scr2.tile([1, 2, 4, 500], bf16, tag="ru4")
            nc.scalar.activation(out=ru4[:1, 0, :, :], in_=st4[:1, 1, :, :],
                                 func=AF.Sqrt, bias=epsv[0:1, :])
            # r = 1/sigma; u = mean*r (bf16, same precision as the inline path)
            with nc.allow_low_precision("ln r/u bf16 as inline path"):
                nc.vector.reciprocal(out=ru4[:1, 1, :, :],
                                     in_=ru4[:1, 0, :, :])
            nc.vector.tensor_tensor(out=ru4[:1, 0, :, :],
                                    in0=st4[:1, 0, :, :],
                                    in1=ru4[:1, 1, :, :], op=A.mult)
            return ru4

        def emit_ln_apply(xn, s, ru4):
            # Broadcast r/u for one sample and normalize. Emitted with one
            # sample of lookahead so the in-order PE queue never stalls on ru4.
            co = s * 500
            pbt = pmB.tile([128, 2, 512], f32, tag="pmB")
            nc.tensor.matmul(out=pbt[:, 0, 0:500], lhsT=ones1b[:],
                             rhs=ru4[:1, 1, s, :], start=True, stop=True)
            nc.tensor.matmul(out=pbt[:, 1, 0:500], lhsT=ones1b[:],
                             rhs=ru4[:1, 0, s, :], start=True, stop=True)
            for blk in range(2):
                nc.vector.tensor_tensor(out=xn[:, blk, co:co + 500],
                                        in0=x[:, blk, co:co + 500],
                                        in1=pbt[:, 0, 0:500], op=A.mult)
                nc.vector.tensor_tensor(out=xn[:, blk, co:co + 500],
                                        in0=xn[:, blk, co:co + 500],
                                        in1=pbt[:, 1, 0:500], op=A.subtract)

        UNITS = [(c4, j) for j in range(2) for c4 in range(4)]
        LAG = 4
        xn = big.tile([128, 2, 2000], bf16, tag="xnA")
        for s in range(ns_run):
            emit_ln(xn, s)
        pend_ln = None
        for i in range(nl_run):
            W = LWslot[i % 2]
            attn = big.tile([128, 2, 2000], bf16, tag="attn")
            xn2 = big.tile([128, 2, 2000], bf16, tag="xn2")
            lnst = {}
            for s in range(ns_run):
                cs = s * 500
                if pend_ln is not None and s + 1 < ns_run:
                    emit_ln_apply(pend_ln[0], s + 1, pend_ln[1])
                if s >= 1:
                    lnst[s - 1] = emit_ln_stats(s - 1)
                # double-buffered per sample so qkv(s+1) can run under the
                # units pipeline of sample s
                qkv = scr.tile([128, 6, 512], bf16, tag=f"qkv{s % 2}")
                cqS = scr.tile([8, 512], f32, tag=f"cqS{s % 2}")
                if i == 0 and s <= 1:
                    # pad keys 500..511: k''=0, v=0 -> score 0, av contrib 0
                    nc.vector.memset(qkv[:, 2:6, 500:512], 0.0)
                    # exp bias -30 at pad keys -> eT ~ 0 there
                    nc.vector.memset(cqS[:, 500:512], -30.0)
                for j in [0, 2, 4, 5, 1, 3]:
                    pq = pmA.tile([128, 512], f32, tag="pmA")
                    for kc in range(2):
                        nc.tensor.matmul(
                            out=pq[:, 0:500],
                            lhsT=W["wqkvT"][:, kc, j * 128:(j + 1) * 128],
                            rhs=xn[:, kc, cs:cs + 500],
                            start=(kc == 0), stop=(kc == 1))
                    if j >= 2:
                        nc.vector.tensor_copy(qkv[:, j, 0:500], pq[:, 0:500])
                    else:
                        nc.scalar.activation(out=qkv[:, j, 0:500],
                                             in_=pq[:, 0:500], func=AF.Copy)
                # per-key exp-bias rows c_h = wc_h . zhat (extra qkv outputs)
                pq8 = pmA.tile([128, 512], f32, tag="pmA")
                for kc in range(2):
                    nc.tensor.matmul(out=pq8[0:8, 0:500],
                                     lhsT=W["wqkvT"][:, kc, 768:776],
                                     rhs=xn[:, kc, cs:cs + 500],
                                     start=(kc == 0), stop=(kc == 1))
                nc.vector.tensor_copy(cqS[0:8, 0:500], pq8[0:8, 0:500])
                # V^T via SBUF->SBUF DMA transpose (128-key blocks)
                vts = []
                for j in range(2):
                    vt = scr2.tile([128, 4, 128], bf16, tag=f"vt{j}{s % 2}")
                    for skc in range(4):
                        nc.sync.dma_start_transpose(
                            out=vt[:, skc, :],
                            in_=qkv[:, 4 + j, 128 * skc:128 * skc + 128])
                    vts.append(vt)
                # c-bias transposed into per-key layout: esT cols 32..64
                esT = pmS.tile([128, 512], f32, tag="pmS")
                pot0 = pmP.tile([128, 512], f32, tag="pmP")
                pot1 = pmB.tile([128, 512], f32, tag="potB")
                pots = [pot0, pot1]
                for skc in range(4):
                    nc.tensor.transpose(out=esT[:, 32 + 8 * skc:40 + 8 * skc],
                                        in_=cqS[0:8, 128 * skc:128 * skc + 128],
                                        identity=identF[:8, 0:8])
                cbS = scr.tile([128, 4, 8], f32, tag="cbS")
                nc.vector.tensor_copy(cbS[:], esT[:, 32:64])
                def attn_tail(j):
                    rrT = sc.tile([125, 16], f32, tag=f"rrT{j}")
                    nc.vector.reciprocal(out=rrT[:], in_=esT[:125, 16 * j:16 * j + 16])
                    pcol = 64 + 128 * j
                    nc.tensor.transpose(out=esT[0:16, pcol:pcol + 125], in_=rrT[:],
                                        identity=identF[:125, 0:125])
                    rrTT = sc.tile([16, 125], bf16, tag=f"rrTT{j}")
                    with nc.allow_low_precision("softmax recip bcast bf16"):
                        nc.vector.tensor_copy(rrTT[:], esT[0:16, pcol:pcol + 125])
                    prb = pmA.tile([128, 512], f32, tag="pmA")
                    for qb in range(4):
                        nc.tensor.matmul(out=prb[:, 125 * qb:125 * qb + 125],
                                         lhsT=C["indTc"][:, qb, :], rhs=rrTT[:],
                                         start=True, stop=True)
                    prbS = scr2.tile([128, 500], bf16, tag=f"prbS{j}")
                    nc.vector.tensor_copy(prbS[:], prb[:, 0:500])
                    nc.vector.tensor_tensor(out=attn[:, j, cs:cs + 500],
                                            in0=pots[j][:, 0:500], in1=prbS[:],
                                            op=A.mult)
                eTs = {}
                for t in range(len(UNITS) + LAG):
                    if t < len(UNITS):
                        c4, j = UNITS[t]
                        poff = 32 * c4
                        eT = scr.tile([128, 4, 500], bf16, tag=f"eT{t % 5}")
                        eTs[t] = eT
                        for skc in range(4):
                            psc = pmA.tile([128, 512], f32, tag="pmA")
                            nc.tensor.matmul(
                                out=psc[:, 0:500],
                                lhsT=qkv[poff:poff + 32, 2 + j,
                                         128 * skc:128 * skc + 128],
                                rhs=qkv[poff:poff + 32, j, 0:500],
                                start=True, stop=True,
                                tile_position=(poff, 0))
                            nc.scalar.activation(
                                out=eT[:, skc, :], in_=psc[:, 0:500],
                                func=AF.Exp,
                                bias=cbS[:, skc, 4 * j + c4:4 * j + c4 + 1])
                    if t >= LAG:
                        c4, j = UNITS[t - LAG]
                        poff = 32 * c4
                        eT = eTs.pop(t - LAG)
                        for skc in range(4):
                            nc.tensor.matmul(out=pots[j][poff:poff + 32, 0:500],
                                             lhsT=vts[j][:, skc, poff:poff + 32],
                                             rhs=eT[:, skc, :],
                                             start=(skc == 0), stop=(skc == 3),
                                             tile_position=(0, poff))
                        # esum: accumulate the 4 key blocks directly in PSUM
                        # (out free size 1 -> ~free on PE) instead of a DVE
                        # tree-add of eT.
                        for qb in range(4):
                            for skc in range(4):
                                nc.tensor.matmul(
                                    out=esT[:125, 16 * j + 4 * c4 + qb:
                                            16 * j + 4 * c4 + qb + 1],
                                    lhsT=eT[:, skc, 125 * qb:125 * qb + 125],
                                    rhs=onesPb[:, 0:1],
                                    start=(skc == 0), stop=(skc == 3))
                        if t - LAG == 3:
                            attn_tail(0)
                        elif t - LAG == 7:
                            attn_tail(1)
                # Wo + residual for this sample (bias via 1-row matmul)
                for mb in range(2):
                    po = pmA.tile([128, 512], f32, tag="pmA")
                    for kc in range(2):
                        nc.tensor.matmul(
                            out=po[:, 0:500],
                            lhsT=W["woT"][:, kc, mb * 128:(mb + 1) * 128],
                            rhs=attn[:, kc, cs:cs + 500],
                            start=(kc == 0), stop=False)
                    nc.tensor.matmul(out=po[:, 0:500],
                                     lhsT=W["boW"][:, mb * 128:(mb + 1) * 128],
                                     rhs=ones500b[:], start=False, stop=True)
                    nc.vector.tensor_tensor(out=x[:, mb, cs:cs + 500],
                                            in0=x[:, mb, cs:cs + 500],
                                            in1=po[:, 0:500], op=A.add)
                if s >= 1:
                    emit_ln_finish(xn2, s - 1, lnst.pop(s - 1))
            emit_ln(xn2, ns_run - 1)
            xn_next = big.tile([128, 2, 2000], bf16,
                               tag=("xnA" if (i + 1) % 2 == 0 else "xnB"))
            st4 = None
            if i + 1 < nl_run:
                st4 = scr2.tile([1, 2, 4, 500], f32, tag="lnsb4",
                                name="lnsb4")
            for s in range(ns_run):
                cs = s * 500
                if i + 1 < nl_run and s >= 1:
                    emit_ln_stats_sb(s - 1, st4)
                fh = scr.tile([128, 8, 500], bf16, tag=f"fh{s % 2}")
                for hb in range(8):
                    phh = pmA.tile([128, 512], f32, tag="pmA")
                    for kc in range(2):
                        nc.tensor.matmul(
                            out=phh[:, 0:500],
                            lhsT=W["w1TT"][:, kc, hb * 128:(hb + 1) * 128],
                            rhs=xn2[:, kc, cs:cs + 500], start=(kc == 0),
                            stop=(kc == 1))
                    nc.scalar.activation(out=fh[:, hb, :], in_=phh[:, 0:500],
                                         func=AF.Gelu, bias=W["b1v"][:, hb:hb + 1])
                for mb in range(2):
                    pf = pmA.tile([128, 512], f32, tag="pmA")
                    for hb in range(8):
                        nc.tensor.matmul(
                            out=pf[:, 0:500],
                            lhsT=W["w2TT"][:, hb, mb * 128:(mb + 1) * 128],
                            rhs=fh[:, hb, :], start=(hb == 0), stop=False)
                    nc.tensor.matmul(out=pf[:, 0:500],
                                     lhsT=W["b2W"][:, mb * 128:(mb + 1) * 128],
                                     rhs=ones500b[:], start=False, stop=True)
                    nc.vector.tensor_tensor(out=x[:, mb, cs:cs + 500],
                                            in0=x[:, mb, cs:cs + 500],
                                            in1=pf[:, 0:500], op=A.add)
            if i + 1 < nl_run:
                emit_ln_stats_sb(ns_run - 1, st4)
                ru4 = emit_ln_ru4(st4)
                emit_ln_apply(xn_next, 0, ru4)
                pend_ln = (xn_next, ru4)
            if i + 2 < nl_run:
                LWslot[i % 2] = load_layer_weights(i % 2, i + 2)
            xn = xn_next

        # ================= pooling + head =================
        for s in range(ns_run):
            cs = s * 500
            plg = pmS.tile([128, 512], f32, tag="pmS")
            for blk in range(2):
                nc.tensor.matmul(out=plg[0:1, 0:500],
                                 lhsT=C["poolq"][:, blk:blk + 1],
                                 rhs=x[:, blk, cs:cs + 500], start=(blk == 0),
                                 stop=(blk == 1))
            wrow = sc.tile([1, 500], f32, tag="wrow")
            nc.scalar.activation(out=wrow[:], in_=plg[0:1, 0:500], func=AF.Exp,
                                 scale=1.0 / 16.0)
            pwb = pmA.tile([128, 512], f32, tag="pmA")
            nc.tensor.matmul(out=pwb[:, 0:500], lhsT=ones1f[:], rhs=wrow[:],
                             start=True, stop=True)
            wx = scr2.tile([128, 2, 500], f32, tag="wx")
            for blk in range(2):
                nc.vector.tensor_tensor(out=wx[:, blk, :], in0=x[:, blk, cs:cs + 500],
                                        in1=pwb[:, 0:500], op=A.mult)
            pooled = sc.tile([128, 4], f32, tag="pooled")
            nc.vector.tensor_reduce(out=pooled[:, 0:2], in_=wx[:], axis=AX.X,
                                    op=A.add)
            nc.scalar.activation(out=pooled[:, 2:4], in_=pooled[:, 0:2],
                                 func=AF.Square)
            pps = pmS.tile([128, 512], f32, tag="pmS")
            nc.tensor.matmul(out=pps[0:1, 0:4], lhsT=onesPf[:], rhs=pooled[:],
                             start=True, stop=True)
            z = sc.tile([1, 16], f32, tag="z")
            nc.vector.tensor_copy(z[:, 12:16], pps[0:1, 0:4])
            nc.vector.tensor_tensor(out=z[:, 0:1], in0=z[:, 12:13],
                                    in1=z[:, 13:14], op=A.add)
            nc.vector.tensor_tensor(out=z[:, 1:2], in0=z[:, 14:15],
                                    in1=z[:, 15:16], op=A.add)
            nc.vector.tensor_scalar(out=z[:, 2:3], in0=z[:, 0:1],
                                    scalar1=1.0 / 256.0, scalar2=None, op0=A.mult)
            nc.vector.tensor_tensor(out=z[:, 3:4], in0=z[:, 2:3], in1=z[:, 2:3],
                                    op=A.mult)
            nc.vector.tensor_scalar(out=z[:, 4:5], in0=z[:, 1:2],
                                    scalar1=1.0 / 256.0, scalar2=None, op0=A.mult)
            nc.vector.tensor_tensor(out=z[:, 4:5], in0=z[:, 4:5], in1=z[:, 3:4],
                                    op=A.subtract)
            nc.scalar.activation(out=z[:, 5:6], in_=z[:, 4:5], func=AF.Ln,
                                 bias=epsv[0:1, :])
            nc.scalar.activation(out=z[:, 6:7], in_=z[:, 5:6], func=AF.Exp,
                                 scale=-0.5)
            cp = sc.tile([128, 2], f32, tag="cp")
            nc.vector.tensor_tensor(out=cp[:], in0=pooled[:, 0:2], in1=C["c1v"][:],
                                    op=A.mult)
            pa = pmS.tile([128, 512], f32, tag="pmS")
            nc.tensor.matmul(out=pa[0:1, 0:2], lhsT=onesPf[:], rhs=cp[:],
                             start=True, stop=True)
            nc.vector.tensor_copy(z[:, 10:12], pa[0:1, 0:2])
            nc.vector.tensor_tensor(out=z[:, 7:8], in0=z[:, 10:11],
                                    in1=z[:, 11:12], op=A.add)
            nc.vector.tensor_scalar(out=z[:, 8:9], in0=z[:, 2:3], scalar1=sc1,
                                    scalar2=None, op0=A.mult)
            nc.vector.tensor_tensor(out=z[:, 8:9], in0=z[:, 7:8], in1=z[:, 8:9],
                                    op=A.subtract)
            nc.vector.tensor_tensor(out=z[:, 8:9], in0=z[:, 8:9], in1=z[:, 6:7],
                                    op=A.mult)
            nc.vector.tensor_scalar(out=z[:, 9:10], in0=z[:, 8:9], scalar1=c2s,
                                    scalar2=None, op0=A.add)
            nc.sync.dma_start(out=Y[s:s + 1, :], in_=z[:, 9:10])

        for p in [pmS, pmP, pmB, pmA, scr2, scr, big, wpool, bigx, sc, sing]:
            p.release()

    nc.compile()
    return nc


_BUILT = {}


def _get_nc(key, **kw):
    if key not in _BUILT:
        _BUILT[key] = _build(**kw)
    return _BUILT[key]


def _make_in_maps(prep, ns=NS, ncores=NCORES):
    in_maps = []
    for c in range(ncores):
        sl = slice(c * ns, (c + 1) * ns)
        m = {k: prep[k] for k in CONST_KEYS}
        m["mel4"] = prep["_mel"][sl]
        m["evrows"] = prep["_evrows"][sl]
        m["tposv"] = prep["_tposv"][sl]
        m["keepv"] = prep["_keepv"][sl]
        m["starbias"] = prep["_starbias"][sl]
        in_maps.append(m)
    return in_maps


def kernel(**inputs):
    from concourse.bass_utils import run_bass_kernel_spmd

    prep = _host_prep(inputs)
    nc = _get_nc("full", nl_run=NLAYERS, ns_run=NS, debug=False,
                 sc1=prep["sc1"], c2s=prep["c2s"])
    res = run_bass_kernel_spmd(nc, _make_in_maps(prep), list(range(NCORES)))
    y = np.concatenate([res.results[c]["y"].reshape(-1) for c in range(NCORES)])
    return y.astype(np.float32)



# revision 50
# speedup vs baseline: 1.0632x; 1.0632x over previous
"""Trainium2 Bass kernel for nn_ChartQualityEvaluator (bf16 rewrite).

Data parallel: 32 samples -> 8 cores x 4 samples. Feature-major activations
[128 part, 2 blocks, 4*500 cols], bf16 residual stream, all matmuls bf16
(1 cyc/row vs 4 for fp32). Host folds LN gain/bias into adjacent weight
matrices, q/k biases into a per-key exp-bias column (softmax shift
invariance), and the v bias into Wo's bias. Softmax esum rides a DVE
tree-add + 1-row matmuls; normalization is batched per 4 heads.
"""
import math
import sys

import numpy as np

_TRN = "/opt/trn_rl_repo"
if _TRN not in sys.path:
    sys.path.insert(0, _TRN)

import ml_dtypes

BF16 = ml_dtypes.bfloat16

D = 256
H = 8
NLAYERS = 6
HALF = 128
S = 500
NEV = 256
NCORES = 8
NS = 4  # samples per core
B = 32
EPS = 1e-5
INV2PI = float(np.float32(1.0 / (2.0 * math.pi)))
TWOPI = 2.0 * math.pi
SC32 = 1.0 / math.sqrt(32.0)


def _host_prep(inp):
    f = np.float32
    f8 = np.float64
    out = {}

    def t2(v):  # [256] -> [128,2] fp32
        return np.ascontiguousarray(np.asarray(v).reshape(2, 128).T.astype(f))

    def b(a):  # to bf16
        return np.ascontiguousarray(np.asarray(a).astype(f).astype(BF16))

    # ---------------- frontend ----------------
    out["wmelT"] = b(np.asarray(inp["mel_W"]).T)
    out["melb"] = np.ascontiguousarray(np.asarray(inp["mel_b"]).reshape(16, 1).astype(f))
    out["w1t"] = b(np.asarray(inp["conv1_w"]).transpose(1, 2, 0))
    out["c1b"] = np.ascontiguousarray(np.asarray(inp["conv1_b"]).reshape(128, 1).astype(f))
    out["gng"] = np.ascontiguousarray(np.asarray(inp["gn_g"]).reshape(128, 1).astype(f))
    out["gnb"] = np.ascontiguousarray(np.asarray(inp["gn_b"]).reshape(128, 1).astype(f))
    out["w2t"] = b(np.asarray(inp["conv2_w"]).transpose(1, 2, 0))
    out["c2b"] = t2(inp["conv2_b"])
    out["cng"] = t2(inp["cn_g"])

    freq = np.exp(np.arange(HALF, dtype=f) * f(-math.log(10000.0) / (HALF - 1)))
    e32 = (np.arange(S, dtype=f)[None, :] * freq[:, None]).astype(f)
    e64 = e32.astype(np.float64)
    pos_fm = np.concatenate([np.sin(e64), np.cos(e64)], axis=0)  # [256,500]
    out["posT"] = np.ascontiguousarray(
        pos_fm.reshape(2, 128, S).transpose(1, 0, 2).astype(f))
    out["freqv"] = np.ascontiguousarray(freq.reshape(128, 1))

    out["epW1T"] = np.ascontiguousarray(
        np.asarray(inp["ep_W1"]).T.reshape(6, 128, 256).transpose(1, 0, 2).astype(f).astype(BF16))
    out["epb1"] = t2(inp["ep_b1"])
    out["epW2T"] = np.ascontiguousarray(
        np.asarray(inp["ep_W2"]).T.reshape(2, 128, 256).transpose(1, 0, 2).astype(f).astype(BF16))
    out["epb2row"] = np.ascontiguousarray(
        np.tile(np.asarray(inp["ep_b2"]).astype(f)[None, :], (128, 1)))

    def wT(w, kc, m, dt=BF16):  # w [m, k] -> [128, kc, m]
        return np.ascontiguousarray(
            np.asarray(w).astype(f).T.reshape(kc, 128, m).transpose(1, 0, 2).astype(dt))

    # ---------------- transformer with LN folding ----------------
    wqkvT, woT, boW, w1TT, b1v, w2TT, b2W = [], [], [], [], [], [], []
    for i in range(NLAYERS):
        g1 = np.asarray(inp["tl_ln1g"][i]).astype(f8)
        b1 = np.asarray(inp["tl_ln1b"][i]).astype(f8)
        g2 = np.asarray(inp["tl_ln2g"][i]).astype(f8)
        b2 = np.asarray(inp["tl_ln2b"][i]).astype(f8)
        Wqkv = np.asarray(inp["tl_Wqkv"][i]).astype(f8)   # [768, 256]
        bqkv = np.asarray(inp["tl_bqkv"][i]).astype(f8)   # [768]
        Wo = np.asarray(inp["tl_Wo"][i]).astype(f8)       # [256, 256]
        bo = np.asarray(inp["tl_bo"][i]).astype(f8)
        W1 = np.asarray(inp["tl_W1"][i]).astype(f8)       # [1024, 256]
        b1f = np.asarray(inp["tl_b1"][i]).astype(f8)
        W2 = np.asarray(inp["tl_W2"][i]).astype(f8)       # [256, 1024]
        b2f = np.asarray(inp["tl_b2"][i]).astype(f8)

        Wq, Wk, Wv = Wqkv[0:256], Wqkv[256:512], Wqkv[512:768]
        Wq_f = Wq * g1[None, :]
        Wk_f = Wk * g1[None, :] * SC32
        Wv_f = Wv * g1[None, :]
        bq_eff = Wq @ b1 + bqkv[0:256]          # survives as per-key exp bias
        bv_eff = Wv @ b1 + bqkv[512:768]        # folds into Wo bias
        # wc_h: c_h[k] = bq_eff_h . k''_h[:,k] = (Wk_f[h].T @ bq_eff[h]) . zhat
        WC = np.stack([Wk_f[32 * h:32 * h + 32].T @ bq_eff[32 * h:32 * h + 32]
                       for h in range(8)])      # [8, 256]
        W_f = np.concatenate([Wq_f, Wk_f, Wv_f, WC], axis=0)  # [776, 256]
        wqkvT.append(wT(W_f, 2, 776))
        bo_eff = bo + Wo @ bv_eff
        woT.append(wT(Wo, 2, 256))
        boW.append(t2(bo_eff.astype(f)))
        W1_f = W1 * g2[None, :]
        b1_eff = W1 @ b2 + b1f
        w1TT.append(wT(W1_f, 2, 1024))
        b1v.append(np.ascontiguousarray(
            b1_eff.astype(f).reshape(8, 128).T.astype(f)))     # [128, 8] fp32
        w2TT.append(wT(W2, 8, 256))
        b2W.append(t2(b2f.astype(f)))
    out["wqkvT"] = np.stack(wqkvT)
    out["woT"] = np.stack(woT)
    out["boW"] = np.stack(boW)
    out["w1TT"] = np.stack(w1TT)
    out["b1v"] = np.stack(b1v)
    out["w2TT"] = np.stack(w2TT)
    out["b2W"] = np.stack(b2W)

    indT = np.zeros((16, 4, 128), np.float32)
    for qb in range(4):
        for p in range(128):
            indT[4 * (p // 32) + qb, qb, p] = 1.0
    out["indTc"] = np.ascontiguousarray(indT.astype(BF16))

    # ---------------- pooling head ----------------
    out["poolq"] = np.ascontiguousarray(
        np.asarray(inp["pool_q"]).astype(f).reshape(2, 128).T)
    c1 = (np.asarray(inp["oh_W"])[0] * np.asarray(inp["on_g"])).astype(f)
    out["c1v"] = t2(c1)
    out["sc1"] = float(c1.astype(np.float64).sum())
    out["c2s"] = float((np.asarray(inp["oh_W"])[0].astype(np.float64)
                        * np.asarray(inp["on_b"]).astype(np.float64)).sum()
                       + float(np.asarray(inp["oh_b"])[0]))

    # ---------------- per-sample host precompute ----------------
    events = np.asarray(inp["events"]).astype(np.int64)
    mask = np.asarray(inp["event_mask"])
    star = np.asarray(inp["star_rating"]).astype(f)
    nb = events.shape[0]
    diff = np.maximum(events[:, 1:] - events[:, :-1], 1)
    g = np.concatenate([diff[:, :1], diff], axis=1)
    gap_ms = (g * 5).astype(f)
    g_f = np.maximum(g.astype(f), f(1.0))
    r = np.clip(g_f[:, 1:] / g_f[:, :-1], f(0.1), f(10.0)).astype(f)
    ones = np.ones((nb, 1), f)
    rb50 = np.trunc(np.concatenate([ones, r], axis=1) * f(50.0)).astype(f)
    ra50 = np.trunc(np.concatenate([r, ones], axis=1) * f(50.0)).astype(f)
    out["_evrows"] = np.ascontiguousarray(np.stack([rb50, ra50, gap_ms], axis=1))
    tp = np.clip(events // 4, 0, S - 1).astype(f)
    keep = (1.0 - mask.astype(f)).astype(f)
    out["_tposv"] = np.ascontiguousarray(tp.reshape(nb, 2, 128).transpose(0, 2, 1))
    out["_keepv"] = np.ascontiguousarray(keep.reshape(nb, 2, 128).transpose(0, 2, 1))
    bucket = np.clip((star / f(0.5)).astype(np.int32), 0, 19)
    sb = (np.asarray(inp["cn_b"])[None, :] + np.asarray(inp["star_table"])[bucket]).astype(f)
    out["_starbias"] = np.ascontiguousarray(sb.reshape(nb, 2, 128).transpose(0, 2, 1))
    out["_mel"] = np.asarray(inp["mel"]).astype(f).astype(BF16)
    return out


CONST_KEYS = ["wmelT", "melb", "w1t", "c1b", "gng", "gnb", "w2t", "c2b", "cng",
              "posT", "freqv", "epW1T", "epb1", "epW2T", "epb2row",
              "wqkvT", "woT", "boW", "w1TT", "b1v", "w2TT", "b2W",
              "poolq", "c1v", "indTc"]


def _build(nl_run=NLAYERS, ns_run=NS, debug=False, sc1=0.0, c2s=0.0):
    import concourse.bacc as bacc
    import concourse.tile as tile
    from concourse import mybir
    from concourse.masks import make_identity

    class _Bacc(bacc.Bacc):
        # Restrict activation-table choice so ln/exp (and everything the
        # kernel needs besides gelu/sin) resolve to one shared table --
        # avoids a table reload on every LayerNorm. Table ids stay
        # positional, so unused entries are blanked rather than removed.
        _KEEP = {"natural_log_exp_and_others", "trig_and_small",
                 "gelu_and_others", "sqrt_and_others"}

        def insert_act_table_loads(self):
            from concourse.hw_specs import get_activation_tables
            import bass_rust as _bass_rust
            has_activation = any(
                isinstance(i, mybir.InstActivation)
                for b in self.main_func.blocks
                for i in b.instructions
            )
            if not has_activation:
                return
            tables = [
                (name, funcs if name in self._KEEP else set())
                for name, funcs in get_activation_tables(self.m.arch).items()
            ]
            _bass_rust.insert_act_table_loads(self, tables)

    f32 = mybir.dt.float32
    bf16 = mybir.dt.bfloat16
    f32r = mybir.dt.float32r
    f8 = mybir.dt.float8e4
    A = mybir.AluOpType
    AF = mybir.ActivationFunctionType
    AX = mybir.AxisListType
    DR = mybir.MatmulPerfMode.DoubleRow

    def r32(ap):
        # fp32 matmul streams at 4 cyc/row; f32r at 1 (ap>=256). Same bytes.
        return ap.bitcast(f32r)

    nc = _Bacc(None)
    P = {}
    shapes = dict(
        mel4=([ns_run, 80, 2000], bf16),
        evrows=([ns_run, 3, 256], f32r),
        tposv=([ns_run, 128, 2], f32),
        keepv=([ns_run, 128, 2], f32),
        starbias=([ns_run, 128, 2], f32),
        wmelT=([80, 16], bf16), melb=([16, 1], f32),
        w1t=([16, 7, 128], bf16), c1b=([128, 1], f32),
        gng=([128, 1], f32), gnb=([128, 1], f32),
        w2t=([128, 7, 256], bf16), c2b=([128, 2], f32),
        cng=([128, 2], f32), posT=([128, 2, 500], f32), freqv=([128, 1], f32),
        epW1T=([128, 6, 256], bf16), epb1=([128, 2], f32),
        epW2T=([128, 2, 256], bf16), epb2row=([128, 256], f32),
        wqkvT=([NLAYERS, 128, 2, 776], bf16),
        woT=([NLAYERS, 128, 2, 256], bf16), boW=([NLAYERS, 128, 2], f32),
        w1TT=([NLAYERS, 128, 2, 1024], bf16), b1v=([NLAYERS, 128, 8], f32),
        w2TT=([NLAYERS, 128, 8, 256], bf16), b2W=([NLAYERS, 128, 2], f32),
        poolq=([128, 2], f32r), c1v=([128, 2], f32),
        indTc=([16, 4, 128], bf16),
    )
    for k, (sh, dt) in shapes.items():
        P[k] = nc.declare_dram_parameter(k, sh, dt, isOutput=False)
    Y = nc.declare_dram_parameter("y", [ns_run, 1], f32, isOutput=True)
    DBG = None
    if debug:
        DBG = nc.declare_dram_parameter("dbg_x0", [128, 2, 2000], f32, isOutput=True)

    with tile.TileContext(nc) as tc:
        sing = tc.alloc_tile_pool(name="sing", bufs=1)
        sc = tc.alloc_tile_pool(name="sc", bufs=2)
        bigx = tc.alloc_tile_pool(name="bigx", bufs=1)
        wpool = tc.alloc_tile_pool(name="wpool", bufs=1)
        # PSUM pools: pmA rotating 1-bank x3, pmB 2-bank, pmP 1-bank,
        # pmS 1-bank (stats+esum), pmT small bf16 transpose staging
        pmA = tc.alloc_tile_pool(name="pmA", bufs=3, space="PSUM")
        pmB = tc.alloc_tile_pool(name="pmB", bufs=1, space="PSUM")
        pmP = tc.alloc_tile_pool(name="pmP", bufs=1, space="PSUM")
        pmS = tc.alloc_tile_pool(name="pmS", bufs=1, space="PSUM")

        fc = tc.alloc_tile_pool(name="fc", bufs=1)
        _FRONT = ["wmelT", "melb", "w1t", "c1b", "gng", "gnb", "w2t", "c2b",
                  "cng", "posT", "freqv", "epW1T", "epb1", "epW2T", "epb2row"]
        C = {}
        for k in _FRONT + ["poolq", "c1v", "indTc"]:
            pool = fc if k in _FRONT else sing
            C[k] = pool.tile(shapes[k][0], shapes[k][1], tag=k, name="c_" + k)
            nc.sync.dma_start(out=C[k][:], in_=P[k][:])
        # transformer weights stream through 2 SBUF slots (DMA is nearly
        # idle); slot for layer i+2 refills while layer i+1 runs
        WSPECS = [("wqkvT", [128, 2, 776], bf16),
                  ("woT", [128, 2, 256], bf16),
                  ("boW", [128, 2], f32),
                  ("w1TT", [128, 2, 1024], bf16),
                  ("b1v", [128, 8], f32),
                  ("w2TT", [128, 8, 256], bf16),
                  ("b2W", [128, 2], f32)]

        def load_layer_weights(slot, i):
            W = {}
            for k, sh, dt in WSPECS:
                W[k] = wpool.tile(sh, dt, tag=f"w{slot}_{k}",
                                  name=f"w{slot}_{k}")
                nc.sync.dma_start(out=W[k][:], in_=P[k][i])
            return W

        LWslot = [load_layer_weights(0, 0)]
        if nl_run > 1:
            LWslot.append(load_layer_weights(1, 1))

        identF = sing.tile([128, 128], f32, tag="identF")
        make_identity(nc, identF[:])
        onesPb = sing.tile([128, 1], bf16, tag="onesPb")      # plain ones bf16
        nc.vector.memset(onesPb[:], 1.0)
        onesP8 = sing.tile([128, 1], f8, tag="onesP8")        # ones fp8e4
        nc.vector.memset(onesP8[:], 1.0)
        onesMb = sing.tile([128, 1], bf16, tag="onesMb")      # 1/256 (stats lhsT)
        nc.vector.memset(onesMb[:], 1.0 / 256.0)
        onesMf = sing.tile([128, 1], f32r, tag="onesMf")      # 1/256 f32r
        _mfs = sing.tile([128, 1], f32, tag="_mfs")
        nc.vector.memset(_mfs[:], 1.0 / 256.0)
        nc.vector.tensor_copy(onesMf[:], _mfs[:])
        ones1b = sing.tile([1, 128], bf16, tag="ones1b")
        nc.vector.memset(ones1b[:], 1.0)
        ones1f = sing.tile([1, 128], f32, tag="ones1f")
        nc.vector.memset(ones1f[:], 1.0)
        ones1r = sing.tile([1, 128], f32r, tag="ones1r")
        _1rs = sing.tile([1, 128], f32, tag="_1rs")
        nc.vector.memset(_1rs[:], 1.0)
        nc.vector.tensor_copy(ones1r[:], _1rs[:])
        ones500b = sing.tile([1, 500], bf16, tag="ones500b")
        nc.vector.memset(ones500b[:], 1.0)
        onesPf = sing.tile([128, 1], f32, tag="onesPf")
        nc.vector.memset(onesPf[:], 1.0)
        zerov = fc.tile([128, 1], f32, tag="zerov")
        nc.vector.memset(zerov[:], 0.0)
        epsv = sing.tile([128, 1], f32, tag="epsv")
        nc.vector.memset(epsv[:], EPS)
        iotaB = fc.tile([128, 500], f32, tag="iotaB")
        nc.gpsimd.iota(iotaB[:], pattern=[[1, 500]], base=0, channel_multiplier=0,
                       allow_small_or_imprecise_dtypes=True)

        # f32r residual stream: mean/pool matmuls stream at 1 cyc/row
        x = bigx.tile([128, 2, 2000], f32r, tag="x_fm")

        def s2(t):  # step-2 view of [p, n] -> [p, n//2]
            return t.rearrange("p (t s) -> p s t", s=2)[:, 0, :]

        # ================= front end =================
        fr = tc.alloc_tile_pool(name="fr", bufs=2)
        for s in range(ns_run):
            cs = s * 500
            melp = fr.tile([80, 2006], bf16, tag="melp")
            nc.vector.memset(melp[:, 0:3], 0.0)
            nc.vector.memset(melp[:, 2003:2006], 0.0)
            nc.sync.dma_start(out=melp[:, 3:2003], in_=P["mel4"][s])
            xmelp = fr.tile([16, 2006], bf16, tag="xmelp")
            nc.vector.memset(xmelp[:, 0:3], 0.0)
            nc.vector.memset(xmelp[:, 2003:2006], 0.0)
            for nch in range(4):
                pcm = pmA.tile([128, 512], f32, tag="pmA")
                nc.tensor.matmul(out=pcm[:16, 0:500], lhsT=C["wmelT"][:],
                                 rhs=melp[:, 3 + nch * 500: 3 + nch * 500 + 500],
                                 start=True, stop=True)
                nc.scalar.activation(out=xmelp[:, 3 + nch * 500: 3 + nch * 500 + 500],
                                     in_=pcm[:16, 0:500], func=AF.Identity,
                                     bias=C["melb"][:, 0:1])
            pc1 = pmB.tile([128, 2, 512], f32, tag="pmB")
            for half in range(2):
                for k in range(7):
                    nc.tensor.matmul(
                        out=pc1[:, half, 0:500], lhsT=C["w1t"][:, k, :],
                        rhs=s2(xmelp[:, k + half * 1000: k + half * 1000 + 1000]),
                        start=(k == 0), stop=(k == 6))
            h1g = fr.tile([128, 2, 500], bf16, tag="h1g")
            stg = fr.tile([128, 2], f32, tag="stg")
            nc.scalar.activation(out=h1g[:], in_=pc1[:, :, 0:500], func=AF.Gelu,
                                 bias=C["c1b"][:, 0:1], accum_out=stg[:, 0:1])
            sqf = fr.tile([128, 2, 500], bf16, tag="sqf")
            nc.scalar.activation(out=sqf[:], in_=h1g[:], func=AF.Square,
                                 accum_out=stg[:, 1:2])
            pg = pmS.tile([128, 512], f32, tag="pmS")
            nc.tensor.matmul(out=pg[:1, 0:2], lhsT=onesPf[:], rhs=stg[:],
                             start=True, stop=True)
            sn = sc.tile([1, 8], f32, tag="sn")
            nc.vector.tensor_scalar(out=sn[:, 0:2], in0=pg[:1, 0:2],
                                    scalar1=1.0 / 128000.0, scalar2=None, op0=A.mult)
            nc.vector.tensor_tensor(out=sn[:, 2:3], in0=sn[:, 0:1], in1=sn[:, 0:1],
                                    op=A.mult)
            nc.vector.tensor_tensor(out=sn[:, 3:4], in0=sn[:, 1:2], in1=sn[:, 2:3],
                                    op=A.subtract)
            nc.scalar.activation(out=sn[:, 4:5], in_=sn[:, 3:4], func=AF.Ln,
                                 bias=epsv[0:1, :])
            nc.scalar.activation(out=sn[:, 1:2], in_=sn[:, 4:5], func=AF.Exp,
                                 scale=-0.5)
            pgb = pmA.tile([128, 512], f32, tag="pmA")
            nc.tensor.matmul(out=pgb[:, 0:2], lhsT=ones1f[:], rhs=sn[:, 0:2],
                             start=True, stop=True)
            sv = sc.tile([128, 2], f32, tag="sv")
            nc.vector.tensor_tensor(out=sv[:, 0:1], in0=pgb[:, 1:2], in1=C["gng"][:],
                                    op=A.mult)
            nc.vector.tensor_tensor(out=sv[:, 1:2], in0=pgb[:, 0:1], in1=sv[:, 0:1],
                                    op=A.mult)
            nc.vector.tensor_tensor(out=sv[:, 1:2], in0=C["gnb"][:], in1=sv[:, 1:2],
                                    op=A.subtract)
            x2p = fr.tile([128, 1006], bf16, tag="x2p")
            nc.vector.memset(x2p[:, 0:3], 0.0)
            nc.vector.memset(x2p[:, 1003:1006], 0.0)
            nc.scalar.activation(out=x2p[:, 3:1003],
                                 in_=h1g.rearrange("p a b -> p (a b)"),
                                 func=AF.Identity, scale=sv[:, 0:1], bias=sv[:, 1:2])
            pc2 = pmB.tile([128, 2, 512], f32, tag="pmB")
            for mb in range(2):
                for k in range(7):
                    nc.tensor.matmul(out=pc2[:, mb, 0:500],
                                     lhsT=C["w2t"][:, k, mb * 128:(mb + 1) * 128],
                                     rhs=s2(x2p[:, k:k + 1000]),
                                     start=(k == 0), stop=(k == 6))
            for mb in range(2):
                nc.scalar.activation(out=x[:, mb, cs:cs + 500], in_=pc2[:, mb, 0:500],
                                     func=AF.Gelu, bias=C["c2b"][:, mb:mb + 1])
            # CN layernorm (stats in bf16) + cng scale + starbias + pos
            sbv = fr.tile([128, 2], f32, tag="sbv")
            nc.sync.dma_start(out=sbv[:], in_=P["starbias"][s])
            sq2 = fr.tile([128, 2, 500], bf16, tag="sqf")
            nc.vector.tensor_tensor(out=sq2[:], in0=x[:, :, cs:cs + 500],
                                    in1=x[:, :, cs:cs + 500], op=A.mult)
            pstt = pmS.tile([128, 512], f32, tag="pmS")
            for blk in range(2):
                nc.tensor.matmul(out=pstt[0:1, 0:500], lhsT=onesMf[:],
                                 rhs=x[:, blk, cs:cs + 500],
                                 start=(blk == 0), stop=(blk == 1))
            for blk in range(2):
                nc.tensor.matmul(out=pstt[32:33, 0:500], lhsT=onesMb[:],
                                 rhs=sq2[:, blk, :], start=(blk == 0), stop=(blk == 1))
            ru = sc.tile([1, 2, 500], bf16, tag="ru")
            tmp = sc.tile([1, 2, 500], f32, tag="tmp1")
            nc.scalar.activation(out=tmp[:1, 0, :], in_=pstt[0:1, 0:500],
                                 func=AF.Square)
            nc.vector.tensor_tensor(out=tmp[:1, 1, :], in0=pstt[32:33, 0:500],
                                    in1=tmp[:1, 0, :], op=A.subtract)
            nc.scalar.activation(out=tmp[:1, 0, :], in_=tmp[:1, 1, :], func=AF.Ln,
                                 bias=epsv[0:1, :])
            nc.scalar.activation(out=ru[:1, 0, :], in_=tmp[:1, 0, :],
                                 func=AF.Exp, scale=-0.5)
            nc.vector.tensor_tensor(out=ru[:1, 1, :], in0=pstt[0:1, 0:500],
                                    in1=ru[:1, 0, :], op=A.mult)
            pbc = pmB.tile([128, 2, 512], f32, tag="pmB")
            for jj in range(2):
                nc.tensor.matmul(out=pbc[:, jj, 0:500], lhsT=ones1b[:],
                                 rhs=ru[:1, jj, :], start=True, stop=True)
            for blk in range(2):
                nc.vector.tensor_tensor(out=x[:, blk, cs:cs + 500],
                                        in0=x[:, blk, cs:cs + 500],
                                        in1=pbc[:, 0, 0:500], op=A.mult)
                nc.vector.tensor_tensor(out=x[:, blk, cs:cs + 500],
                                        in0=x[:, blk, cs:cs + 500],
                                        in1=pbc[:, 1, 0:500], op=A.subtract)
                nc.scalar.activation(out=x[:, blk, cs:cs + 500],
                                     in_=x[:, blk, cs:cs + 500], func=AF.Identity,
                                     scale=C["cng"][:, blk:blk + 1],
                                     bias=sbv[:, blk:blk + 1])
            nc.vector.tensor_tensor(out=x[:, :, cs:cs + 500], in0=x[:, :, cs:cs + 500],
                                    in1=C["posT"][:], op=A.add)

            # events
            evr = fr.tile([1, 3, 256], f32r, tag="evr")
            nc.sync.dma_start(out=evr[:], in_=P["evrows"][s])
            tpv = fr.tile([128, 2], f32, tag="tpv")
            nc.sync.dma_start(out=tpv[:], in_=P["tposv"][s])
            kpv = fr.tile([128, 2], f32, tag="kpv")
            nc.sync.dma_start(out=kpv[:], in_=P["keepv"][s])
            comb = fr.tile([128, 6, 256], bf16, tag="comb")
            for vr in range(3):
                pb = pmA.tile([128, 512], f32, tag="pmA")
                nc.tensor.matmul(out=pb[:, 0:256], lhsT=ones1r[:],
                                 rhs=evr[:1, vr, :], start=True, stop=True)
                arg = fr.tile([128, 256], f32, tag="arg")
                nc.scalar.activation(out=arg[:], in_=pb[:, 0:256], func=AF.Copy,
                                     scale=C["freqv"][:])
                nc.vector.tensor_scalar(out=arg[:], in0=arg[:], scalar1=INV2PI,
                                        scalar2=None, op0=A.mult)
                w1_ = fr.tile([128, 256], f32, tag="w1_")
                ti_ = fr.tile([128, 256], mybir.dt.int32, tag="ti_")
                tf_ = fr.tile([128, 256], f32, tag="tf_")
                nc.vector.tensor_copy(ti_[:], arg[:])
                nc.vector.tensor_copy(tf_[:], ti_[:])
                nc.vector.tensor_tensor(out=w1_[:], in0=arg[:], in1=tf_[:],
                                        op=A.subtract)
                nc.scalar.activation(out=comb[:, 2 * vr, :], in_=w1_[:], func=AF.Sin,
                                     scale=TWOPI, bias=zerov[:])
                nc.vector.tensor_scalar(out=arg[:], in0=arg[:], scalar1=0.25,
                                        scalar2=None, op0=A.add)
                nc.vector.tensor_copy(ti_[:], arg[:])
                nc.vector.tensor_copy(tf_[:], ti_[:])
                nc.vector.tensor_tensor(out=w1_[:], in0=arg[:], in1=tf_[:],
                                        op=A.subtract)
                nc.scalar.activation(out=comb[:, 2 * vr + 1, :], in_=w1_[:],
                                     func=AF.Sin, scale=TWOPI, bias=zerov[:])
            hmid = fr.tile([128, 2, 256], bf16, tag="hmid")
            for mb in range(2):
                ph = pmA.tile([128, 512], f32, tag="pmA")
                for kc in range(6):
                    nc.tensor.matmul(out=ph[:, 0:256],
                                     lhsT=C["epW1T"][:, kc, mb * 128:(mb + 1) * 128],
                                     rhs=comb[:, kc, :], start=(kc == 0),
                                     stop=(kc == 5))
                nc.scalar.activation(out=hmid[:, mb, :], in_=ph[:, 0:256],
                                     func=AF.Gelu, bias=C["epb1"][:, mb:mb + 1])
            evt = fr.tile([128, 2, 256], bf16, tag="evt")
            for ec in range(2):
                pe = pmA.tile([128, 512], f32, tag="pmA")
                for kc in range(2):
                    nc.tensor.matmul(out=pe[:, 0:256],
                                     lhsT=hmid[:, kc, ec * 128:(ec + 1) * 128],
                                     rhs=C["epW2T"][:, kc, :], start=(kc == 0),
                                     stop=(kc == 1))
                nc.vector.tensor_tensor(out=evt[:, ec, :], in0=pe[:, 0:256],
                                        in1=C["epb2row"][:], op=A.add)
                nc.vector.tensor_scalar(out=evt[:, ec, :], in0=evt[:, ec, :],
                                        scalar1=kpv[:, ec:ec + 1], scalar2=None,
                                        op0=A.mult)
            oh = fr.tile([128, 2, 500], bf16, tag="oh")
            for ec in range(2):
                nc.vector.tensor_scalar(out=oh[:, ec, :], in0=iotaB[:],
                                        scalar1=tpv[:, ec:ec + 1], scalar2=None,
                                        op0=A.is_equal)
            for mb in range(2):
                px = pmA.tile([128, 512], f32, tag="pmA")
                for ec in range(2):
                    nc.tensor.matmul(out=px[:, 0:500],
                                     lhsT=evt[:, ec, mb * 128:(mb + 1) * 128],
                                     rhs=oh[:, ec, :], start=(ec == 0), stop=(ec == 1))
                nc.vector.tensor_tensor(out=x[:, mb, cs:cs + 500],
                                        in0=x[:, mb, cs:cs + 500], in1=px[:, 0:500],
                                        op=A.add)
        fr.release()
        fc.release()
        big = tc.alloc_tile_pool(name="big", bufs=1)
        scr = tc.alloc_tile_pool(name="scr", bufs=1)
        scr2 = tc.alloc_tile_pool(name="scr2", bufs=1)

        if debug:
            nc.sync.dma_start(out=DBG[:], in_=x[:])

        # ================= transformer =================
        def emit_ln_stats(s):
            # Per-column mean / E[x^2] of x(s) into rows 0/32 of a pmB tile
            # (same tile later reused for the r/u broadcast).
            co = s * 500
            sq = scr2.tile([128, 2, 500], bf16, tag="sq")
            nc.vector.tensor_tensor(out=sq[:], in0=x[:, :, co:co + 500],
                                    in1=x[:, :, co:co + 500], op=A.mult)
            pbt = pmB.tile([128, 2, 512], f32, tag="pmB")
            for blk in range(2):
                nc.tensor.matmul(out=pbt[0:1, 0, 0:500], lhsT=onesMf[:],
                                 rhs=x[:, blk, co:co + 500],
                                 start=(blk == 0), stop=(blk == 1))
            for blk in range(2):
                nc.tensor.matmul(out=pbt[32:33, 0, 0:500], lhsT=onesMb[:],
                                 rhs=sq[:, blk, :], start=(blk == 0),
                                 stop=(blk == 1))
            return pbt

        def emit_ln_finish(xn, s, pbt):
            # 1/sigma = exp(-0.5*ln(var)) keeps ACT on the exp/ln table.
            co = s * 500
            ru = sc.tile([1, 2, 500], bf16, tag="ru")
            tmp = sc.tile([1, 2, 500], f32, tag="tmp1")
            nc.scalar.activation(out=tmp[:1, 0, :], in_=pbt[0:1, 0, 0:500],
                                 func=AF.Square)
            nc.vector.tensor_tensor(out=tmp[:1, 1, :], in0=pbt[32:33, 0, 0:500],
                                    in1=tmp[:1, 0, :], op=A.subtract)
            nc.scalar.activation(out=tmp[:1, 0, :], in_=tmp[:1, 1, :],
                                 func=AF.Ln, bias=epsv[0:1, :])
            nc.scalar.activation(out=ru[:1, 0, :], in_=tmp[:1, 0, :],
                                 func=AF.Exp, scale=-0.5)
            nc.vector.tensor_tensor(out=ru[:1, 1, :], in0=pbt[0:1, 0, 0:500],
                                    in1=ru[:1, 0, :], op=A.mult)
            for jj in range(2):
                nc.tensor.matmul(out=pbt[:, jj, 0:500], lhsT=ones1b[:],
                                 rhs=ru[:1, jj, :], start=True, stop=True)
            for blk in range(2):
                nc.vector.tensor_tensor(out=xn[:, blk, co:co + 500],
                                        in0=x[:, blk, co:co + 500],
                                        in1=pbt[:, 0, 0:500], op=A.mult)
                nc.vector.tensor_tensor(out=xn[:, blk, co:co + 500],
                                        in0=xn[:, blk, co:co + 500],
                                        in1=pbt[:, 1, 0:500], op=A.subtract)

        def emit_ln(xn, s):
            emit_ln_finish(xn, s, emit_ln_stats(s))

        def emit_ln_stats_sb(s, st4):
            # Stage (mean, var) at partition 0 so the sqrt/recip finish can
            # run after the whole gelu phase (one table swap per phase).
            # Square is in every act table, so no load here.
            pbt = emit_ln_stats(s)
            m2 = sc.tile([1, 512], f32, tag="m2sc")
            nc.vector.tensor_copy(st4[0:1, 0, s, :], pbt[0:1, 0, 0:500])
            nc.scalar.activation(out=m2[:1, 0:500], in_=pbt[0:1, 0, 0:500],
                                 func=AF.Square)
            nc.vector.tensor_tensor(out=st4[0:1, 1, s, :],
                                    in0=pbt[32:33, 0, 0:500],
                                    in1=m2[:1, 0:500], op=A.subtract)

        def emit_ln_ru4(st4):
            # Batched r/u for all 4 samples (Sqrt + DVE recip: one table load
            # at the phase boundary). Reading the full st4 var plane makes
            # this depend on sample 3's stats, keeping table-based ACT work
            # off the gelu phase.
            ru4 = scr2.tile([1, 2, 4, 500], bf16, tag="ru4")
            nc.scalar.activation(out=ru4[:1, 0, :, :], in_=st4[:1, 1, :, :],
                                 func=AF.Sqrt, bias=epsv[0:1, :])
            # r = 1/sigma; u = mean*r (bf16, same precision as the inline path)
            with nc.allow_low_precision("ln r/u bf16 as inline path"):
                nc.vector.reciprocal(out=ru4[:1, 1, :, :],
                                     in_=ru4[:1, 0, :, :])
            nc.vector.tensor_tensor(out=ru4[:1, 0, :, :],
                                    in0=st4[:1, 0, :, :],
                                    in1=ru4[:1, 1, :, :], op=A.mult)
            return ru4

        def emit_ln_apply(xn, s, ru4):
            # Broadcast r/u for one sample and normalize. Emitted with one
            # sample of lookahead so the in-order PE queue never stalls on ru4.
            co = s * 500
            pbt = pmB.tile([128, 2, 512], f32, tag="pmB")
            nc.tensor.matmul(out=pbt[:, 0, 0:500], lhsT=ones1b[:],
                             rhs=ru4[:1, 1, s, :], start=True, stop=True)
            nc.tensor.matmul(out=pbt[:, 1, 0:500], lhsT=ones1b[:],
                             rhs=ru4[:1, 0, s, :], start=True, stop=True)
            for blk in range(2):
                nc.vector.tensor_tensor(out=xn[:, blk, co:co + 500],
                                        in0=x[:, blk, co:co + 500],
                                        in1=pbt[:, 0, 0:500], op=A.mult)
                nc.vector.tensor_tensor(out=xn[:, blk, co:co + 500],
                                        in0=xn[:, blk, co:co + 500],
                                        in1=pbt[:, 1, 0:500], op=A.subtract)

        UNITS = [(c4, j) for j in range(2) for c4 in range(4)]
        LAG = 4
        xn = big.tile([128, 2, 2000], bf16, tag="xnA")
        for s in range(ns_run):
            emit_ln(xn, s)
        pend_ln = None
        for i in range(nl_run):
            W = LWslot[i % 2]
            attn = big.tile([128, 2, 2000], bf16, tag="attn")
            xn2 = big.tile([128, 2, 2000], bf16, tag="xn2")
            lnst = {}
            for s in range(ns_run):
                cs = s * 500
                if pend_ln is not None and s + 1 < ns_run:
                    emit_ln_apply(pend_ln[0], s + 1, pend_ln[1])
                if s >= 1:
                    lnst[s - 1] = emit_ln_stats(s - 1)
                # double-buffered per sample so qkv(s+1) can run under the
                # units pipeline of sample s
                qkv = scr.tile([128, 6, 512], bf16, tag=f"qkv{s % 2}")
                cqS = scr.tile([8, 512], f32, tag=f"cqS{s % 2}")
                if i == 0 and s <= 1:
                    # pad keys 500..511: k''=0, v=0 -> score 0, av contrib 0
                    nc.vector.memset(qkv[:, 2:6, 500:512], 0.0)
                    # exp bias -30 at pad keys -> eT ~ 0 there
                    nc.vector.memset(cqS[:, 500:512], -30.0)
                for j in [0, 2, 4, 5, 1, 3]:
                    pq = pmA.tile([128, 512], f32, tag="pmA")
                    for kc in range(2):
                        nc.tensor.matmul(
                            out=pq[:, 0:500],
                            lhsT=W["wqkvT"][:, kc, j * 128:(j + 1) * 128],
                            rhs=xn[:, kc, cs:cs + 500],
                            start=(kc == 0), stop=(kc == 1))
                    if j >= 2:
                        nc.vector.tensor_copy(qkv[:, j, 0:500], pq[:, 0:500])
                    else:
                        nc.scalar.activation(out=qkv[:, j, 0:500],
                                             in_=pq[:, 0:500], func=AF.Copy)
                # per-key exp-bias rows c_h = wc_h . zhat (extra qkv outputs)
                pq8 = pmA.tile([128, 512], f32, tag="pmA")
                for kc in range(2):
                    nc.tensor.matmul(out=pq8[0:8, 0:500],
                                     lhsT=W["wqkvT"][:, kc, 768:776],
                                     rhs=xn[:, kc, cs:cs + 500],
                                     start=(kc == 0), stop=(kc == 1))
                nc.vector.tensor_copy(cqS[0:8, 0:500], pq8[0:8, 0:500])
                # V^T via SBUF->SBUF DMA transpose (128-key blocks)
                vts = []
                for j in range(2):
                    vt = scr2.tile([128, 4, 128], bf16, tag=f"vt{j}{s % 2}")
                    for skc in range(4):
                        nc.sync.dma_start_transpose(
                            out=vt[:, skc, :],
                            in_=qkv[:, 4 + j, 128 * skc:128 * skc + 128])
                    vts.append(vt)
                # c-bias transposed into per-key layout: esT cols 32..64
                esT = pmS.tile([128, 512], f32, tag="pmS")
                pot0 = pmP.tile([128, 512], f32, tag="pmP")
                pot1 = pmB.tile([128, 512], f32, tag="potB")
                pots = [pot0, pot1]
                for skc in range(4):
                    nc.tensor.transpose(out=esT[:, 32 + 8 * skc:40 + 8 * skc],
                                        in_=cqS[0:8, 128 * skc:128 * skc + 128],
                                        identity=identF[:8, 0:8])
                cbS = scr.tile([128, 4, 8], f32, tag="cbS")
                nc.vector.tensor_copy(cbS[:], esT[:, 32:64])
                def attn_tail(j):
                    rrT = sc.tile([125, 16], f32, tag=f"rrT{j}")
                    nc.vector.reciprocal(out=rrT[:], in_=esT[:125, 16 * j:16 * j + 16])
                    pcol = 64 + 128 * j
                    nc.tensor.transpose(out=esT[0:16, pcol:pcol + 125], in_=rrT[:],
                                        identity=identF[:125, 0:125])
                    rrTT = sc.tile([16, 125], bf16, tag=f"rrTT{j}")
                    with nc.allow_low_precision("softmax recip bcast bf16"):
                        nc.vector.tensor_copy(rrTT[:], esT[0:16, pcol:pcol + 125])
                    prb = pmA.tile([128, 512], f32, tag="pmA")
                    for qb in range(4):
                        nc.tensor.matmul(out=prb[:, 125 * qb:125 * qb + 125],
                                         lhsT=C["indTc"][:, qb, :], rhs=rrTT[:],
                                         start=True, stop=True)
                    prbS = scr2.tile([128, 500], bf16, tag=f"prbS{j}")
                    nc.vector.tensor_copy(prbS[:], prb[:, 0:500])
                    nc.vector.tensor_tensor(out=attn[:, j, cs:cs + 500],
                                            in0=pots[j][:, 0:500], in1=prbS[:],
                                            op=A.mult)
                eTs = {}
                for t in range(len(UNITS) + LAG):
                    if t < len(UNITS):
                        c4, j = UNITS[t]
                        poff = 32 * c4
                        eT = scr.tile([128, 4, 500], bf16, tag=f"eT{t % 5}")
                        eTs[t] = eT
                        for skc in range(4):
                            psc = pmA.tile([128, 512], f32, tag="pmA")
                            nc.tensor.matmul(
                                out=psc[:, 0:500],
                                lhsT=qkv[poff:poff + 32, 2 + j,
                                         128 * skc:128 * skc + 128],
                                rhs=qkv[poff:poff + 32, j, 0:500],
                                start=True, stop=True,
                                tile_position=(poff, 0))
                            nc.scalar.activation(
                                out=eT[:, skc, :], in_=psc[:, 0:500],
                                func=AF.Exp,
                                bias=cbS[:, skc, 4 * j + c4:4 * j + c4 + 1])
                    if t >= LAG:
                        c4, j = UNITS[t - LAG]
                        poff = 32 * c4
                        eT = eTs.pop(t - LAG)
                        for skc in range(4):
                            nc.tensor.matmul(out=pots[j][poff:poff + 32, 0:500],
                                             lhsT=vts[j][:, skc, poff:poff + 32],
                                             rhs=eT[:, skc, :],
                                             start=(skc == 0), stop=(skc == 3),
                                             tile_position=(0, poff))
                        # esum: accumulate the 4 key blocks directly in PSUM
                        # (out free size 1 -> ~free on PE) instead of a DVE
                        # tree-add of eT.
                        for qb in range(4):
                            for skc in range(4):
                                nc.tensor.matmul(
                                    out=esT[:125, 16 * j + 4 * c4 + qb:
                                            16 * j + 4 * c4 + qb + 1],
                                    lhsT=eT[:, skc, 125 * qb:125 * qb + 125],
                                    rhs=onesPb[:, 0:1],
                                    start=(skc == 0), stop=(skc == 3))
                        if t - LAG == 3:
                            attn_tail(0)
                        elif t - LAG == 7:
                            attn_tail(1)
                # Wo + residual for this sample (bias via 1-row matmul)
                for mb in range(2):
                    po = pmA.tile([128, 512], f32, tag="pmA")
                    for kc in range(2):
                        nc.tensor.matmul(
                            out=po[:, 0:500],
                            lhsT=W["woT"][:, kc, mb * 128:(mb + 1) * 128],
                            rhs=attn[:, kc, cs:cs + 500],
                            start=(kc == 0), stop=(kc == 1))
                    # bias folded into the residual add (saves a 500-col
                    # rank-1 matmul and shortens the psum group)
                    nc.vector.scalar_tensor_tensor(
                        out=x[:, mb, cs:cs + 500], in0=po[:, 0:500],
                        scalar=W["boW"][:, mb:mb + 1],
                        in1=x[:, mb, cs:cs + 500], op0=A.add, op1=A.add)
                if s >= 1:
                    emit_ln_finish(xn2, s - 1, lnst.pop(s - 1))
            emit_ln(xn2, ns_run - 1)
            xn_next = big.tile([128, 2, 2000], bf16,
                               tag=("xnA" if (i + 1) % 2 == 0 else "xnB"))
            st4 = None
            if i + 1 < nl_run:
                st4 = scr2.tile([1, 2, 4, 500], f32, tag="lnsb4",
                                name="lnsb4")
            for s in range(ns_run):
                cs = s * 500
                if i + 1 < nl_run and s >= 1:
                    emit_ln_stats_sb(s - 1, st4)
                fh = scr.tile([128, 8, 500], bf16, tag=f"fh{s % 2}")
                for hb in range(8):
                    phh = pmA.tile([128, 512], f32, tag="pmA")
                    for kc in range(2):
                        nc.tensor.matmul(
                            out=phh[:, 0:500],
                            lhsT=W["w1TT"][:, kc, hb * 128:(hb + 1) * 128],
                            rhs=xn2[:, kc, cs:cs + 500], start=(kc == 0),
                            stop=(kc == 1))
                    nc.scalar.activation(out=fh[:, hb, :], in_=phh[:, 0:500],
                                         func=AF.Gelu, bias=W["b1v"][:, hb:hb + 1])
                for mb in range(2):
                    pf = pmA.tile([128, 512], f32, tag="pmA")
                    for hb in range(8):
                        nc.tensor.matmul(
                            out=pf[:, 0:500],
                            lhsT=W["w2TT"][:, hb, mb * 128:(mb + 1) * 128],
                            rhs=fh[:, hb, :], start=(hb == 0), stop=(hb == 7))
                    nc.vector.scalar_tensor_tensor(
                        out=x[:, mb, cs:cs + 500], in0=pf[:, 0:500],
                        scalar=W["b2W"][:, mb:mb + 1],
                        in1=x[:, mb, cs:cs + 500], op0=A.add, op1=A.add)
            if i + 1 < nl_run:
                emit_ln_stats_sb(ns_run - 1, st4)
                ru4 = emit_ln_ru4(st4)
                emit_ln_apply(xn_next, 0, ru4)
                pend_ln = (xn_next, ru4)
            if i + 2 < nl_run:
                LWslot[i % 2] = load_layer_weights(i % 2, i + 2)
            xn = xn_next

        # ================= pooling + head =================
        for s in range(ns_run):
            cs = s * 500
            plg = pmS.tile([128, 512], f32, tag="pmS")
            for blk in range(2):
                nc.tensor.matmul(out=plg[0:1, 0:500],
                                 lhsT=C["poolq"][:, blk:blk + 1],
                                 rhs=x[:, blk, cs:cs + 500], start=(blk == 0),
                                 stop=(blk == 1))
            wrow = sc.tile([1, 500], f32r, tag="wrow")
            nc.scalar.activation(out=wrow[:], in_=plg[0:1, 0:500], func=AF.Exp,
                                 scale=1.0 / 16.0)
            pwb = pmA.tile([128, 512], f32, tag="pmA")
            nc.tensor.matmul(out=pwb[:, 0:500], lhsT=ones1r[:], rhs=wrow[:],
                             start=True, stop=True)
            wx = scr2.tile([128, 2, 500], f32, tag="wx")
            for blk in range(2):
                nc.vector.tensor_tensor(out=wx[:, blk, :], in0=x[:, blk, cs:cs + 500],
                                        in1=pwb[:, 0:500], op=A.mult)
            pooled = sc.tile([128, 4], f32, tag="pooled")
            nc.vector.tensor_reduce(out=pooled[:, 0:2], in_=wx[:], axis=AX.X,
                                    op=A.add)
            nc.scalar.activation(out=pooled[:, 2:4], in_=pooled[:, 0:2],
                                 func=AF.Square)
            pps = pmS.tile([128, 512], f32, tag="pmS")
            nc.tensor.matmul(out=pps[0:1, 0:4], lhsT=onesPf[:], rhs=pooled[:],
                             start=True, stop=True)
            z = sc.tile([1, 16], f32, tag="z")
            nc.vector.tensor_copy(z[:, 12:16], pps[0:1, 0:4])
            nc.vector.tensor_tensor(out=z[:, 0:1], in0=z[:, 12:13],
                                    in1=z[:, 13:14], op=A.add)
            nc.vector.tensor_tensor(out=z[:, 1:2], in0=z[:, 14:15],
                                    in1=z[:, 15:16], op=A.add)
            nc.vector.tensor_scalar(out=z[:, 2:3], in0=z[:, 0:1],
                                    scalar1=1.0 / 256.0, scalar2=None, op0=A.mult)
            nc.vector.tensor_tensor(out=z[:, 3:4], in0=z[:, 2:3], in1=z[:, 2:3],
                                    op=A.mult)
            nc.vector.tensor_scalar(out=z[:, 4:5], in0=z[:, 1:2],
                                    scalar1=1.0 / 256.0, scalar2=None, op0=A.mult)
            nc.vector.tensor_tensor(out=z[:, 4:5], in0=z[:, 4:5], in1=z[:, 3:4],
                                    op=A.subtract)
            nc.scalar.activation(out=z[:, 5:6], in_=z[:, 4:5], func=AF.Ln,
                                 bias=epsv[0:1, :])
            nc.scalar.activation(out=z[:, 6:7], in_=z[:, 5:6], func=AF.Exp,
                                 scale=-0.5)
            cp = sc.tile([128, 2], f32, tag="cp")
            nc.vector.tensor_tensor(out=cp[:], in0=pooled[:, 0:2], in1=C["c1v"][:],
                                    op=A.mult)
            pa = pmS.tile([128, 512], f32, tag="pmS")
            nc.tensor.matmul(out=pa[0:1, 0:2], lhsT=onesPf[:], rhs=cp[:],
                             start=True, stop=True)
            nc.vector.tensor_copy(z[:, 10:12], pa[0:1, 0:2])
            nc.vector.tensor_tensor(out=z[:, 7:8], in0=z[:, 10:11],
                                    in1=z[:, 11:12], op=A.add)
            nc.vector.tensor_scalar(out=z[:, 8:9], in0=z[:, 2:3], scalar1=sc1,
                                    scalar2=None, op0=A.mult)
            nc.vector.tensor_tensor(out=z[:, 8:9], in0=z[:, 7:8], in1=z[:, 8:9],
                                    op=A.subtract)
            nc.vector.tensor_tensor(out=z[:, 8:9], in0=z[:, 8:9], in1=z[:, 6:7],
                                    op=A.mult)
            nc.vector.tensor_scalar(out=z[:, 9:10], in0=z[:, 8:9], scalar1=c2s,
                                    scalar2=None, op0=A.add)
            nc.sync.dma_start(out=Y[s:s + 1, :], in_=z[:, 9:10])

        for p in [pmS, pmP, pmB, pmA, scr2, scr, big, wpool, bigx, sc, sing]:
            p.release()

    nc.compile()
    return nc


_BUILT = {}


def _get_nc(key, **kw):
    if key not in _BUILT:
        _BUILT[key] = _build(**kw)
    return _BUILT[key]


def _make_in_maps(prep, ns=NS, ncores=NCORES):
    in_maps = []
    for c in range(ncores):
        sl = slice(c * ns, (c + 1) * ns)
        m = {k: prep[k] for k in CONST_KEYS}
        m["mel4"] = prep["_mel"][sl]
        m["evrows"] = prep["_evrows"][sl]
        m["tposv"] = prep["_tposv"][sl]
        m["keepv"] = prep["_keepv"][sl]
        m["starbias"] = prep["_starbias"][sl]
        in_maps.append(m)
    return in_maps


def kernel(**inputs):
    from concourse.bass_utils import run_bass_kernel_spmd

    prep = _host_prep(inputs)
    nc = _get_nc("full", nl_run=NLAYERS, ns_run=NS, debug=False,
                 sc1=prep["sc1"], c2s=prep["c2s"])
    res = run_bass_kernel_spmd(nc, _make_in_maps(prep), list(range(NCORES)))
    y = np.concatenate([res.results[c]["y"].reshape(-1) for c in range(NCORES)])
    return y.astype(np.float32)



# revision 52
# speedup vs baseline: 1.0912x; 1.0263x over previous
"""Trainium2 Bass kernel for nn_ChartQualityEvaluator (bf16 rewrite).

Data parallel: 32 samples -> 8 cores x 4 samples. Feature-major activations
[128 part, 2 blocks, 4*500 cols], bf16 residual stream, all matmuls bf16
(1 cyc/row vs 4 for fp32). Host folds LN gain/bias into adjacent weight
matrices, q/k biases into a per-key exp-bias column (softmax shift
invariance), and the v bias into Wo's bias. Softmax esum rides a DVE
tree-add + 1-row matmuls; normalization is batched per 4 heads.
"""
import math
import sys

import numpy as np

_TRN = "/opt/trn_rl_repo"
if _TRN not in sys.path:
    sys.path.insert(0, _TRN)

import ml_dtypes

BF16 = ml_dtypes.bfloat16

D = 256
H = 8
NLAYERS = 6
HALF = 128
S = 500
NEV = 256
NCORES = 8
NS = 4  # samples per core
B = 32
EPS = 1e-5
INV2PI = float(np.float32(1.0 / (2.0 * math.pi)))
TWOPI = 2.0 * math.pi
SC32 = 1.0 / math.sqrt(32.0)


def _host_prep(inp):
    f = np.float32
    f8 = np.float64
    out = {}

    def t2(v):  # [256] -> [128,2] fp32
        return np.ascontiguousarray(np.asarray(v).reshape(2, 128).T.astype(f))

    def b(a):  # to bf16
        return np.ascontiguousarray(np.asarray(a).astype(f).astype(BF16))

    # ---------------- frontend ----------------
    out["wmelT"] = b(np.asarray(inp["mel_W"]).T)
    out["melb"] = np.ascontiguousarray(np.asarray(inp["mel_b"]).reshape(16, 1).astype(f))
    out["w1t"] = b(np.asarray(inp["conv1_w"]).transpose(1, 2, 0))
    out["c1b"] = np.ascontiguousarray(np.asarray(inp["conv1_b"]).reshape(128, 1).astype(f))
    out["gng"] = np.ascontiguousarray(np.asarray(inp["gn_g"]).reshape(128, 1).astype(f))
    out["gnb"] = np.ascontiguousarray(np.asarray(inp["gn_b"]).reshape(128, 1).astype(f))
    out["w2t"] = b(np.asarray(inp["conv2_w"]).transpose(1, 2, 0))
    out["c2b"] = t2(inp["conv2_b"])
    out["cng"] = t2(inp["cn_g"])

    freq = np.exp(np.arange(HALF, dtype=f) * f(-math.log(10000.0) / (HALF - 1)))
    e32 = (np.arange(S, dtype=f)[None, :] * freq[:, None]).astype(f)
    e64 = e32.astype(np.float64)
    pos_fm = np.concatenate([np.sin(e64), np.cos(e64)], axis=0)  # [256,500]
    out["posT"] = np.ascontiguousarray(
        pos_fm.reshape(2, 128, S).transpose(1, 0, 2).astype(f))
    out["freqv"] = np.ascontiguousarray(freq.reshape(128, 1))

    out["epW1T"] = np.ascontiguousarray(
        np.asarray(inp["ep_W1"]).T.reshape(6, 128, 256).transpose(1, 0, 2).astype(f).astype(BF16))
    out["epb1"] = t2(inp["ep_b1"])
    out["epW2T"] = np.ascontiguousarray(
        np.asarray(inp["ep_W2"]).T.reshape(2, 128, 256).transpose(1, 0, 2).astype(f).astype(BF16))
    out["epb2row"] = np.ascontiguousarray(
        np.tile(np.asarray(inp["ep_b2"]).astype(f)[None, :], (128, 1)))

    def wT(w, kc, m, dt=BF16):  # w [m, k] -> [128, kc, m]
        return np.ascontiguousarray(
            np.asarray(w).astype(f).T.reshape(kc, 128, m).transpose(1, 0, 2).astype(dt))

    # ---------------- transformer with LN folding ----------------
    wqkvT, woT, boW, w1TT, b1v, w2TT, b2W = [], [], [], [], [], [], []
    for i in range(NLAYERS):
        g1 = np.asarray(inp["tl_ln1g"][i]).astype(f8)
        b1 = np.asarray(inp["tl_ln1b"][i]).astype(f8)
        g2 = np.asarray(inp["tl_ln2g"][i]).astype(f8)
        b2 = np.asarray(inp["tl_ln2b"][i]).astype(f8)
        Wqkv = np.asarray(inp["tl_Wqkv"][i]).astype(f8)   # [768, 256]
        bqkv = np.asarray(inp["tl_bqkv"][i]).astype(f8)   # [768]
        Wo = np.asarray(inp["tl_Wo"][i]).astype(f8)       # [256, 256]
        bo = np.asarray(inp["tl_bo"][i]).astype(f8)
        W1 = np.asarray(inp["tl_W1"][i]).astype(f8)       # [1024, 256]
        b1f = np.asarray(inp["tl_b1"][i]).astype(f8)
        W2 = np.asarray(inp["tl_W2"][i]).astype(f8)       # [256, 1024]
        b2f = np.asarray(inp["tl_b2"][i]).astype(f8)

        Wq, Wk, Wv = Wqkv[0:256], Wqkv[256:512], Wqkv[512:768]
        Wq_f = Wq * g1[None, :]
        Wk_f = Wk * g1[None, :] * SC32
        Wv_f = Wv * g1[None, :]
        bq_eff = Wq @ b1 + bqkv[0:256]          # survives as per-key exp bias
        bv_eff = Wv @ b1 + bqkv[512:768]        # folds into Wo bias
        # wc_h: c_h[k] = bq_eff_h . k''_h[:,k] = (Wk_f[h].T @ bq_eff[h]) . zhat
        WC = np.stack([Wk_f[32 * h:32 * h + 32].T @ bq_eff[32 * h:32 * h + 32]
                       for h in range(8)])      # [8, 256]
        W_f = np.concatenate([Wq_f, Wk_f, Wv_f, WC], axis=0)  # [776, 256]
        wqkvT.append(wT(W_f, 2, 776))
        bo_eff = bo + Wo @ bv_eff
        woT.append(wT(Wo, 2, 256))
        boW.append(t2(bo_eff.astype(f)))
        W1_f = W1 * g2[None, :]
        b1_eff = W1 @ b2 + b1f
        w1TT.append(wT(W1_f, 2, 1024))
        b1v.append(np.ascontiguousarray(
            b1_eff.astype(f).reshape(8, 128).T.astype(f)))     # [128, 8] fp32
        w2TT.append(wT(W2, 8, 256))
        b2W.append(t2(b2f.astype(f)))
    out["wqkvT"] = np.stack(wqkvT)
    out["woT"] = np.stack(woT)
    out["boW"] = np.stack(boW)
    out["w1TT"] = np.stack(w1TT)
    out["b1v"] = np.stack(b1v)
    out["w2TT"] = np.stack(w2TT)
    out["b2W"] = np.stack(b2W)

    indT = np.zeros((16, 4, 128), np.float32)
    for qb in range(4):
        for p in range(128):
            indT[4 * (p // 32) + qb, qb, p] = 1.0
    out["indTc"] = np.ascontiguousarray(indT.astype(BF16))

    # ---------------- pooling head ----------------
    out["poolq"] = np.ascontiguousarray(
        np.asarray(inp["pool_q"]).astype(f).reshape(2, 128).T)
    c1 = (np.asarray(inp["oh_W"])[0] * np.asarray(inp["on_g"])).astype(f)
    out["c1v"] = t2(c1)
    out["sc1"] = float(c1.astype(np.float64).sum())
    out["c2s"] = float((np.asarray(inp["oh_W"])[0].astype(np.float64)
                        * np.asarray(inp["on_b"]).astype(np.float64)).sum()
                       + float(np.asarray(inp["oh_b"])[0]))

    # ---------------- per-sample host precompute ----------------
    events = np.asarray(inp["events"]).astype(np.int64)
    mask = np.asarray(inp["event_mask"])
    star = np.asarray(inp["star_rating"]).astype(f)
    nb = events.shape[0]
    diff = np.maximum(events[:, 1:] - events[:, :-1], 1)
    g = np.concatenate([diff[:, :1], diff], axis=1)
    gap_ms = (g * 5).astype(f)
    g_f = np.maximum(g.astype(f), f(1.0))
    r = np.clip(g_f[:, 1:] / g_f[:, :-1], f(0.1), f(10.0)).astype(f)
    ones = np.ones((nb, 1), f)
    rb50 = np.trunc(np.concatenate([ones, r], axis=1) * f(50.0)).astype(f)
    ra50 = np.trunc(np.concatenate([r, ones], axis=1) * f(50.0)).astype(f)
    out["_evrows"] = np.ascontiguousarray(np.stack([rb50, ra50, gap_ms], axis=1))
    # event sinusoids are pure functions of host data: precompute [B,128,6,256]
    argv = (out["_evrows"][:, None, :, :]
            * freq[None, :, None, None]).astype(np.float32)
    combv = np.empty((nb, HALF, 6, NEV), np.float32)
    combv[:, :, 0::2, :] = np.sin(argv)
    combv[:, :, 1::2, :] = np.cos(argv)
    out["_combv"] = np.ascontiguousarray(combv.astype(BF16))
    tp = np.clip(events // 4, 0, S - 1).astype(f)
    keep = (1.0 - mask.astype(f)).astype(f)
    out["_tposv"] = np.ascontiguousarray(tp.reshape(nb, 2, 128).transpose(0, 2, 1))
    out["_keepv"] = np.ascontiguousarray(keep.reshape(nb, 2, 128).transpose(0, 2, 1))
    bucket = np.clip((star / f(0.5)).astype(np.int32), 0, 19)
    sb = (np.asarray(inp["cn_b"])[None, :] + np.asarray(inp["star_table"])[bucket]).astype(f)
    out["_starbias"] = np.ascontiguousarray(sb.reshape(nb, 2, 128).transpose(0, 2, 1))
    out["_mel"] = np.asarray(inp["mel"]).astype(f).astype(BF16)
    return out


CONST_KEYS = ["wmelT", "melb", "w1t", "c1b", "gng", "gnb", "w2t", "c2b", "cng",
              "posT", "freqv", "epW1T", "epb1", "epW2T", "epb2row",
              "wqkvT", "woT", "boW", "w1TT", "b1v", "w2TT", "b2W",
              "poolq", "c1v", "indTc"]


def _build(nl_run=NLAYERS, ns_run=NS, debug=False, sc1=0.0, c2s=0.0):
    import concourse.bacc as bacc
    import concourse.tile as tile
    from concourse import mybir
    from concourse.masks import make_identity

    class _Bacc(bacc.Bacc):
        # Restrict activation-table choice so ln/exp (and everything the
        # kernel needs besides gelu/sin) resolve to one shared table --
        # avoids a table reload on every LayerNorm. Table ids stay
        # positional, so unused entries are blanked rather than removed.
        _KEEP = {"natural_log_exp_and_others", "trig_and_small",
                 "gelu_and_others", "sqrt_and_others"}

        def insert_act_table_loads(self):
            from concourse.hw_specs import get_activation_tables
            import bass_rust as _bass_rust
            has_activation = any(
                isinstance(i, mybir.InstActivation)
                for b in self.main_func.blocks
                for i in b.instructions
            )
            if not has_activation:
                return
            tables = [
                (name, funcs if name in self._KEEP else set())
                for name, funcs in get_activation_tables(self.m.arch).items()
            ]
            _bass_rust.insert_act_table_loads(self, tables)

    f32 = mybir.dt.float32
    bf16 = mybir.dt.bfloat16
    f32r = mybir.dt.float32r
    f8 = mybir.dt.float8e4
    A = mybir.AluOpType
    AF = mybir.ActivationFunctionType
    AX = mybir.AxisListType
    DR = mybir.MatmulPerfMode.DoubleRow

    def r32(ap):
        # fp32 matmul streams at 4 cyc/row; f32r at 1 (ap>=256). Same bytes.
        return ap.bitcast(f32r)

    nc = _Bacc(None)
    P = {}
    shapes = dict(
        mel4=([ns_run, 80, 2000], bf16),
        evrows=([ns_run, 3, 256], f32r),
        combv=([ns_run, 128, 6, 256], bf16),
        tposv=([ns_run, 128, 2], f32),
        keepv=([ns_run, 128, 2], f32),
        starbias=([ns_run, 128, 2], f32),
        wmelT=([80, 16], bf16), melb=([16, 1], f32),
        w1t=([16, 7, 128], bf16), c1b=([128, 1], f32),
        gng=([128, 1], f32), gnb=([128, 1], f32),
        w2t=([128, 7, 256], bf16), c2b=([128, 2], f32),
        cng=([128, 2], f32), posT=([128, 2, 500], f32), freqv=([128, 1], f32),
        epW1T=([128, 6, 256], bf16), epb1=([128, 2], f32),
        epW2T=([128, 2, 256], bf16), epb2row=([128, 256], f32),
        wqkvT=([NLAYERS, 128, 2, 776], bf16),
        woT=([NLAYERS, 128, 2, 256], bf16), boW=([NLAYERS, 128, 2], f32),
        w1TT=([NLAYERS, 128, 2, 1024], bf16), b1v=([NLAYERS, 128, 8], f32),
        w2TT=([NLAYERS, 128, 8, 256], bf16), b2W=([NLAYERS, 128, 2], f32),
        poolq=([128, 2], f32r), c1v=([128, 2], f32),
        indTc=([16, 4, 128], bf16),
    )
    for k, (sh, dt) in shapes.items():
        P[k] = nc.declare_dram_parameter(k, sh, dt, isOutput=False)
    Y = nc.declare_dram_parameter("y", [ns_run, 1], f32, isOutput=True)
    DBG = None
    if debug:
        DBG = nc.declare_dram_parameter("dbg_x0", [128, 2, 2000], f32, isOutput=True)

    with tile.TileContext(nc) as tc:
        sing = tc.alloc_tile_pool(name="sing", bufs=1)
        sc = tc.alloc_tile_pool(name="sc", bufs=2)
        bigx = tc.alloc_tile_pool(name="bigx", bufs=1)
        wpool = tc.alloc_tile_pool(name="wpool", bufs=1)
        # PSUM pools: pmA rotating 1-bank x3, pmB 2-bank, pmP 1-bank,
        # pmS 1-bank (stats+esum), pmT small bf16 transpose staging
        pmA = tc.alloc_tile_pool(name="pmA", bufs=3, space="PSUM")
        pmB = tc.alloc_tile_pool(name="pmB", bufs=1, space="PSUM")
        pmP = tc.alloc_tile_pool(name="pmP", bufs=1, space="PSUM")
        pmS = tc.alloc_tile_pool(name="pmS", bufs=1, space="PSUM")

        fc = tc.alloc_tile_pool(name="fc", bufs=1)
        _FRONT = ["wmelT", "melb", "w1t", "c1b", "gng", "gnb", "w2t", "c2b",
                  "cng", "posT", "freqv", "epW1T", "epb1", "epW2T", "epb2row"]
        C = {}
        for k in _FRONT + ["poolq", "c1v", "indTc"]:
            pool = fc if k in _FRONT else sing
            C[k] = pool.tile(shapes[k][0], shapes[k][1], tag=k, name="c_" + k)
            nc.sync.dma_start(out=C[k][:], in_=P[k][:])
        # transformer weights stream through 2 SBUF slots (DMA is nearly
        # idle); slot for layer i+2 refills while layer i+1 runs
        WSPECS = [("wqkvT", [128, 2, 776], bf16),
                  ("woT", [128, 2, 256], bf16),
                  ("boW", [128, 2], f32),
                  ("w1TT", [128, 2, 1024], bf16),
                  ("b1v", [128, 8], f32),
                  ("w2TT", [128, 8, 256], bf16),
                  ("b2W", [128, 2], f32)]

        def load_layer_weights(slot, i):
            W = {}
            for k, sh, dt in WSPECS:
                W[k] = wpool.tile(sh, dt, tag=f"w{slot}_{k}",
                                  name=f"w{slot}_{k}")
                nc.sync.dma_start(out=W[k][:], in_=P[k][i])
            return W

        LWslot = [load_layer_weights(0, 0)]
        if nl_run > 1:
            LWslot.append(load_layer_weights(1, 1))

        identF = sing.tile([128, 128], f32, tag="identF")
        make_identity(nc, identF[:])
        onesPb = sing.tile([128, 1], bf16, tag="onesPb")      # plain ones bf16
        nc.vector.memset(onesPb[:], 1.0)
        onesP8 = sing.tile([128, 1], f8, tag="onesP8")        # ones fp8e4
        nc.vector.memset(onesP8[:], 1.0)
        onesMb = sing.tile([128, 1], bf16, tag="onesMb")      # 1/256 (stats lhsT)
        nc.vector.memset(onesMb[:], 1.0 / 256.0)
        onesMf = sing.tile([128, 1], f32r, tag="onesMf")      # 1/256 f32r
        _mfs = sing.tile([128, 1], f32, tag="_mfs")
        nc.vector.memset(_mfs[:], 1.0 / 256.0)
        nc.vector.tensor_copy(onesMf[:], _mfs[:])
        ones1b = sing.tile([1, 128], bf16, tag="ones1b")
        nc.vector.memset(ones1b[:], 1.0)
        ones1f = sing.tile([1, 128], f32, tag="ones1f")
        nc.vector.memset(ones1f[:], 1.0)
        ones1r = sing.tile([1, 128], f32r, tag="ones1r")
        _1rs = sing.tile([1, 128], f32, tag="_1rs")
        nc.vector.memset(_1rs[:], 1.0)
        nc.vector.tensor_copy(ones1r[:], _1rs[:])
        ones500b = sing.tile([1, 500], bf16, tag="ones500b")
        nc.vector.memset(ones500b[:], 1.0)
        onesPf = sing.tile([128, 1], f32, tag="onesPf")
        nc.vector.memset(onesPf[:], 1.0)
        zerov = fc.tile([128, 1], f32, tag="zerov")
        nc.vector.memset(zerov[:], 0.0)
        epsv = sing.tile([128, 1], f32, tag="epsv")
        nc.vector.memset(epsv[:], EPS)
        iotaB = fc.tile([128, 500], f32, tag="iotaB")
        nc.gpsimd.iota(iotaB[:], pattern=[[1, 500]], base=0, channel_multiplier=0,
                       allow_small_or_imprecise_dtypes=True)

        # f32r residual stream: mean/pool matmuls stream at 1 cyc/row
        x = bigx.tile([128, 2, 2000], f32r, tag="x_fm")

        def s2(t):  # step-2 view of [p, n] -> [p, n//2]
            return t.rearrange("p (t s) -> p s t", s=2)[:, 0, :]

        # ================= front end =================
        fr = tc.alloc_tile_pool(name="fr", bufs=2)
        for s in range(ns_run):
            cs = s * 500
            melp = fr.tile([80, 2006], bf16, tag="melp")
            nc.vector.memset(melp[:, 0:3], 0.0)
            nc.vector.memset(melp[:, 2003:2006], 0.0)
            nc.sync.dma_start(out=melp[:, 3:2003], in_=P["mel4"][s])
            xmelp = fr.tile([16, 2006], bf16, tag="xmelp")
            nc.vector.memset(xmelp[:, 0:3], 0.0)
            nc.vector.memset(xmelp[:, 2003:2006], 0.0)
            for nch in range(4):
                pcm = pmA.tile([128, 512], f32, tag="pmA")
                nc.tensor.matmul(out=pcm[:16, 0:500], lhsT=C["wmelT"][:],
                                 rhs=melp[:, 3 + nch * 500: 3 + nch * 500 + 500],
                                 start=True, stop=True)
                nc.scalar.activation(out=xmelp[:, 3 + nch * 500: 3 + nch * 500 + 500],
                                     in_=pcm[:16, 0:500], func=AF.Identity,
                                     bias=C["melb"][:, 0:1])
            pc1 = pmB.tile([128, 2, 512], f32, tag="pmB")
            for half in range(2):
                for k in range(7):
                    nc.tensor.matmul(
                        out=pc1[:, half, 0:500], lhsT=C["w1t"][:, k, :],
                        rhs=s2(xmelp[:, k + half * 1000: k + half * 1000 + 1000]),
                        start=(k == 0), stop=(k == 6))
            h1g = fr.tile([128, 2, 500], bf16, tag="h1g")
            stg = fr.tile([128, 2], f32, tag="stg")
            nc.scalar.activation(out=h1g[:], in_=pc1[:, :, 0:500], func=AF.Gelu,
                                 bias=C["c1b"][:, 0:1], accum_out=stg[:, 0:1])
            sqf = fr.tile([128, 2, 500], bf16, tag="sqf")
            nc.scalar.activation(out=sqf[:], in_=h1g[:], func=AF.Square,
                                 accum_out=stg[:, 1:2])
            pg = pmS.tile([128, 512], f32, tag="pmS")
            nc.tensor.matmul(out=pg[:1, 0:2], lhsT=onesPf[:], rhs=stg[:],
                             start=True, stop=True)
            sn = sc.tile([1, 8], f32, tag="sn")
            nc.vector.tensor_scalar(out=sn[:, 0:2], in0=pg[:1, 0:2],
                                    scalar1=1.0 / 128000.0, scalar2=None, op0=A.mult)
            nc.vector.tensor_tensor(out=sn[:, 2:3], in0=sn[:, 0:1], in1=sn[:, 0:1],
                                    op=A.mult)
            nc.vector.tensor_tensor(out=sn[:, 3:4], in0=sn[:, 1:2], in1=sn[:, 2:3],
                                    op=A.subtract)
            nc.scalar.activation(out=sn[:, 4:5], in_=sn[:, 3:4], func=AF.Ln,
                                 bias=epsv[0:1, :])
            nc.scalar.activation(out=sn[:, 1:2], in_=sn[:, 4:5], func=AF.Exp,
                                 scale=-0.5)
            pgb = pmA.tile([128, 512], f32, tag="pmA")
            nc.tensor.matmul(out=pgb[:, 0:2], lhsT=ones1f[:], rhs=sn[:, 0:2],
                             start=True, stop=True)
            sv = sc.tile([128, 2], f32, tag="sv")
            nc.vector.tensor_tensor(out=sv[:, 0:1], in0=pgb[:, 1:2], in1=C["gng"][:],
                                    op=A.mult)
            nc.vector.tensor_tensor(out=sv[:, 1:2], in0=pgb[:, 0:1], in1=sv[:, 0:1],
                                    op=A.mult)
            nc.vector.tensor_tensor(out=sv[:, 1:2], in0=C["gnb"][:], in1=sv[:, 1:2],
                                    op=A.subtract)
            x2p = fr.tile([128, 1006], bf16, tag="x2p")
            nc.vector.memset(x2p[:, 0:3], 0.0)
            nc.vector.memset(x2p[:, 1003:1006], 0.0)
            nc.scalar.activation(out=x2p[:, 3:1003],
                                 in_=h1g.rearrange("p a b -> p (a b)"),
                                 func=AF.Identity, scale=sv[:, 0:1], bias=sv[:, 1:2])
            pc2 = pmB.tile([128, 2, 512], f32, tag="pmB")
            for mb in range(2):
                for k in range(7):
                    nc.tensor.matmul(out=pc2[:, mb, 0:500],
                                     lhsT=C["w2t"][:, k, mb * 128:(mb + 1) * 128],
                                     rhs=s2(x2p[:, k:k + 1000]),
                                     start=(k == 0), stop=(k == 6))
            for mb in range(2):
                nc.scalar.activation(out=x[:, mb, cs:cs + 500], in_=pc2[:, mb, 0:500],
                                     func=AF.Gelu, bias=C["c2b"][:, mb:mb + 1])
            # CN layernorm (stats in bf16) + cng scale + starbias + pos
            sbv = fr.tile([128, 2], f32, tag="sbv")
            nc.sync.dma_start(out=sbv[:], in_=P["starbias"][s])
            sq2 = fr.tile([128, 2, 500], bf16, tag="sqf")
            nc.vector.tensor_tensor(out=sq2[:], in0=x[:, :, cs:cs + 500],
                                    in1=x[:, :, cs:cs + 500], op=A.mult)
            pstt = pmS.tile([128, 512], f32, tag="pmS")
            for blk in range(2):
                nc.tensor.matmul(out=pstt[0:1, 0:500], lhsT=onesMf[:],
                                 rhs=x[:, blk, cs:cs + 500],
                                 start=(blk == 0), stop=(blk == 1))
            for blk in range(2):
                nc.tensor.matmul(out=pstt[32:33, 0:500], lhsT=onesMb[:],
                                 rhs=sq2[:, blk, :], start=(blk == 0), stop=(blk == 1))
            ru = sc.tile([1, 2, 500], bf16, tag="ru")
            tmp = sc.tile([1, 2, 500], f32, tag="tmp1")
            nc.scalar.activation(out=tmp[:1, 0, :], in_=pstt[0:1, 0:500],
                                 func=AF.Square)
            nc.vector.tensor_tensor(out=tmp[:1, 1, :], in0=pstt[32:33, 0:500],
                                    in1=tmp[:1, 0, :], op=A.subtract)
            nc.scalar.activation(out=tmp[:1, 0, :], in_=tmp[:1, 1, :], func=AF.Ln,
                                 bias=epsv[0:1, :])
            nc.scalar.activation(out=ru[:1, 0, :], in_=tmp[:1, 0, :],
                                 func=AF.Exp, scale=-0.5)
            nc.vector.tensor_tensor(out=ru[:1, 1, :], in0=pstt[0:1, 0:500],
                                    in1=ru[:1, 0, :], op=A.mult)
            pbc = pmB.tile([128, 2, 512], f32, tag="pmB")
            for jj in range(2):
                nc.tensor.matmul(out=pbc[:, jj, 0:500], lhsT=ones1b[:],
                                 rhs=ru[:1, jj, :], start=True, stop=True)
            for blk in range(2):
                nc.vector.tensor_tensor(out=x[:, blk, cs:cs + 500],
                                        in0=x[:, blk, cs:cs + 500],
                                        in1=pbc[:, 0, 0:500], op=A.mult)
                nc.vector.tensor_tensor(out=x[:, blk, cs:cs + 500],
                                        in0=x[:, blk, cs:cs + 500],
                                        in1=pbc[:, 1, 0:500], op=A.subtract)
                nc.scalar.activation(out=x[:, blk, cs:cs + 500],
                                     in_=x[:, blk, cs:cs + 500], func=AF.Identity,
                                     scale=C["cng"][:, blk:blk + 1],
                                     bias=sbv[:, blk:blk + 1])
            nc.vector.tensor_tensor(out=x[:, :, cs:cs + 500], in0=x[:, :, cs:cs + 500],
                                    in1=C["posT"][:], op=A.add)

            # events
            tpv = fr.tile([128, 2], f32, tag="tpv")
            nc.sync.dma_start(out=tpv[:], in_=P["tposv"][s])
            kpv = fr.tile([128, 2], f32, tag="kpv")
            nc.sync.dma_start(out=kpv[:], in_=P["keepv"][s])
            comb = fr.tile([128, 6, 256], bf16, tag="comb")
            nc.sync.dma_start(out=comb[:], in_=P["combv"][s])
            hmid = fr.tile([128, 2, 256], bf16, tag="hmid")
            for mb in range(2):
                ph = pmA.tile([128, 512], f32, tag="pmA")
                for kc in range(6):
                    nc.tensor.matmul(out=ph[:, 0:256],
                                     lhsT=C["epW1T"][:, kc, mb * 128:(mb + 1) * 128],
                                     rhs=comb[:, kc, :], start=(kc == 0),
                                     stop=(kc == 5))
                nc.scalar.activation(out=hmid[:, mb, :], in_=ph[:, 0:256],
                                     func=AF.Gelu, bias=C["epb1"][:, mb:mb + 1])
            evt = fr.tile([128, 2, 256], bf16, tag="evt")
            for ec in range(2):
                pe = pmA.tile([128, 512], f32, tag="pmA")
                for kc in range(2):
                    nc.tensor.matmul(out=pe[:, 0:256],
                                     lhsT=hmid[:, kc, ec * 128:(ec + 1) * 128],
                                     rhs=C["epW2T"][:, kc, :], start=(kc == 0),
                                     stop=(kc == 1))
                nc.vector.tensor_tensor(out=evt[:, ec, :], in0=pe[:, 0:256],
                                        in1=C["epb2row"][:], op=A.add)
                nc.vector.tensor_scalar(out=evt[:, ec, :], in0=evt[:, ec, :],
                                        scalar1=kpv[:, ec:ec + 1], scalar2=None,
                                        op0=A.mult)
            oh = fr.tile([128, 2, 500], bf16, tag="oh")
            for ec in range(2):
                nc.vector.tensor_scalar(out=oh[:, ec, :], in0=iotaB[:],
                                        scalar1=tpv[:, ec:ec + 1], scalar2=None,
                                        op0=A.is_equal)
            for mb in range(2):
                px = pmA.tile([128, 512], f32, tag="pmA")
                for ec in range(2):
                    nc.tensor.matmul(out=px[:, 0:500],
                                     lhsT=evt[:, ec, mb * 128:(mb + 1) * 128],
                                     rhs=oh[:, ec, :], start=(ec == 0), stop=(ec == 1))
                nc.vector.tensor_tensor(out=x[:, mb, cs:cs + 500],
                                        in0=x[:, mb, cs:cs + 500], in1=px[:, 0:500],
                                        op=A.add)
        fr.release()
        fc.release()
        big = tc.alloc_tile_pool(name="big", bufs=1)
        scr = tc.alloc_tile_pool(name="scr", bufs=1)
        scr2 = tc.alloc_tile_pool(name="scr2", bufs=1)

        if debug:
            nc.sync.dma_start(out=DBG[:], in_=x[:])

        # ================= transformer =================
        def emit_ln_stats(s):
            # Per-column mean / E[x^2] of x(s) into rows 0/32 of a pmB tile
            # (same tile later reused for the r/u broadcast).
            co = s * 500
            sq = scr2.tile([128, 2, 500], bf16, tag="sq")
            nc.vector.tensor_tensor(out=sq[:], in0=x[:, :, co:co + 500],
                                    in1=x[:, :, co:co + 500], op=A.mult)
            pbt = pmB.tile([128, 2, 512], f32, tag="pmB")
            for blk in range(2):
                nc.tensor.matmul(out=pbt[0:1, 0, 0:500], lhsT=onesMf[:],
                                 rhs=x[:, blk, co:co + 500],
                                 start=(blk == 0), stop=(blk == 1))
            for blk in range(2):
                nc.tensor.matmul(out=pbt[32:33, 0, 0:500], lhsT=onesMb[:],
                                 rhs=sq[:, blk, :], start=(blk == 0),
                                 stop=(blk == 1))
            return pbt

        def emit_ln_finish(xn, s, pbt):
            # 1/sigma = exp(-0.5*ln(var)) keeps ACT on the exp/ln table.
            co = s * 500
            ru = sc.tile([1, 2, 500], bf16, tag="ru")
            tmp = sc.tile([1, 2, 500], f32, tag="tmp1")
            nc.scalar.activation(out=tmp[:1, 0, :], in_=pbt[0:1, 0, 0:500],
                                 func=AF.Square)
            nc.vector.tensor_tensor(out=tmp[:1, 1, :], in0=pbt[32:33, 0, 0:500],
                                    in1=tmp[:1, 0, :], op=A.subtract)
            nc.scalar.activation(out=tmp[:1, 0, :], in_=tmp[:1, 1, :],
                                 func=AF.Ln, bias=epsv[0:1, :])
            nc.scalar.activation(out=ru[:1, 0, :], in_=tmp[:1, 0, :],
                                 func=AF.Exp, scale=-0.5)
            nc.vector.tensor_tensor(out=ru[:1, 1, :], in0=pbt[0:1, 0, 0:500],
                                    in1=ru[:1, 0, :], op=A.mult)
            for jj in range(2):
                nc.tensor.matmul(out=pbt[:, jj, 0:500], lhsT=ones1b[:],
                                 rhs=ru[:1, jj, :], start=True, stop=True)
            for blk in range(2):
                nc.vector.tensor_tensor(out=xn[:, blk, co:co + 500],
                                        in0=x[:, blk, co:co + 500],
                                        in1=pbt[:, 0, 0:500], op=A.mult)
                nc.vector.tensor_tensor(out=xn[:, blk, co:co + 500],
                                        in0=xn[:, blk, co:co + 500],
                                        in1=pbt[:, 1, 0:500], op=A.subtract)

        def emit_ln(xn, s):
            emit_ln_finish(xn, s, emit_ln_stats(s))

        def emit_ln_stats_sb(s, st4):
            # Stage (mean, var) at partition 0 so the sqrt/recip finish can
            # run after the whole gelu phase (one table swap per phase).
            # Square is in every act table, so no load here.
            pbt = emit_ln_stats(s)
            m2 = sc.tile([1, 512], f32, tag="m2sc")
            nc.vector.tensor_copy(st4[0:1, 0, s, :], pbt[0:1, 0, 0:500])
            nc.scalar.activation(out=m2[:1, 0:500], in_=pbt[0:1, 0, 0:500],
                                 func=AF.Square)
            nc.vector.tensor_tensor(out=st4[0:1, 1, s, :],
                                    in0=pbt[32:33, 0, 0:500],
                                    in1=m2[:1, 0:500], op=A.subtract)

        def emit_ln_ru4(st4):
            # Batched r/u for all 4 samples (Sqrt + DVE recip: one table load
            # at the phase boundary). Reading the full st4 var plane makes
            # this depend on sample 3's stats, keeping table-based ACT work
            # off the gelu phase.
            ru4 = scr2.tile([1, 2, 4, 500], bf16, tag="ru4")
            nc.scalar.activation(out=ru4[:1, 0, :, :], in_=st4[:1, 1, :, :],
                                 func=AF.Sqrt, bias=epsv[0:1, :])
            # r = 1/sigma; u = mean*r (bf16, same precision as the inline path)
            with nc.allow_low_precision("ln r/u bf16 as inline path"):
                nc.vector.reciprocal(out=ru4[:1, 1, :, :],
                                     in_=ru4[:1, 0, :, :])
            nc.vector.tensor_tensor(out=ru4[:1, 0, :, :],
                                    in0=st4[:1, 0, :, :],
                                    in1=ru4[:1, 1, :, :], op=A.mult)
            return ru4

        def emit_ln_apply(xn, s, ru4):
            # Broadcast r/u for one sample and normalize. Emitted with one
            # sample of lookahead so the in-order PE queue never stalls on ru4.
            co = s * 500
            pbt = pmB.tile([128, 2, 512], f32, tag="pmB")
            nc.tensor.matmul(out=pbt[:, 0, 0:500], lhsT=ones1b[:],
                             rhs=ru4[:1, 1, s, :], start=True, stop=True)
            nc.tensor.matmul(out=pbt[:, 1, 0:500], lhsT=ones1b[:],
                             rhs=ru4[:1, 0, s, :], start=True, stop=True)
            for blk in range(2):
                nc.vector.tensor_tensor(out=xn[:, blk, co:co + 500],
                                        in0=x[:, blk, co:co + 500],
                                        in1=pbt[:, 0, 0:500], op=A.mult)
                nc.vector.tensor_tensor(out=xn[:, blk, co:co + 500],
                                        in0=xn[:, blk, co:co + 500],
                                        in1=pbt[:, 1, 0:500], op=A.subtract)

        UNITS = [(c4, j) for j in range(2) for c4 in range(4)]
        LAG = 4
        xn = big.tile([128, 2, 2000], bf16, tag="xnA")
        for s in range(ns_run):
            emit_ln(xn, s)
        pend_ln = None
        for i in range(nl_run):
            W = LWslot[i % 2]
            attn = big.tile([128, 2, 2000], bf16, tag="attn")
            xn2 = big.tile([128, 2, 2000], bf16, tag="xn2")
            lnst = {}
            for s in range(ns_run):
                cs = s * 500
                if pend_ln is not None and s + 1 < ns_run:
                    emit_ln_apply(pend_ln[0], s + 1, pend_ln[1])
                if s >= 1:
                    lnst[s - 1] = emit_ln_stats(s - 1)
                # double-buffered per sample so qkv(s+1) can run under the
                # units pipeline of sample s
                qkv = scr.tile([128, 6, 512], bf16, tag=f"qkv{s % 2}")
                cqS = scr.tile([8, 512], f32, tag=f"cqS{s % 2}")
                if i == 0 and s <= 1:
                    # pad keys 500..511: k''=0, v=0 -> score 0, av contrib 0
                    nc.vector.memset(qkv[:, 2:6, 500:512], 0.0)
                    # exp bias -30 at pad keys -> eT ~ 0 there
                    nc.vector.memset(cqS[:, 500:512], -30.0)
                for j in [0, 2, 4, 5, 1, 3]:
                    pq = pmA.tile([128, 512], f32, tag="pmA")
                    for kc in range(2):
                        nc.tensor.matmul(
                            out=pq[:, 0:500],
                            lhsT=W["wqkvT"][:, kc, j * 128:(j + 1) * 128],
                            rhs=xn[:, kc, cs:cs + 500],
                            start=(kc == 0), stop=(kc == 1))
                    if j >= 2:
                        nc.vector.tensor_copy(qkv[:, j, 0:500], pq[:, 0:500])
                    else:
                        nc.scalar.activation(out=qkv[:, j, 0:500],
                                             in_=pq[:, 0:500], func=AF.Copy)
                # per-key exp-bias rows c_h = wc_h . zhat (extra qkv outputs)
                pq8 = pmA.tile([128, 512], f32, tag="pmA")
                for kc in range(2):
                    nc.tensor.matmul(out=pq8[0:8, 0:500],
                                     lhsT=W["wqkvT"][:, kc, 768:776],
                                     rhs=xn[:, kc, cs:cs + 500],
                                     start=(kc == 0), stop=(kc == 1))
                nc.vector.tensor_copy(cqS[0:8, 0:500], pq8[0:8, 0:500])
                # V^T via SBUF->SBUF DMA transpose (128-key blocks)
                vts = []
                for j in range(2):
                    vt = scr2.tile([128, 4, 128], bf16, tag=f"vt{j}{s % 2}")
                    for skc in range(4):
                        nc.sync.dma_start_transpose(
                            out=vt[:, skc, :],
                            in_=qkv[:, 4 + j, 128 * skc:128 * skc + 128])
                    vts.append(vt)
                # c-bias transposed into per-key layout: esT cols 32..64
                esT = pmS.tile([128, 512], f32, tag="pmS")
                pot0 = pmP.tile([128, 512], f32, tag="pmP")
                pot1 = pmB.tile([128, 512], f32, tag="potB")
                pots = [pot0, pot1]
                for skc in range(4):
                    nc.tensor.transpose(out=esT[:, 32 + 8 * skc:40 + 8 * skc],
                                        in_=cqS[0:8, 128 * skc:128 * skc + 128],
                                        identity=identF[:8, 0:8])
                cbS = scr.tile([128, 4, 8], f32, tag="cbS")
                nc.vector.tensor_copy(cbS[:], esT[:, 32:64])
                def attn_tail(j):
                    rrT = sc.tile([125, 16], f32, tag=f"rrT{j}")
                    nc.vector.reciprocal(out=rrT[:], in_=esT[:125, 16 * j:16 * j + 16])
                    pcol = 64 + 128 * j
                    nc.tensor.transpose(out=esT[0:16, pcol:pcol + 125], in_=rrT[:],
                                        identity=identF[:125, 0:125])
                    rrTT = sc.tile([16, 125], bf16, tag=f"rrTT{j}")
                    with nc.allow_low_precision("softmax recip bcast bf16"):
                        nc.vector.tensor_copy(rrTT[:], esT[0:16, pcol:pcol + 125])
                    prb = pmA.tile([128, 512], f32, tag="pmA")
                    for qb in range(4):
                        nc.tensor.matmul(out=prb[:, 125 * qb:125 * qb + 125],
                                         lhsT=C["indTc"][:, qb, :], rhs=rrTT[:],
                                         start=True, stop=True)
                    prbS = scr2.tile([128, 500], bf16, tag=f"prbS{j}")
                    nc.vector.tensor_copy(prbS[:], prb[:, 0:500])
                    nc.vector.tensor_tensor(out=attn[:, j, cs:cs + 500],
                                            in0=pots[j][:, 0:500], in1=prbS[:],
                                            op=A.mult)
                eTs = {}
                for t in range(len(UNITS) + LAG):
                    if t < len(UNITS):
                        c4, j = UNITS[t]
                        poff = 32 * c4
                        eT = scr.tile([128, 4, 500], bf16, tag=f"eT{t % 5}")
                        eTs[t] = eT
                        for skc in range(4):
                            psc = pmA.tile([128, 512], f32, tag="pmA")
                            nc.tensor.matmul(
                                out=psc[:, 0:500],
                                lhsT=qkv[poff:poff + 32, 2 + j,
                                         128 * skc:128 * skc + 128],
                                rhs=qkv[poff:poff + 32, j, 0:500],
                                start=True, stop=True,
                                tile_position=(poff, 0))
                            nc.scalar.activation(
                                out=eT[:, skc, :], in_=psc[:, 0:500],
                                func=AF.Exp,
                                bias=cbS[:, skc, 4 * j + c4:4 * j + c4 + 1])
                    if t >= LAG:
                        c4, j = UNITS[t - LAG]
                        poff = 32 * c4
                        eT = eTs.pop(t - LAG)
                        for skc in range(4):
                            nc.tensor.matmul(out=pots[j][poff:poff + 32, 0:500],
                                             lhsT=vts[j][:, skc, poff:poff + 32],
                                             rhs=eT[:, skc, :],
                                             start=(skc == 0), stop=(skc == 3),
                                             tile_position=(0, poff))
                        # esum: accumulate the 4 key blocks directly in PSUM
                        # (out free size 1 -> ~free on PE) instead of a DVE
                        # tree-add of eT.
                        for qb in range(4):
                            for skc in range(4):
                                nc.tensor.matmul(
                                    out=esT[:125, 16 * j + 4 * c4 + qb:
                                            16 * j + 4 * c4 + qb + 1],
                                    lhsT=eT[:, skc, 125 * qb:125 * qb + 125],
                                    rhs=onesPb[:, 0:1],
                                    start=(skc == 0), stop=(skc == 3))
                        if t - LAG == 3:
                            attn_tail(0)
                        elif t - LAG == 7:
                            attn_tail(1)
                # Wo + residual for this sample (bias via 1-row matmul)
                for mb in range(2):
                    po = pmA.tile([128, 512], f32, tag="pmA")
                    for kc in range(2):
                        nc.tensor.matmul(
                            out=po[:, 0:500],
                            lhsT=W["woT"][:, kc, mb * 128:(mb + 1) * 128],
                            rhs=attn[:, kc, cs:cs + 500],
                            start=(kc == 0), stop=(kc == 1))
                    # bias folded into the residual add (saves a 500-col
                    # rank-1 matmul and shortens the psum group)
                    nc.vector.scalar_tensor_tensor(
                        out=x[:, mb, cs:cs + 500], in0=po[:, 0:500],
                        scalar=W["boW"][:, mb:mb + 1],
                        in1=x[:, mb, cs:cs + 500], op0=A.add, op1=A.add)
                if s >= 1:
                    emit_ln_finish(xn2, s - 1, lnst.pop(s - 1))
            emit_ln(xn2, ns_run - 1)
            xn_next = big.tile([128, 2, 2000], bf16,
                               tag=("xnA" if (i + 1) % 2 == 0 else "xnB"))
            st4 = None
            if i + 1 < nl_run:
                st4 = scr2.tile([1, 2, 4, 500], f32, tag="lnsb4",
                                name="lnsb4")
            for s in range(ns_run):
                cs = s * 500
                if i + 1 < nl_run and s >= 1:
                    emit_ln_stats_sb(s - 1, st4)
                fh = scr.tile([128, 8, 500], bf16, tag=f"fh{s % 2}")
                for hb in range(8):
                    phh = pmA.tile([128, 512], f32, tag="pmA")
                    for kc in range(2):
                        nc.tensor.matmul(
                            out=phh[:, 0:500],
                            lhsT=W["w1TT"][:, kc, hb * 128:(hb + 1) * 128],
                            rhs=xn2[:, kc, cs:cs + 500], start=(kc == 0),
                            stop=(kc == 1))
                    nc.scalar.activation(out=fh[:, hb, :], in_=phh[:, 0:500],
                                         func=AF.Gelu, bias=W["b1v"][:, hb:hb + 1])
                for mb in range(2):
                    pf = pmA.tile([128, 512], f32, tag="pmA")
                    for hb in range(8):
                        nc.tensor.matmul(
                            out=pf[:, 0:500],
                            lhsT=W["w2TT"][:, hb, mb * 128:(mb + 1) * 128],
                            rhs=fh[:, hb, :], start=(hb == 0), stop=(hb == 7))
                    nc.vector.scalar_tensor_tensor(
                        out=x[:, mb, cs:cs + 500], in0=pf[:, 0:500],
                        scalar=W["b2W"][:, mb:mb + 1],
                        in1=x[:, mb, cs:cs + 500], op0=A.add, op1=A.add)
            if i + 1 < nl_run:
                emit_ln_stats_sb(ns_run - 1, st4)
                ru4 = emit_ln_ru4(st4)
                emit_ln_apply(xn_next, 0, ru4)
                pend_ln = (xn_next, ru4)
            if i + 2 < nl_run:
                LWslot[i % 2] = load_layer_weights(i % 2, i + 2)
            xn = xn_next

        # ================= pooling + head =================
        for s in range(ns_run):
            cs = s * 500
            plg = pmS.tile([128, 512], f32, tag="pmS")
            for blk in range(2):
                nc.tensor.matmul(out=plg[0:1, 0:500],
                                 lhsT=C["poolq"][:, blk:blk + 1],
                                 rhs=x[:, blk, cs:cs + 500], start=(blk == 0),
                                 stop=(blk == 1))
            wrow = sc.tile([1, 500], f32r, tag="wrow")
            nc.scalar.activation(out=wrow[:], in_=plg[0:1, 0:500], func=AF.Exp,
                                 scale=1.0 / 16.0)
            pwb = pmA.tile([128, 512], f32, tag="pmA")
            nc.tensor.matmul(out=pwb[:, 0:500], lhsT=ones1r[:], rhs=wrow[:],
                             start=True, stop=True)
            wx = scr2.tile([128, 2, 500], f32, tag="wx")
            for blk in range(2):
                nc.vector.tensor_tensor(out=wx[:, blk, :], in0=x[:, blk, cs:cs + 500],
                                        in1=pwb[:, 0:500], op=A.mult)
            pooled = sc.tile([128, 4], f32, tag="pooled")
            nc.vector.tensor_reduce(out=pooled[:, 0:2], in_=wx[:], axis=AX.X,
                                    op=A.add)
            nc.scalar.activation(out=pooled[:, 2:4], in_=pooled[:, 0:2],
                                 func=AF.Square)
            pps = pmS.tile([128, 512], f32, tag="pmS")
            nc.tensor.matmul(out=pps[0:1, 0:4], lhsT=onesPf[:], rhs=pooled[:],
                             start=True, stop=True)
            z = sc.tile([1, 16], f32, tag="z")
            nc.vector.tensor_copy(z[:, 12:16], pps[0:1, 0:4])
            nc.vector.tensor_tensor(out=z[:, 0:1], in0=z[:, 12:13],
                                    in1=z[:, 13:14], op=A.add)
            nc.vector.tensor_tensor(out=z[:, 1:2], in0=z[:, 14:15],
                                    in1=z[:, 15:16], op=A.add)
            nc.vector.tensor_scalar(out=z[:, 2:3], in0=z[:, 0:1],
                                    scalar1=1.0 / 256.0, scalar2=None, op0=A.mult)
            nc.vector.tensor_tensor(out=z[:, 3:4], in0=z[:, 2:3], in1=z[:, 2:3],
                                    op=A.mult)
            nc.vector.tensor_scalar(out=z[:, 4:5], in0=z[:, 1:2],
                                    scalar1=1.0 / 256.0, scalar2=None, op0=A.mult)
            nc.vector.tensor_tensor(out=z[:, 4:5], in0=z[:, 4:5], in1=z[:, 3:4],
                                    op=A.subtract)
            nc.scalar.activation(out=z[:, 5:6], in_=z[:, 4:5], func=AF.Ln,
                                 bias=epsv[0:1, :])
            nc.scalar.activation(out=z[:, 6:7], in_=z[:, 5:6], func=AF.Exp,
                                 scale=-0.5)
            cp = sc.tile([128, 2], f32, tag="cp")
            nc.vector.tensor_tensor(out=cp[:], in0=pooled[:, 0:2], in1=C["c1v"][:],
                                    op=A.mult)
            pa = pmS.tile([128, 512], f32, tag="pmS")
            nc.tensor.matmul(out=pa[0:1, 0:2], lhsT=onesPf[:], rhs=cp[:],
                             start=True, stop=True)
            nc.vector.tensor_copy(z[:, 10:12], pa[0:1, 0:2])
            nc.vector.tensor_tensor(out=z[:, 7:8], in0=z[:, 10:11],
                                    in1=z[:, 11:12], op=A.add)
            nc.vector.tensor_scalar(out=z[:, 8:9], in0=z[:, 2:3], scalar1=sc1,
                                    scalar2=None, op0=A.mult)
            nc.vector.tensor_tensor(out=z[:, 8:9], in0=z[:, 7:8], in1=z[:, 8:9],
                                    op=A.subtract)
            nc.vector.tensor_tensor(out=z[:, 8:9], in0=z[:, 8:9], in1=z[:, 6:7],
                                    op=A.mult)
            nc.vector.tensor_scalar(out=z[:, 9:10], in0=z[:, 8:9], scalar1=c2s,
                                    scalar2=None, op0=A.add)
            nc.sync.dma_start(out=Y[s:s + 1, :], in_=z[:, 9:10])

        for p in [pmS, pmP, pmB, pmA, scr2, scr, big, wpool, bigx, sc, sing]:
            p.release()

    nc.compile()
    return nc


_BUILT = {}


def _get_nc(key, **kw):
    if key not in _BUILT:
        _BUILT[key] = _build(**kw)
    return _BUILT[key]


def _make_in_maps(prep, ns=NS, ncores=NCORES):
    in_maps = []
    for c in range(ncores):
        sl = slice(c * ns, (c + 1) * ns)
        m = {k: prep[k] for k in CONST_KEYS}
        m["mel4"] = prep["_mel"][sl]
        m["evrows"] = prep["_evrows"][sl]
        m["combv"] = prep["_combv"][sl]
        m["tposv"] = prep["_tposv"][sl]
        m["keepv"] = prep["_keepv"][sl]
        m["starbias"] = prep["_starbias"][sl]
        in_maps.append(m)
    return in_maps


def kernel(**inputs):
    from concourse.bass_utils import run_bass_kernel_spmd

    prep = _host_prep(inputs)
    nc = _get_nc("full", nl_run=NLAYERS, ns_run=NS, debug=False,
                 sc1=prep["sc1"], c2s=prep["c2s"])
    res = run_bass_kernel_spmd(nc, _make_in_maps(prep), list(range(NCORES)))
    y = np.concatenate([res.results[c]["y"].reshape(-1) for c in range(NCORES)])
    return y.astype(np.float32)



# revision 57
# speedup vs baseline: 1.1042x; 1.0119x over previous
"""Trainium2 Bass kernel for nn_ChartQualityEvaluator.

Data parallel: 32 samples -> 8 cores x 4 samples. Feature-major activations
[128 part, 2 blocks, 4*500 cols]; residual stream kept in float32r so the
LN-mean / pooling matmuls stream at 1 cyc/row instead of fp32's 4. Host
folds LN gain/bias into adjacent weights, q/k biases into a per-key exp
bias, the v bias into Wo's bias, and precomputes the event sinusoid
features (pure functions of host data, DMA'd in). Softmax esum accumulates
straight from eT via free 1-col matmuls (no DVE tree). Wo/W2 biases ride
the residual add (scalar_tensor_tensor). Transformer weights stream
through two SBUF slots with DMA prefetch. Inter-layer LN: stats staged to
SBUF during the gelu phase (Square is in every act table), batched
sqrt+reciprocal at the phase boundary, and per-sample broadcast/normalize
emitted one sample ahead in the next attention loop so the in-order PE
queue never stalls on it.
"""
import math
import sys

import numpy as np

_TRN = "/opt/trn_rl_repo"
if _TRN not in sys.path:
    sys.path.insert(0, _TRN)

import ml_dtypes

BF16 = ml_dtypes.bfloat16

D = 256
H = 8
NLAYERS = 6
HALF = 128
S = 500
NEV = 256
NCORES = 8
NS = 4  # samples per core
B = 32
EPS = 1e-5
INV2PI = float(np.float32(1.0 / (2.0 * math.pi)))
TWOPI = 2.0 * math.pi
SC32 = 1.0 / math.sqrt(32.0)


def _host_prep(inp):
    f = np.float32
    f8 = np.float64
    out = {}

    def t2(v):  # [256] -> [128,2] fp32
        return np.ascontiguousarray(np.asarray(v).reshape(2, 128).T.astype(f))

    def b(a):  # to bf16
        return np.ascontiguousarray(np.asarray(a).astype(f).astype(BF16))

    # ---------------- frontend ----------------
    out["wmelT"] = b(np.asarray(inp["mel_W"]).T)
    out["melb"] = np.ascontiguousarray(np.asarray(inp["mel_b"]).reshape(16, 1).astype(f))
    out["w1t"] = b(np.asarray(inp["conv1_w"]).transpose(1, 2, 0))
    out["c1b"] = np.ascontiguousarray(np.asarray(inp["conv1_b"]).reshape(128, 1).astype(f))
    out["gng"] = np.ascontiguousarray(np.asarray(inp["gn_g"]).reshape(128, 1).astype(f))
    out["gnb"] = np.ascontiguousarray(np.asarray(inp["gn_b"]).reshape(128, 1).astype(f))
    out["w2t"] = b(np.asarray(inp["conv2_w"]).transpose(1, 2, 0))
    out["c2b"] = t2(inp["conv2_b"])
    out["cng"] = t2(inp["cn_g"])

    freq = np.exp(np.arange(HALF, dtype=f) * f(-math.log(10000.0) / (HALF - 1)))
    e32 = (np.arange(S, dtype=f)[None, :] * freq[:, None]).astype(f)
    e64 = e32.astype(np.float64)
    pos_fm = np.concatenate([np.sin(e64), np.cos(e64)], axis=0)  # [256,500]
    out["posT"] = np.ascontiguousarray(
        pos_fm.reshape(2, 128, S).transpose(1, 0, 2).astype(f))
    out["freqv"] = np.ascontiguousarray(freq.reshape(128, 1))

    out["epW1T"] = np.ascontiguousarray(
        np.asarray(inp["ep_W1"]).T.reshape(6, 128, 256).transpose(1, 0, 2).astype(f).astype(BF16))
    out["epb1"] = t2(inp["ep_b1"])
    out["epW2T"] = np.ascontiguousarray(
        np.asarray(inp["ep_W2"]).T.reshape(2, 128, 256).transpose(1, 0, 2).astype(f).astype(BF16))
    out["epb2row"] = np.ascontiguousarray(
        np.tile(np.asarray(inp["ep_b2"]).astype(f)[None, :], (128, 1)))

    def wT(w, kc, m, dt=BF16):  # w [m, k] -> [128, kc, m]
        return np.ascontiguousarray(
            np.asarray(w).astype(f).T.reshape(kc, 128, m).transpose(1, 0, 2).astype(dt))

    # ---------------- transformer with LN folding ----------------
    wqkvT, woT, boW, w1TT, b1v, w2TT, b2W = [], [], [], [], [], [], []
    for i in range(NLAYERS):
        g1 = np.asarray(inp["tl_ln1g"][i]).astype(f8)
        b1 = np.asarray(inp["tl_ln1b"][i]).astype(f8)
        g2 = np.asarray(inp["tl_ln2g"][i]).astype(f8)
        b2 = np.asarray(inp["tl_ln2b"][i]).astype(f8)
        Wqkv = np.asarray(inp["tl_Wqkv"][i]).astype(f8)   # [768, 256]
        bqkv = np.asarray(inp["tl_bqkv"][i]).astype(f8)   # [768]
        Wo = np.asarray(inp["tl_Wo"][i]).astype(f8)       # [256, 256]
        bo = np.asarray(inp["tl_bo"][i]).astype(f8)
        W1 = np.asarray(inp["tl_W1"][i]).astype(f8)       # [1024, 256]
        b1f = np.asarray(inp["tl_b1"][i]).astype(f8)
        W2 = np.asarray(inp["tl_W2"][i]).astype(f8)       # [256, 1024]
        b2f = np.asarray(inp["tl_b2"][i]).astype(f8)

        Wq, Wk, Wv = Wqkv[0:256], Wqkv[256:512], Wqkv[512:768]
        Wq_f = Wq * g1[None, :]
        Wk_f = Wk * g1[None, :] * SC32
        Wv_f = Wv * g1[None, :]
        bq_eff = Wq @ b1 + bqkv[0:256]          # survives as per-key exp bias
        bv_eff = Wv @ b1 + bqkv[512:768]        # folds into Wo bias
        # wc_h: c_h[k] = bq_eff_h . k''_h[:,k] = (Wk_f[h].T @ bq_eff[h]) . zhat
        WC = np.stack([Wk_f[32 * h:32 * h + 32].T @ bq_eff[32 * h:32 * h + 32]
                       for h in range(8)])      # [8, 256]
        W_f = np.concatenate([Wq_f, Wk_f, Wv_f, WC], axis=0)  # [776, 256]
        wqkvT.append(wT(W_f, 2, 776))
        bo_eff = bo + Wo @ bv_eff
        woT.append(wT(Wo, 2, 256))
        boW.append(t2(bo_eff.astype(f)))
        W1_f = W1 * g2[None, :]
        b1_eff = W1 @ b2 + b1f
        w1TT.append(wT(W1_f, 2, 1024))
        b1v.append(np.ascontiguousarray(
            b1_eff.astype(f).reshape(8, 128).T.astype(f)))     # [128, 8] fp32
        w2TT.append(wT(W2, 8, 256))
        b2W.append(t2(b2f.astype(f)))
    out["wqkvT"] = np.stack(wqkvT)
    out["woT"] = np.stack(woT)
    out["boW"] = np.stack(boW)
    out["w1TT"] = np.stack(w1TT)
    out["b1v"] = np.stack(b1v)
    out["w2TT"] = np.stack(w2TT)
    out["b2W"] = np.stack(b2W)

    indT = np.zeros((16, 4, 128), np.float32)
    for qb in range(4):
        for p in range(128):
            indT[4 * (p // 32) + qb, qb, p] = 1.0
    out["indTc"] = np.ascontiguousarray(indT.astype(BF16))

    # ---------------- pooling head ----------------
    out["poolq"] = np.ascontiguousarray(
        np.asarray(inp["pool_q"]).astype(f).reshape(2, 128).T)
    c1 = (np.asarray(inp["oh_W"])[0] * np.asarray(inp["on_g"])).astype(f)
    out["c1v"] = t2(c1)
    out["sc1"] = float(c1.astype(np.float64).sum())
    out["c2s"] = float((np.asarray(inp["oh_W"])[0].astype(np.float64)
                        * np.asarray(inp["on_b"]).astype(np.float64)).sum()
                       + float(np.asarray(inp["oh_b"])[0]))

    # ---------------- per-sample host precompute ----------------
    events = np.asarray(inp["events"]).astype(np.int64)
    mask = np.asarray(inp["event_mask"])
    star = np.asarray(inp["star_rating"]).astype(f)
    nb = events.shape[0]
    diff = np.maximum(events[:, 1:] - events[:, :-1], 1)
    g = np.concatenate([diff[:, :1], diff], axis=1)
    gap_ms = (g * 5).astype(f)
    g_f = np.maximum(g.astype(f), f(1.0))
    r = np.clip(g_f[:, 1:] / g_f[:, :-1], f(0.1), f(10.0)).astype(f)
    ones = np.ones((nb, 1), f)
    rb50 = np.trunc(np.concatenate([ones, r], axis=1) * f(50.0)).astype(f)
    ra50 = np.trunc(np.concatenate([r, ones], axis=1) * f(50.0)).astype(f)
    out["_evrows"] = np.ascontiguousarray(np.stack([rb50, ra50, gap_ms], axis=1))
    # event sinusoids are pure functions of host data: precompute [B,128,6,256]
    argv = (out["_evrows"][:, None, :, :]
            * freq[None, :, None, None]).astype(np.float32)
    combv = np.empty((nb, HALF, 6, NEV), np.float32)
    combv[:, :, 0::2, :] = np.sin(argv)
    combv[:, :, 1::2, :] = np.cos(argv)
    out["_combv"] = np.ascontiguousarray(combv.astype(BF16))
    tp = np.clip(events // 4, 0, S - 1).astype(f)
    keep = (1.0 - mask.astype(f)).astype(f)
    out["_tposv"] = np.ascontiguousarray(tp.reshape(nb, 2, 128).transpose(0, 2, 1))
    out["_keepv"] = np.ascontiguousarray(keep.reshape(nb, 2, 128).transpose(0, 2, 1))
    bucket = np.clip((star / f(0.5)).astype(np.int32), 0, 19)
    sb = (np.asarray(inp["cn_b"])[None, :] + np.asarray(inp["star_table"])[bucket]).astype(f)
    out["_starbias"] = np.ascontiguousarray(sb.reshape(nb, 2, 128).transpose(0, 2, 1))
    out["_mel"] = np.asarray(inp["mel"]).astype(f).astype(BF16)
    return out


CONST_KEYS = ["wmelT", "melb", "w1t", "c1b", "gng", "gnb", "w2t", "c2b", "cng",
              "posT", "freqv", "epW1T", "epb1", "epW2T", "epb2row",
              "wqkvT", "woT", "boW", "w1TT", "b1v", "w2TT", "b2W",
              "poolq", "c1v", "indTc"]


def _build(nl_run=NLAYERS, ns_run=NS, debug=False, sc1=0.0, c2s=0.0):
    import concourse.bacc as bacc
    import concourse.tile as tile
    from concourse import mybir
    from concourse.masks import make_identity

    class _Bacc(bacc.Bacc):
        # Restrict activation-table choice so ln/exp (and everything the
        # kernel needs besides gelu/sin) resolve to one shared table --
        # avoids a table reload on every LayerNorm. Table ids stay
        # positional, so unused entries are blanked rather than removed.
        _KEEP = {"natural_log_exp_and_others", "trig_and_small",
                 "gelu_and_others", "sqrt_and_others"}

        def insert_act_table_loads(self):
            from concourse.hw_specs import get_activation_tables
            import bass_rust as _bass_rust
            has_activation = any(
                isinstance(i, mybir.InstActivation)
                for b in self.main_func.blocks
                for i in b.instructions
            )
            if not has_activation:
                return
            tables = [
                (name, funcs if name in self._KEEP else set())
                for name, funcs in get_activation_tables(self.m.arch).items()
            ]
            _bass_rust.insert_act_table_loads(self, tables)

    f32 = mybir.dt.float32
    bf16 = mybir.dt.bfloat16
    f32r = mybir.dt.float32r
    f8 = mybir.dt.float8e4
    A = mybir.AluOpType
    AF = mybir.ActivationFunctionType
    AX = mybir.AxisListType
    DR = mybir.MatmulPerfMode.DoubleRow

    def r32(ap):
        # fp32 matmul streams at 4 cyc/row; f32r at 1 (ap>=256). Same bytes.
        return ap.bitcast(f32r)

    nc = _Bacc(None)
    P = {}
    shapes = dict(
        mel4=([ns_run, 80, 2000], bf16),
        evrows=([ns_run, 3, 256], f32r),
        combv=([ns_run, 128, 6, 256], bf16),
        tposv=([ns_run, 128, 2], f32),
        keepv=([ns_run, 128, 2], f32),
        starbias=([ns_run, 128, 2], f32),
        wmelT=([80, 16], bf16), melb=([16, 1], f32),
        w1t=([16, 7, 128], bf16), c1b=([128, 1], f32),
        gng=([128, 1], f32), gnb=([128, 1], f32),
        w2t=([128, 7, 256], bf16), c2b=([128, 2], f32),
        cng=([128, 2], f32), posT=([128, 2, 500], f32), freqv=([128, 1], f32),
        epW1T=([128, 6, 256], bf16), epb1=([128, 2], f32),
        epW2T=([128, 2, 256], bf16), epb2row=([128, 256], f32),
        wqkvT=([NLAYERS, 128, 2, 776], bf16),
        woT=([NLAYERS, 128, 2, 256], bf16), boW=([NLAYERS, 128, 2], f32),
        w1TT=([NLAYERS, 128, 2, 1024], bf16), b1v=([NLAYERS, 128, 8], f32),
        w2TT=([NLAYERS, 128, 8, 256], bf16), b2W=([NLAYERS, 128, 2], f32),
        poolq=([128, 2], f32r), c1v=([128, 2], f32),
        indTc=([16, 4, 128], bf16),
    )
    for k, (sh, dt) in shapes.items():
        P[k] = nc.declare_dram_parameter(k, sh, dt, isOutput=False)
    Y = nc.declare_dram_parameter("y", [ns_run, 1], f32, isOutput=True)
    DBG = None
    if debug:
        DBG = nc.declare_dram_parameter("dbg_x0", [128, 2, 2000], f32, isOutput=True)

    with tile.TileContext(nc) as tc:
        sing = tc.alloc_tile_pool(name="sing", bufs=1)
        sc = tc.alloc_tile_pool(name="sc", bufs=2)
        bigx = tc.alloc_tile_pool(name="bigx", bufs=1)
        wpool = tc.alloc_tile_pool(name="wpool", bufs=1)
        # PSUM pools: pmA rotating 1-bank x3, pmB 2-bank, pmP 1-bank,
        # pmS 1-bank (stats+esum), pmT small bf16 transpose staging
        pmA = tc.alloc_tile_pool(name="pmA", bufs=3, space="PSUM")
        pmB = tc.alloc_tile_pool(name="pmB", bufs=1, space="PSUM")
        pmP = tc.alloc_tile_pool(name="pmP", bufs=1, space="PSUM")
        pmS = tc.alloc_tile_pool(name="pmS", bufs=1, space="PSUM")

        fc = tc.alloc_tile_pool(name="fc", bufs=1)
        _FRONT = ["wmelT", "melb", "w1t", "c1b", "gng", "gnb", "w2t", "c2b",
                  "cng", "posT", "freqv", "epW1T", "epb1", "epW2T", "epb2row"]
        C = {}
        for k in _FRONT + ["poolq", "c1v", "indTc"]:
            pool = fc if k in _FRONT else sing
            C[k] = pool.tile(shapes[k][0], shapes[k][1], tag=k, name="c_" + k)
            nc.sync.dma_start(out=C[k][:], in_=P[k][:])
        # transformer weights stream through 2 SBUF slots (DMA is nearly
        # idle); slot for layer i+2 refills while layer i+1 runs
        WSPECS = [("wqkvT", [128, 2, 776], bf16),
                  ("woT", [128, 2, 256], bf16),
                  ("boW", [128, 2], f32),
                  ("w1TT", [128, 2, 1024], bf16),
                  ("b1v", [128, 8], f32),
                  ("w2TT", [128, 8, 256], bf16),
                  ("b2W", [128, 2], f32)]

        def load_layer_weights(slot, i):
            W = {}
            for k, sh, dt in WSPECS:
                W[k] = wpool.tile(sh, dt, tag=f"w{slot}_{k}",
                                  name=f"w{slot}_{k}")
                nc.sync.dma_start(out=W[k][:], in_=P[k][i])
            return W

        LWslot = None  # loaded after the frontend DMAs are queued

        identF = sing.tile([128, 128], f32, tag="identF")
        make_identity(nc, identF[:])
        onesPb = sing.tile([128, 1], bf16, tag="onesPb")      # plain ones bf16
        nc.vector.memset(onesPb[:], 1.0)
        onesP8 = sing.tile([128, 1], f8, tag="onesP8")        # ones fp8e4
        nc.vector.memset(onesP8[:], 1.0)
        onesMb = sing.tile([128, 1], bf16, tag="onesMb")      # 1/256 (stats lhsT)
        nc.vector.memset(onesMb[:], 1.0 / 256.0)
        onesMf = sing.tile([128, 1], f32r, tag="onesMf")      # 1/256 f32r
        _mfs = sing.tile([128, 1], f32, tag="_mfs")
        nc.vector.memset(_mfs[:], 1.0 / 256.0)
        nc.vector.tensor_copy(onesMf[:], _mfs[:])
        ones1b = sing.tile([1, 128], bf16, tag="ones1b")
        nc.vector.memset(ones1b[:], 1.0)
        ones1f = sing.tile([1, 128], f32, tag="ones1f")
        nc.vector.memset(ones1f[:], 1.0)
        ones1r = sing.tile([1, 128], f32r, tag="ones1r")
        _1rs = sing.tile([1, 128], f32, tag="_1rs")
        nc.vector.memset(_1rs[:], 1.0)
        nc.vector.tensor_copy(ones1r[:], _1rs[:])
        ones500b = sing.tile([1, 500], bf16, tag="ones500b")
        nc.vector.memset(ones500b[:], 1.0)
        onesPf = sing.tile([128, 1], f32, tag="onesPf")
        nc.vector.memset(onesPf[:], 1.0)
        zerov = fc.tile([128, 1], f32, tag="zerov")
        nc.vector.memset(zerov[:], 0.0)
        epsv = sing.tile([128, 1], f32, tag="epsv")
        nc.vector.memset(epsv[:], EPS)
        iotaB = fc.tile([128, 500], f32, tag="iotaB")
        nc.gpsimd.iota(iotaB[:], pattern=[[1, 500]], base=0, channel_multiplier=0,
                       allow_small_or_imprecise_dtypes=True)

        # f32r residual stream: mean/pool matmuls stream at 1 cyc/row
        x = bigx.tile([128, 2, 2000], f32r, tag="x_fm")

        def s2(t):  # step-2 view of [p, n] -> [p, n//2]
            return t.rearrange("p (t s) -> p s t", s=2)[:, 0, :]

        # ================= front end =================
        fr = tc.alloc_tile_pool(name="fr", bufs=2)
        for s in range(ns_run):
            cs = s * 500
            melp = fr.tile([80, 2006], bf16, tag="melp")
            nc.vector.memset(melp[:, 0:3], 0.0)
            nc.vector.memset(melp[:, 2003:2006], 0.0)
            nc.sync.dma_start(out=melp[:, 3:2003], in_=P["mel4"][s])
            xmelp = fr.tile([16, 2006], bf16, tag="xmelp")
            nc.vector.memset(xmelp[:, 0:3], 0.0)
            nc.vector.memset(xmelp[:, 2003:2006], 0.0)
            for nch in range(4):
                pcm = pmA.tile([128, 512], f32, tag="pmA")
                nc.tensor.matmul(out=pcm[:16, 0:500], lhsT=C["wmelT"][:],
                                 rhs=melp[:, 3 + nch * 500: 3 + nch * 500 + 500],
                                 start=True, stop=True)
                nc.scalar.activation(out=xmelp[:, 3 + nch * 500: 3 + nch * 500 + 500],
                                     in_=pcm[:16, 0:500], func=AF.Identity,
                                     bias=C["melb"][:, 0:1])
            pc1 = pmB.tile([128, 2, 512], f32, tag="pmB")
            for half in range(2):
                for k in range(7):
                    nc.tensor.matmul(
                        out=pc1[:, half, 0:500], lhsT=C["w1t"][:, k, :],
                        rhs=s2(xmelp[:, k + half * 1000: k + half * 1000 + 1000]),
                        start=(k == 0), stop=(k == 6))
            h1g = fr.tile([128, 2, 500], bf16, tag="h1g")
            stg = fr.tile([128, 2], f32, tag="stg")
            nc.scalar.activation(out=h1g[:], in_=pc1[:, :, 0:500], func=AF.Gelu,
                                 bias=C["c1b"][:, 0:1], accum_out=stg[:, 0:1])
            sqf = fr.tile([128, 2, 500], bf16, tag="sqf")
            nc.scalar.activation(out=sqf[:], in_=h1g[:], func=AF.Square,
                                 accum_out=stg[:, 1:2])
            pg = pmS.tile([128, 512], f32, tag="pmS")
            nc.tensor.matmul(out=pg[:1, 0:2], lhsT=onesPf[:], rhs=stg[:],
                             start=True, stop=True)
            sn = sc.tile([1, 8], f32, tag="sn")
            nc.vector.tensor_scalar(out=sn[:, 0:2], in0=pg[:1, 0:2],
                                    scalar1=1.0 / 128000.0, scalar2=None, op0=A.mult)
            nc.vector.tensor_tensor(out=sn[:, 2:3], in0=sn[:, 0:1], in1=sn[:, 0:1],
                                    op=A.mult)
            nc.vector.tensor_tensor(out=sn[:, 3:4], in0=sn[:, 1:2], in1=sn[:, 2:3],
                                    op=A.subtract)
            nc.scalar.activation(out=sn[:, 4:5], in_=sn[:, 3:4], func=AF.Ln,
                                 bias=epsv[0:1, :])
            nc.scalar.activation(out=sn[:, 1:2], in_=sn[:, 4:5], func=AF.Exp,
                                 scale=-0.5)
            pgb = pmA.tile([128, 512], f32, tag="pmA")
            nc.tensor.matmul(out=pgb[:, 0:2], lhsT=ones1f[:], rhs=sn[:, 0:2],
                             start=True, stop=True)
            sv = sc.tile([128, 2], f32, tag="sv")
            nc.vector.tensor_tensor(out=sv[:, 0:1], in0=pgb[:, 1:2], in1=C["gng"][:],
                                    op=A.mult)
            nc.vector.tensor_tensor(out=sv[:, 1:2], in0=pgb[:, 0:1], in1=sv[:, 0:1],
                                    op=A.mult)
            nc.vector.tensor_tensor(out=sv[:, 1:2], in0=C["gnb"][:], in1=sv[:, 1:2],
                                    op=A.subtract)
            x2p = fr.tile([128, 1006], bf16, tag="x2p")
            nc.vector.memset(x2p[:, 0:3], 0.0)
            nc.vector.memset(x2p[:, 1003:1006], 0.0)
            nc.scalar.activation(out=x2p[:, 3:1003],
                                 in_=h1g.rearrange("p a b -> p (a b)"),
                                 func=AF.Identity, scale=sv[:, 0:1], bias=sv[:, 1:2])
            pc2 = pmB.tile([128, 2, 512], f32, tag="pmB")
            for mb in range(2):
                for k in range(7):
                    nc.tensor.matmul(out=pc2[:, mb, 0:500],
                                     lhsT=C["w2t"][:, k, mb * 128:(mb + 1) * 128],
                                     rhs=s2(x2p[:, k:k + 1000]),
                                     start=(k == 0), stop=(k == 6))
            for mb in range(2):
                nc.scalar.activation(out=x[:, mb, cs:cs + 500], in_=pc2[:, mb, 0:500],
                                     func=AF.Gelu, bias=C["c2b"][:, mb:mb + 1])
            # CN layernorm (stats in bf16) + cng scale + starbias + pos
            sbv = fr.tile([128, 2], f32, tag="sbv")
            nc.sync.dma_start(out=sbv[:], in_=P["starbias"][s])
            sq2 = fr.tile([128, 2, 500], bf16, tag="sqf")
            nc.vector.tensor_tensor(out=sq2[:], in0=x[:, :, cs:cs + 500],
                                    in1=x[:, :, cs:cs + 500], op=A.mult)
            pstt = pmS.tile([128, 512], f32, tag="pmS")
            for blk in range(2):
                nc.tensor.matmul(out=pstt[0:1, 0:500], lhsT=onesMf[:],
                                 rhs=x[:, blk, cs:cs + 500],
                                 start=(blk == 0), stop=(blk == 1))
            for blk in range(2):
                nc.tensor.matmul(out=pstt[32:33, 0:500], lhsT=onesMb[:],
                                 rhs=sq2[:, blk, :], start=(blk == 0), stop=(blk == 1))
            ru = sc.tile([1, 2, 500], bf16, tag="ru")
            tmp = sc.tile([1, 2, 500], f32, tag="tmp1")
            nc.scalar.activation(out=tmp[:1, 0, :], in_=pstt[0:1, 0:500],
                                 func=AF.Square)
            nc.vector.tensor_tensor(out=tmp[:1, 1, :], in0=pstt[32:33, 0:500],
                                    in1=tmp[:1, 0, :], op=A.subtract)
            nc.scalar.activation(out=tmp[:1, 0, :], in_=tmp[:1, 1, :], func=AF.Ln,
                                 bias=epsv[0:1, :])
            nc.scalar.activation(out=ru[:1, 0, :], in_=tmp[:1, 0, :],
                                 func=AF.Exp, scale=-0.5)
            nc.vector.tensor_tensor(out=ru[:1, 1, :], in0=pstt[0:1, 0:500],
                                    in1=ru[:1, 0, :], op=A.mult)
            pbc = pmB.tile([128, 2, 512], f32, tag="pmB")
            for jj in range(2):
                nc.tensor.matmul(out=pbc[:, jj, 0:500], lhsT=ones1b[:],
                                 rhs=ru[:1, jj, :], start=True, stop=True)
            for blk in range(2):
                nc.vector.tensor_tensor(out=x[:, blk, cs:cs + 500],
                                        in0=x[:, blk, cs:cs + 500],
                                        in1=pbc[:, 0, 0:500], op=A.mult)
                nc.vector.tensor_tensor(out=x[:, blk, cs:cs + 500],
                                        in0=x[:, blk, cs:cs + 500],
                                        in1=pbc[:, 1, 0:500], op=A.subtract)
                nc.scalar.activation(out=x[:, blk, cs:cs + 500],
                                     in_=x[:, blk, cs:cs + 500], func=AF.Identity,
                                     scale=C["cng"][:, blk:blk + 1],
                                     bias=sbv[:, blk:blk + 1])
            nc.vector.tensor_tensor(out=x[:, :, cs:cs + 500], in0=x[:, :, cs:cs + 500],
                                    in1=C["posT"][:], op=A.add)

            # events
            tpv = fr.tile([128, 2], f32, tag="tpv")
            nc.sync.dma_start(out=tpv[:], in_=P["tposv"][s])
            kpv = fr.tile([128, 2], f32, tag="kpv")
            nc.sync.dma_start(out=kpv[:], in_=P["keepv"][s])
            comb = fr.tile([128, 6, 256], bf16, tag="comb")
            nc.sync.dma_start(out=comb[:], in_=P["combv"][s])
            hmid = fr.tile([128, 2, 256], bf16, tag="hmid")
            for mb in range(2):
                ph = pmA.tile([128, 512], f32, tag="pmA")
                for kc in range(6):
                    nc.tensor.matmul(out=ph[:, 0:256],
                                     lhsT=C["epW1T"][:, kc, mb * 128:(mb + 1) * 128],
                                     rhs=comb[:, kc, :], start=(kc == 0),
                                     stop=(kc == 5))
                nc.scalar.activation(out=hmid[:, mb, :], in_=ph[:, 0:256],
                                     func=AF.Gelu, bias=C["epb1"][:, mb:mb + 1])
            evt = fr.tile([128, 2, 256], bf16, tag="evt")
            for ec in range(2):
                pe = pmA.tile([128, 512], f32, tag="pmA")
                for kc in range(2):
                    nc.tensor.matmul(out=pe[:, 0:256],
                                     lhsT=hmid[:, kc, ec * 128:(ec + 1) * 128],
                                     rhs=C["epW2T"][:, kc, :], start=(kc == 0),
                                     stop=(kc == 1))
                nc.vector.tensor_tensor(out=evt[:, ec, :], in0=pe[:, 0:256],
                                        in1=C["epb2row"][:], op=A.add)
                nc.vector.tensor_scalar(out=evt[:, ec, :], in0=evt[:, ec, :],
                                        scalar1=kpv[:, ec:ec + 1], scalar2=None,
                                        op0=A.mult)
            oh = fr.tile([128, 2, 500], bf16, tag="oh")
            for ec in range(2):
                nc.vector.tensor_scalar(out=oh[:, ec, :], in0=iotaB[:],
                                        scalar1=tpv[:, ec:ec + 1], scalar2=None,
                                        op0=A.is_equal)
            for mb in range(2):
                px = pmA.tile([128, 512], f32, tag="pmA")
                for ec in range(2):
                    nc.tensor.matmul(out=px[:, 0:500],
                                     lhsT=evt[:, ec, mb * 128:(mb + 1) * 128],
                                     rhs=oh[:, ec, :], start=(ec == 0), stop=(ec == 1))
                nc.vector.tensor_tensor(out=x[:, mb, cs:cs + 500],
                                        in0=x[:, mb, cs:cs + 500], in1=px[:, 0:500],
                                        op=A.add)
        fr.release()
        fc.release()
        LWslot = [load_layer_weights(0, 0)]
        if nl_run > 1:
            LWslot.append(load_layer_weights(1, 1))
        big = tc.alloc_tile_pool(name="big", bufs=1)
        scr = tc.alloc_tile_pool(name="scr", bufs=1)
        scr2 = tc.alloc_tile_pool(name="scr2", bufs=1)

        if debug:
            nc.sync.dma_start(out=DBG[:], in_=x[:])

        # ================= transformer =================
        def emit_ln_stats(s):
            # Per-column mean / E[x^2] of x(s) into rows 0/32 of a pmB tile
            # (same tile later reused for the r/u broadcast).
            co = s * 500
            sq = scr2.tile([128, 2, 500], bf16, tag="sq")
            nc.vector.tensor_tensor(out=sq[:], in0=x[:, :, co:co + 500],
                                    in1=x[:, :, co:co + 500], op=A.mult)
            pbt = pmB.tile([128, 2, 512], f32, tag="pmB")
            for blk in range(2):
                nc.tensor.matmul(out=pbt[0:1, 0, 0:500], lhsT=onesMf[:],
                                 rhs=x[:, blk, co:co + 500],
                                 start=(blk == 0), stop=(blk == 1))
            for blk in range(2):
                nc.tensor.matmul(out=pbt[32:33, 0, 0:500], lhsT=onesMb[:],
                                 rhs=sq[:, blk, :], start=(blk == 0),
                                 stop=(blk == 1))
            return pbt

        def emit_ln_finish(xn, s, pbt):
            # 1/sigma = exp(-0.5*ln(var)) keeps ACT on the exp/ln table.
            co = s * 500
            ru = sc.tile([1, 2, 500], bf16, tag="ru")
            tmp = sc.tile([1, 2, 500], f32, tag="tmp1")
            nc.scalar.activation(out=tmp[:1, 0, :], in_=pbt[0:1, 0, 0:500],
                                 func=AF.Square)
            nc.vector.tensor_tensor(out=tmp[:1, 1, :], in0=pbt[32:33, 0, 0:500],
                                    in1=tmp[:1, 0, :], op=A.subtract)
            nc.scalar.activation(out=tmp[:1, 0, :], in_=tmp[:1, 1, :],
                                 func=AF.Ln, bias=epsv[0:1, :])
            nc.scalar.activation(out=ru[:1, 0, :], in_=tmp[:1, 0, :],
                                 func=AF.Exp, scale=-0.5)
            nc.vector.tensor_tensor(out=ru[:1, 1, :], in0=pbt[0:1, 0, 0:500],
                                    in1=ru[:1, 0, :], op=A.mult)
            for jj in range(2):
                nc.tensor.matmul(out=pbt[:, jj, 0:500], lhsT=ones1b[:],
                                 rhs=ru[:1, jj, :], start=True, stop=True)
            for blk in range(2):
                nc.vector.tensor_tensor(out=xn[:, blk, co:co + 500],
                                        in0=x[:, blk, co:co + 500],
                                        in1=pbt[:, 0, 0:500], op=A.mult)
                nc.vector.tensor_tensor(out=xn[:, blk, co:co + 500],
                                        in0=xn[:, blk, co:co + 500],
                                        in1=pbt[:, 1, 0:500], op=A.subtract)

        def emit_ln(xn, s):
            emit_ln_finish(xn, s, emit_ln_stats(s))

        def emit_ln_stats_sb(s, st4):
            # Stage (mean, var) at partition 0 so the sqrt/recip finish can
            # run after the whole gelu phase (one table swap per phase).
            # Square is in every act table, so no load here.
            pbt = emit_ln_stats(s)
            m2 = sc.tile([1, 512], f32, tag="m2sc")
            nc.vector.tensor_copy(st4[0:1, 0, s, :], pbt[0:1, 0, 0:500])
            nc.scalar.activation(out=m2[:1, 0:500], in_=pbt[0:1, 0, 0:500],
                                 func=AF.Square)
            nc.vector.tensor_tensor(out=st4[0:1, 1, s, :],
                                    in0=pbt[32:33, 0, 0:500],
                                    in1=m2[:1, 0:500], op=A.subtract)

        def emit_ln_ru4(st4):
            # Batched r/u for all 4 samples (Sqrt + DVE recip: one table load
            # at the phase boundary). Reading the full st4 var plane makes
            # this depend on sample 3's stats, keeping table-based ACT work
            # off the gelu phase.
            ru4 = scr2.tile([1, 2, 4, 500], bf16, tag="ru4")
            nc.scalar.activation(out=ru4[:1, 0, :, :], in_=st4[:1, 1, :, :],
                                 func=AF.Sqrt, bias=epsv[0:1, :])
            # r = 1/sigma; u = mean*r (bf16, same precision as the inline path)
            with nc.allow_low_precision("ln r/u bf16 as inline path"):
                nc.vector.reciprocal(out=ru4[:1, 1, :, :],
                                     in_=ru4[:1, 0, :, :])
            nc.vector.tensor_tensor(out=ru4[:1, 0, :, :],
                                    in0=st4[:1, 0, :, :],
                                    in1=ru4[:1, 1, :, :], op=A.mult)
            return ru4

        def emit_ln_apply(xn, s, ru4):
            # Broadcast r/u for one sample and normalize. Emitted with one
            # sample of lookahead so the in-order PE queue never stalls on ru4.
            co = s * 500
            pbt = pmB.tile([128, 2, 512], f32, tag="pmB")
            nc.tensor.matmul(out=pbt[:, 0, 0:500], lhsT=ones1b[:],
                             rhs=ru4[:1, 1, s, :], start=True, stop=True)
            nc.tensor.matmul(out=pbt[:, 1, 0:500], lhsT=ones1b[:],
                             rhs=ru4[:1, 0, s, :], start=True, stop=True)
            for blk in range(2):
                nc.vector.tensor_tensor(out=xn[:, blk, co:co + 500],
                                        in0=x[:, blk, co:co + 500],
                                        in1=pbt[:, 0, 0:500], op=A.mult)
                nc.vector.tensor_tensor(out=xn[:, blk, co:co + 500],
                                        in0=xn[:, blk, co:co + 500],
                                        in1=pbt[:, 1, 0:500], op=A.subtract)

        UNITS = [(c4, j) for j in range(2) for c4 in range(4)]
        LAG = 4
        xn = big.tile([128, 2, 2000], bf16, tag="xnA")
        for s in range(ns_run):
            emit_ln(xn, s)
        pend_ln = None
        for i in range(nl_run):
            W = LWslot[i % 2]
            attn = big.tile([128, 2, 2000], bf16, tag="attn")
            xn2 = big.tile([128, 2, 2000], bf16, tag="xn2")
            lnst = {}
            for s in range(ns_run):
                cs = s * 500
                if pend_ln is not None and s + 1 < ns_run:
                    emit_ln_apply(pend_ln[0], s + 1, pend_ln[1])
                if s >= 1:
                    lnst[s - 1] = emit_ln_stats(s - 1)
                # double-buffered per sample so qkv(s+1) can run under the
                # units pipeline of sample s
                qkv = scr.tile([128, 6, 512], bf16, tag=f"qkv{s % 2}")
                cqS = scr.tile([8, 512], f32, tag=f"cqS{s % 2}")
                if i == 0 and s <= 1:
                    # pad keys 500..511: k''=0, v=0 -> score 0, av contrib 0
                    nc.vector.memset(qkv[:, 2:6, 500:512], 0.0)
                    # exp bias -30 at pad keys -> eT ~ 0 there
                    nc.vector.memset(cqS[:, 500:512], -30.0)
                for j in [0, 2, 4, 5, 1, 3]:
                    pq = pmA.tile([128, 512], f32, tag="pmA")
                    for kc in range(2):
                        nc.tensor.matmul(
                            out=pq[:, 0:500],
                            lhsT=W["wqkvT"][:, kc, j * 128:(j + 1) * 128],
                            rhs=xn[:, kc, cs:cs + 500],
                            start=(kc == 0), stop=(kc == 1))
                    if j >= 2:
                        nc.vector.tensor_copy(qkv[:, j, 0:500], pq[:, 0:500])
                    else:
                        nc.scalar.activation(out=qkv[:, j, 0:500],
                                             in_=pq[:, 0:500], func=AF.Copy)
                # per-key exp-bias rows c_h = wc_h . zhat (extra qkv outputs)
                pq8 = pmA.tile([128, 512], f32, tag="pmA")
                for kc in range(2):
                    nc.tensor.matmul(out=pq8[0:8, 0:500],
                                     lhsT=W["wqkvT"][:, kc, 768:776],
                                     rhs=xn[:, kc, cs:cs + 500],
                                     start=(kc == 0), stop=(kc == 1))
                nc.vector.tensor_copy(cqS[0:8, 0:500], pq8[0:8, 0:500])
                # V^T via SBUF->SBUF DMA transpose (128-key blocks)
                vts = []
                for j in range(2):
                    vt = scr2.tile([128, 4, 128], bf16, tag=f"vt{j}{s % 2}")
                    for skc in range(4):
                        nc.sync.dma_start_transpose(
                            out=vt[:, skc, :],
                            in_=qkv[:, 4 + j, 128 * skc:128 * skc + 128])
                    vts.append(vt)
                # c-bias transposed into per-key layout: esT cols 32..64
                esT = pmS.tile([128, 512], f32, tag="pmS")
                pot0 = pmP.tile([128, 512], f32, tag="pmP")
                pot1 = pmB.tile([128, 512], f32, tag="potB")
                pots = [pot0, pot1]
                for skc in range(4):
                    nc.tensor.transpose(out=esT[:, 32 + 8 * skc:40 + 8 * skc],
                                        in_=cqS[0:8, 128 * skc:128 * skc + 128],
                                        identity=identF[:8, 0:8])
                cbS = scr.tile([128, 4, 8], f32, tag="cbS")
                nc.vector.tensor_copy(cbS[:], esT[:, 32:64])
                def attn_tail(j):
                    rrT = sc.tile([125, 16], f32, tag=f"rrT{j}")
                    nc.vector.reciprocal(out=rrT[:], in_=esT[:125, 16 * j:16 * j + 16])
                    pcol = 64 + 128 * j
                    nc.tensor.transpose(out=esT[0:16, pcol:pcol + 125], in_=rrT[:],
                                        identity=identF[:125, 0:125])
                    rrTT = sc.tile([16, 125], bf16, tag=f"rrTT{j}")
                    with nc.allow_low_precision("softmax recip bcast bf16"):
                        nc.vector.tensor_copy(rrTT[:], esT[0:16, pcol:pcol + 125])
                    prb = pmA.tile([128, 512], f32, tag="pmA")
                    for qb in range(4):
                        nc.tensor.matmul(out=prb[:, 125 * qb:125 * qb + 125],
                                         lhsT=C["indTc"][:, qb, :], rhs=rrTT[:],
                                         start=True, stop=True)
                    prbS = scr2.tile([128, 500], bf16, tag=f"prbS{j}")
                    nc.vector.tensor_copy(prbS[:], prb[:, 0:500])
                    nc.vector.tensor_tensor(out=attn[:, j, cs:cs + 500],
                                            in0=pots[j][:, 0:500], in1=prbS[:],
                                            op=A.mult)
                eTs = {}
                for t in range(len(UNITS) + LAG):
                    if t < len(UNITS):
                        c4, j = UNITS[t]
                        poff = 32 * c4
                        eT = scr.tile([128, 4, 500], bf16, tag=f"eT{t % 5}")
                        eTs[t] = eT
                        for skc in range(4):
                            psc = pmA.tile([128, 512], f32, tag="pmA")
                            nc.tensor.matmul(
                                out=psc[:, 0:500],
                                lhsT=qkv[poff:poff + 32, 2 + j,
                                         128 * skc:128 * skc + 128],
                                rhs=qkv[poff:poff + 32, j, 0:500],
                                start=True, stop=True,
                                tile_position=(poff, 0))
                            nc.scalar.activation(
                                out=eT[:, skc, :], in_=psc[:, 0:500],
                                func=AF.Exp,
                                bias=cbS[:, skc, 4 * j + c4:4 * j + c4 + 1])
                    if t >= LAG:
                        c4, j = UNITS[t - LAG]
                        poff = 32 * c4
                        eT = eTs.pop(t - LAG)
                        for skc in range(4):
                            nc.tensor.matmul(out=pots[j][poff:poff + 32, 0:500],
                                             lhsT=vts[j][:, skc, poff:poff + 32],
                                             rhs=eT[:, skc, :],
                                             start=(skc == 0), stop=(skc == 3),
                                             tile_position=(0, poff))
                        # esum: accumulate the 4 key blocks directly in PSUM
                        # (out free size 1 -> ~free on PE) instead of a DVE
                        # tree-add of eT.
                        for qb in range(4):
                            for skc in range(4):
                                nc.tensor.matmul(
                                    out=esT[:125, 16 * j + 4 * c4 + qb:
                                            16 * j + 4 * c4 + qb + 1],
                                    lhsT=eT[:, skc, 125 * qb:125 * qb + 125],
                                    rhs=onesPb[:, 0:1],
                                    start=(skc == 0), stop=(skc == 3))
                        if t - LAG == 3:
                            attn_tail(0)
                        elif t - LAG == 7:
                            attn_tail(1)
                # Wo + residual for this sample (bias via 1-row matmul)
                for mb in range(2):
                    po = pmA.tile([128, 512], f32, tag="pmA")
                    for kc in range(2):
                        nc.tensor.matmul(
                            out=po[:, 0:500],
                            lhsT=W["woT"][:, kc, mb * 128:(mb + 1) * 128],
                            rhs=attn[:, kc, cs:cs + 500],
                            start=(kc == 0), stop=(kc == 1))
                    # bias folded into the residual add (saves a 500-col
                    # rank-1 matmul and shortens the psum group)
                    nc.vector.scalar_tensor_tensor(
                        out=x[:, mb, cs:cs + 500], in0=po[:, 0:500],
                        scalar=W["boW"][:, mb:mb + 1],
                        in1=x[:, mb, cs:cs + 500], op0=A.add, op1=A.add)
                if s >= 1:
                    emit_ln_finish(xn2, s - 1, lnst.pop(s - 1))
            emit_ln(xn2, ns_run - 1)
            xn_next = big.tile([128, 2, 2000], bf16,
                               tag=("xnA" if (i + 1) % 2 == 0 else "xnB"))
            st4 = None
            if i + 1 < nl_run:
                st4 = scr2.tile([1, 2, 4, 500], f32, tag="lnsb4",
                                name="lnsb4")
            for s in range(ns_run):
                cs = s * 500
                if i + 1 < nl_run and s >= 1:
                    emit_ln_stats_sb(s - 1, st4)
                fh = scr.tile([128, 8, 500], bf16, tag=f"fh{s % 2}")
                for hb in range(8):
                    phh = pmA.tile([128, 512], f32, tag="pmA")
                    for kc in range(2):
                        nc.tensor.matmul(
                            out=phh[:, 0:500],
                            lhsT=W["w1TT"][:, kc, hb * 128:(hb + 1) * 128],
                            rhs=xn2[:, kc, cs:cs + 500], start=(kc == 0),
                            stop=(kc == 1))
                    nc.scalar.activation(out=fh[:, hb, :], in_=phh[:, 0:500],
                                         func=AF.Gelu, bias=W["b1v"][:, hb:hb + 1])
                for mb in range(2):
                    pf = pmA.tile([128, 512], f32, tag="pmA")
                    for hb in range(8):
                        nc.tensor.matmul(
                            out=pf[:, 0:500],
                            lhsT=W["w2TT"][:, hb, mb * 128:(mb + 1) * 128],
                            rhs=fh[:, hb, :], start=(hb == 0), stop=(hb == 7))
                    nc.vector.scalar_tensor_tensor(
                        out=x[:, mb, cs:cs + 500], in0=pf[:, 0:500],
                        scalar=W["b2W"][:, mb:mb + 1],
                        in1=x[:, mb, cs:cs + 500], op0=A.add, op1=A.add)
            if i + 1 < nl_run:
                emit_ln_stats_sb(ns_run - 1, st4)
                ru4 = emit_ln_ru4(st4)
                emit_ln_apply(xn_next, 0, ru4)
                pend_ln = (xn_next, ru4)
            if i + 2 < nl_run:
                LWslot[i % 2] = load_layer_weights(i % 2, i + 2)
            xn = xn_next

        # ================= pooling + head =================
        # pass A: attention-pool weights + fused weighted-sum per sample;
        # pass B: the scalar head chain, overlapped across samples
        pooleds = []
        for s in range(ns_run):
            cs = s * 500
            plg = pmS.tile([128, 512], f32, tag="pmS")
            for blk in range(2):
                nc.tensor.matmul(out=plg[0:1, 0:500],
                                 lhsT=C["poolq"][:, blk:blk + 1],
                                 rhs=x[:, blk, cs:cs + 500], start=(blk == 0),
                                 stop=(blk == 1))
            wrow = sc.tile([1, 500], f32r, tag="wrow")
            nc.scalar.activation(out=wrow[:], in_=plg[0:1, 0:500], func=AF.Exp,
                                 scale=1.0 / 16.0)
            pwb = pmA.tile([128, 512], f32, tag="pmA")
            nc.tensor.matmul(out=pwb[:, 0:500], lhsT=ones1r[:], rhs=wrow[:],
                             start=True, stop=True)
            wx = scr2.tile([128, 2, 500], f32, tag="wx")
            pooled = sc.tile([128, 4], f32, tag=f"pooled{s}")
            pooleds.append(pooled)
            for blk in range(2):
                nc.vector.tensor_tensor(out=wx[:, blk, :],
                                        in0=x[:, blk, cs:cs + 500],
                                        in1=pwb[:, 0:500], op=A.mult)
            nc.vector.tensor_reduce(out=pooled[:, 0:2], in_=wx[:], axis=AX.X,
                                    op=A.add)
        for s in range(ns_run):
            pooled = pooleds[s]
            nc.scalar.activation(out=pooled[:, 2:4], in_=pooled[:, 0:2],
                                 func=AF.Square)
            pps = pmS.tile([128, 512], f32, tag="pmS")
            nc.tensor.matmul(out=pps[0:1, 0:4], lhsT=onesPf[:], rhs=pooled[:],
                             start=True, stop=True)
            z = sc.tile([1, 16], f32, tag="z")
            nc.vector.tensor_copy(z[:, 12:16], pps[0:1, 0:4])
            nc.vector.tensor_tensor(out=z[:, 0:1], in0=z[:, 12:13],
                                    in1=z[:, 13:14], op=A.add)
            nc.vector.tensor_tensor(out=z[:, 1:2], in0=z[:, 14:15],
                                    in1=z[:, 15:16], op=A.add)
            nc.vector.tensor_scalar(out=z[:, 2:3], in0=z[:, 0:1],
                                    scalar1=1.0 / 256.0, scalar2=None, op0=A.mult)
            nc.vector.tensor_tensor(out=z[:, 3:4], in0=z[:, 2:3], in1=z[:, 2:3],
                                    op=A.mult)
            nc.vector.tensor_scalar(out=z[:, 4:5], in0=z[:, 1:2],
                                    scalar1=1.0 / 256.0, scalar2=None, op0=A.mult)
            nc.vector.tensor_tensor(out=z[:, 4:5], in0=z[:, 4:5], in1=z[:, 3:4],
                                    op=A.subtract)
            nc.scalar.activation(out=z[:, 5:6], in_=z[:, 4:5], func=AF.Ln,
                                 bias=epsv[0:1, :])
            nc.scalar.activation(out=z[:, 6:7], in_=z[:, 5:6], func=AF.Exp,
                                 scale=-0.5)
            cp = sc.tile([128, 2], f32, tag="cp")
            nc.vector.tensor_tensor(out=cp[:], in0=pooled[:, 0:2], in1=C["c1v"][:],
                                    op=A.mult)
            pa = pmS.tile([128, 512], f32, tag="pmS")
            nc.tensor.matmul(out=pa[0:1, 0:2], lhsT=onesPf[:], rhs=cp[:],
                             start=True, stop=True)
            nc.vector.tensor_copy(z[:, 10:12], pa[0:1, 0:2])
            nc.vector.tensor_tensor(out=z[:, 7:8], in0=z[:, 10:11],
                                    in1=z[:, 11:12], op=A.add)
            nc.vector.tensor_scalar(out=z[:, 8:9], in0=z[:, 2:3], scalar1=sc1,
                                    scalar2=None, op0=A.mult)
            nc.vector.tensor_tensor(out=z[:, 8:9], in0=z[:, 7:8], in1=z[:, 8:9],
                                    op=A.subtract)
            nc.vector.tensor_tensor(out=z[:, 8:9], in0=z[:, 8:9], in1=z[:, 6:7],
                                    op=A.mult)
            nc.vector.tensor_scalar(out=z[:, 9:10], in0=z[:, 8:9], scalar1=c2s,
                                    scalar2=None, op0=A.add)
            nc.sync.dma_start(out=Y[s:s + 1, :], in_=z[:, 9:10])

        for p in [pmS, pmP, pmB, pmA, scr2, scr, big, wpool, bigx, sc, sing]:
            p.release()

    nc.compile()
    return nc


_BUILT = {}


def _get_nc(key, **kw):
    if key not in _BUILT:
        _BUILT[key] = _build(**kw)
    return _BUILT[key]


def _make_in_maps(prep, ns=NS, ncores=NCORES):
    in_maps = []
    for c in range(ncores):
        sl = slice(c * ns, (c + 1) * ns)
        m = {k: prep[k] for k in CONST_KEYS}
        m["mel4"] = prep["_mel"][sl]
        m["evrows"] = prep["_evrows"][sl]
        m["combv"] = prep["_combv"][sl]
        m["tposv"] = prep["_tposv"][sl]
        m["keepv"] = prep["_keepv"][sl]
        m["starbias"] = prep["_starbias"][sl]
        in_maps.append(m)
    return in_maps


def kernel(**inputs):
    from concourse.bass_utils import run_bass_kernel_spmd

    prep = _host_prep(inputs)
    nc = _get_nc("full", nl_run=NLAYERS, ns_run=NS, debug=False,
                 sc1=prep["sc1"], c2s=prep["c2s"])
    res = run_bass_kernel_spmd(nc, _make_in_maps(prep), list(range(NCORES)))
    y = np.concatenate([res.results[c]["y"].reshape(-1) for c in range(NCORES)])
    return y.astype(np.float32)



# revision 60
# speedup vs baseline: 1.1073x; 1.0028x over previous
"""Trainium2 Bass kernel for nn_ChartQualityEvaluator.

Data parallel: 32 samples -> 8 cores x 4 samples. Feature-major activations
[128 part, 2 blocks, 4*500 cols]; residual stream kept in float32r so the
LN-mean / pooling matmuls stream at 1 cyc/row instead of fp32's 4. Host
folds LN gain/bias into adjacent weights, q/k biases into a per-key exp
bias, the v bias into Wo's bias, and precomputes the event sinusoid
features (pure functions of host data, DMA'd in). Softmax esum accumulates
straight from eT via free 1-col matmuls (no DVE tree). Wo/W2 biases ride
the residual add (scalar_tensor_tensor). Transformer weights stream
through two SBUF slots with DMA prefetch. Inter-layer LN: stats staged to
SBUF during the gelu phase (Square is in every act table), batched
sqrt+reciprocal at the phase boundary, and per-sample broadcast/normalize
emitted one sample ahead in the next attention loop so the in-order PE
queue never stalls on it.
"""
import math
import sys

import numpy as np

_TRN = "/opt/trn_rl_repo"
if _TRN not in sys.path:
    sys.path.insert(0, _TRN)

import ml_dtypes

BF16 = ml_dtypes.bfloat16

D = 256
H = 8
NLAYERS = 6
HALF = 128
S = 500
NEV = 256
NCORES = 8
NS = 4  # samples per core
B = 32
EPS = 1e-5
INV2PI = float(np.float32(1.0 / (2.0 * math.pi)))
TWOPI = 2.0 * math.pi
SC32 = 1.0 / math.sqrt(32.0)


def _host_prep(inp):
    f = np.float32
    f8 = np.float64
    out = {}

    def t2(v):  # [256] -> [128,2] fp32
        return np.ascontiguousarray(np.asarray(v).reshape(2, 128).T.astype(f))

    def b(a):  # to bf16
        return np.ascontiguousarray(np.asarray(a).astype(f).astype(BF16))

    # ---------------- frontend ----------------
    out["wmelT"] = b(np.asarray(inp["mel_W"]).T)
    out["melb"] = np.ascontiguousarray(np.asarray(inp["mel_b"]).reshape(16, 1).astype(f))
    out["w1t"] = b(np.asarray(inp["conv1_w"]).transpose(1, 2, 0))
    out["c1b"] = np.ascontiguousarray(np.asarray(inp["conv1_b"]).reshape(128, 1).astype(f))
    out["gng"] = np.ascontiguousarray(np.asarray(inp["gn_g"]).reshape(128, 1).astype(f))
    out["gnb"] = np.ascontiguousarray(np.asarray(inp["gn_b"]).reshape(128, 1).astype(f))
    out["w2t"] = b(np.asarray(inp["conv2_w"]).transpose(1, 2, 0))
    out["c2b"] = t2(inp["conv2_b"])
    out["cng"] = t2(inp["cn_g"])

    freq = np.exp(np.arange(HALF, dtype=f) * f(-math.log(10000.0) / (HALF - 1)))
    e32 = (np.arange(S, dtype=f)[None, :] * freq[:, None]).astype(f)
    e64 = e32.astype(np.float64)
    pos_fm = np.concatenate([np.sin(e64), np.cos(e64)], axis=0)  # [256,500]
    out["posT"] = np.ascontiguousarray(
        pos_fm.reshape(2, 128, S).transpose(1, 0, 2).astype(f))
    out["freqv"] = np.ascontiguousarray(freq.reshape(128, 1))

    out["epW1T"] = np.ascontiguousarray(
        np.asarray(inp["ep_W1"]).T.reshape(6, 128, 256).transpose(1, 0, 2).astype(f).astype(BF16))
    out["epb1"] = t2(inp["ep_b1"])
    out["epW2T"] = np.ascontiguousarray(
        np.asarray(inp["ep_W2"]).T.reshape(2, 128, 256).transpose(1, 0, 2).astype(f).astype(BF16))
    out["epb2row"] = np.ascontiguousarray(
        np.tile(np.asarray(inp["ep_b2"]).astype(f)[None, :], (128, 1)))

    def wT(w, kc, m, dt=BF16):  # w [m, k] -> [128, kc, m]
        return np.ascontiguousarray(
            np.asarray(w).astype(f).T.reshape(kc, 128, m).transpose(1, 0, 2).astype(dt))

    # ---------------- transformer with LN folding ----------------
    wqkvT, woT, boW, w1TT, b1v, w2TT, b2W = [], [], [], [], [], [], []
    for i in range(NLAYERS):
        g1 = np.asarray(inp["tl_ln1g"][i]).astype(f8)
        b1 = np.asarray(inp["tl_ln1b"][i]).astype(f8)
        g2 = np.asarray(inp["tl_ln2g"][i]).astype(f8)
        b2 = np.asarray(inp["tl_ln2b"][i]).astype(f8)
        Wqkv = np.asarray(inp["tl_Wqkv"][i]).astype(f8)   # [768, 256]
        bqkv = np.asarray(inp["tl_bqkv"][i]).astype(f8)   # [768]
        Wo = np.asarray(inp["tl_Wo"][i]).astype(f8)       # [256, 256]
        bo = np.asarray(inp["tl_bo"][i]).astype(f8)
        W1 = np.asarray(inp["tl_W1"][i]).astype(f8)       # [1024, 256]
        b1f = np.asarray(inp["tl_b1"][i]).astype(f8)
        W2 = np.asarray(inp["tl_W2"][i]).astype(f8)       # [256, 1024]
        b2f = np.asarray(inp["tl_b2"][i]).astype(f8)

        Wq, Wk, Wv = Wqkv[0:256], Wqkv[256:512], Wqkv[512:768]
        Wq_f = Wq * g1[None, :]
        Wk_f = Wk * g1[None, :] * SC32
        Wv_f = Wv * g1[None, :]
        bq_eff = Wq @ b1 + bqkv[0:256]          # survives as per-key exp bias
        bv_eff = Wv @ b1 + bqkv[512:768]        # folds into Wo bias
        # wc_h: c_h[k] = bq_eff_h . k''_h[:,k] = (Wk_f[h].T @ bq_eff[h]) . zhat
        WC = np.stack([Wk_f[32 * h:32 * h + 32].T @ bq_eff[32 * h:32 * h + 32]
                       for h in range(8)])      # [8, 256]
        W_f = np.concatenate([Wq_f, Wk_f, Wv_f, WC], axis=0)  # [776, 256]
        wqkvT.append(wT(W_f, 2, 776))
        bo_eff = bo + Wo @ bv_eff
        woT.append(wT(Wo, 2, 256))
        boW.append(t2(bo_eff.astype(f)))
        W1_f = W1 * g2[None, :]
        b1_eff = W1 @ b2 + b1f
        w1TT.append(wT(W1_f, 2, 1024))
        b1v.append(np.ascontiguousarray(
            b1_eff.astype(f).reshape(8, 128).T.astype(f)))     # [128, 8] fp32
        w2TT.append(wT(W2, 8, 256))
        b2W.append(t2(b2f.astype(f)))
    out["wqkvT"] = np.stack(wqkvT)
    out["woT"] = np.stack(woT)
    out["boW"] = np.stack(boW)
    out["w1TT"] = np.stack(w1TT)
    out["b1v"] = np.stack(b1v)
    out["w2TT"] = np.stack(w2TT)
    out["b2W"] = np.stack(b2W)

    indT = np.zeros((16, 4, 128), np.float32)
    for qb in range(4):
        for p in range(128):
            indT[4 * (p // 32) + qb, qb, p] = 1.0
    out["indTc"] = np.ascontiguousarray(indT.astype(BF16))

    # ---------------- pooling head ----------------
    out["poolq"] = np.ascontiguousarray(
        np.asarray(inp["pool_q"]).astype(f).reshape(2, 128).T)
    c1 = (np.asarray(inp["oh_W"])[0] * np.asarray(inp["on_g"])).astype(f)
    out["c1v"] = t2(c1)
    out["sc1"] = float(c1.astype(np.float64).sum())
    out["c2s"] = float((np.asarray(inp["oh_W"])[0].astype(np.float64)
                        * np.asarray(inp["on_b"]).astype(np.float64)).sum()
                       + float(np.asarray(inp["oh_b"])[0]))

    # ---------------- per-sample host precompute ----------------
    events = np.asarray(inp["events"]).astype(np.int64)
    mask = np.asarray(inp["event_mask"])
    star = np.asarray(inp["star_rating"]).astype(f)
    nb = events.shape[0]
    diff = np.maximum(events[:, 1:] - events[:, :-1], 1)
    g = np.concatenate([diff[:, :1], diff], axis=1)
    gap_ms = (g * 5).astype(f)
    g_f = np.maximum(g.astype(f), f(1.0))
    r = np.clip(g_f[:, 1:] / g_f[:, :-1], f(0.1), f(10.0)).astype(f)
    ones = np.ones((nb, 1), f)
    rb50 = np.trunc(np.concatenate([ones, r], axis=1) * f(50.0)).astype(f)
    ra50 = np.trunc(np.concatenate([r, ones], axis=1) * f(50.0)).astype(f)
    out["_evrows"] = np.ascontiguousarray(np.stack([rb50, ra50, gap_ms], axis=1))
    # event sinusoids are pure functions of host data: precompute [B,128,6,256]
    argv = (out["_evrows"][:, None, :, :]
            * freq[None, :, None, None]).astype(np.float32)
    combv = np.empty((nb, HALF, 6, NEV), np.float32)
    combv[:, :, 0::2, :] = np.sin(argv)
    combv[:, :, 1::2, :] = np.cos(argv)
    out["_combv"] = np.ascontiguousarray(combv.astype(BF16))
    tp = np.clip(events // 4, 0, S - 1).astype(f)
    keep = (1.0 - mask.astype(f)).astype(f)
    out["_tposv"] = np.ascontiguousarray(tp.reshape(nb, 2, 128).transpose(0, 2, 1))
    out["_keepv"] = np.ascontiguousarray(keep.reshape(nb, 2, 128).transpose(0, 2, 1))
    bucket = np.clip((star / f(0.5)).astype(np.int32), 0, 19)
    sb = (np.asarray(inp["cn_b"])[None, :] + np.asarray(inp["star_table"])[bucket]).astype(f)
    out["_starbias"] = np.ascontiguousarray(sb.reshape(nb, 2, 128).transpose(0, 2, 1))
    out["_mel"] = np.asarray(inp["mel"]).astype(f).astype(BF16)
    return out


CONST_KEYS = ["wmelT", "melb", "w1t", "c1b", "gng", "gnb", "w2t", "c2b", "cng",
              "posT", "freqv", "epW1T", "epb1", "epW2T", "epb2row",
              "wqkvT", "woT", "boW", "w1TT", "b1v", "w2TT", "b2W",
              "poolq", "c1v", "indTc"]


def _build(nl_run=NLAYERS, ns_run=NS, debug=False, sc1=0.0, c2s=0.0):
    import concourse.bacc as bacc
    import concourse.tile as tile
    from concourse import mybir
    from concourse.masks import make_identity

    class _Bacc(bacc.Bacc):
        # Restrict activation-table choice so ln/exp (and everything the
        # kernel needs besides gelu/sin) resolve to one shared table --
        # avoids a table reload on every LayerNorm. Table ids stay
        # positional, so unused entries are blanked rather than removed.
        _KEEP = {"natural_log_exp_and_others", "trig_and_small",
                 "gelu_and_others", "sqrt_and_others"}

        def insert_act_table_loads(self):
            from concourse.hw_specs import get_activation_tables
            import bass_rust as _bass_rust
            has_activation = any(
                isinstance(i, mybir.InstActivation)
                for b in self.main_func.blocks
                for i in b.instructions
            )
            if not has_activation:
                return
            tables = [
                (name, funcs if name in self._KEEP else set())
                for name, funcs in get_activation_tables(self.m.arch).items()
            ]
            _bass_rust.insert_act_table_loads(self, tables)

    f32 = mybir.dt.float32
    bf16 = mybir.dt.bfloat16
    f32r = mybir.dt.float32r
    f8 = mybir.dt.float8e4
    A = mybir.AluOpType
    AF = mybir.ActivationFunctionType
    AX = mybir.AxisListType
    DR = mybir.MatmulPerfMode.DoubleRow

    def r32(ap):
        # fp32 matmul streams at 4 cyc/row; f32r at 1 (ap>=256). Same bytes.
        return ap.bitcast(f32r)

    nc = _Bacc(None)
    P = {}
    shapes = dict(
        mel4=([ns_run, 80, 2000], bf16),
        evrows=([ns_run, 3, 256], f32r),
        combv=([ns_run, 128, 6, 256], bf16),
        tposv=([ns_run, 128, 2], f32),
        keepv=([ns_run, 128, 2], f32),
        starbias=([ns_run, 128, 2], f32),
        wmelT=([80, 16], bf16), melb=([16, 1], f32),
        w1t=([16, 7, 128], bf16), c1b=([128, 1], f32),
        gng=([128, 1], f32), gnb=([128, 1], f32),
        w2t=([128, 7, 256], bf16), c2b=([128, 2], f32),
        cng=([128, 2], f32), posT=([128, 2, 500], f32), freqv=([128, 1], f32),
        epW1T=([128, 6, 256], bf16), epb1=([128, 2], f32),
        epW2T=([128, 2, 256], bf16), epb2row=([128, 256], f32),
        wqkvT=([NLAYERS, 128, 2, 776], bf16),
        woT=([NLAYERS, 128, 2, 256], bf16), boW=([NLAYERS, 128, 2], f32),
        w1TT=([NLAYERS, 128, 2, 1024], bf16), b1v=([NLAYERS, 128, 8], f32),
        w2TT=([NLAYERS, 128, 8, 256], bf16), b2W=([NLAYERS, 128, 2], f32),
        poolq=([128, 2], f32r), c1v=([128, 2], f32),
        indTc=([16, 4, 128], bf16),
    )
    for k, (sh, dt) in shapes.items():
        P[k] = nc.declare_dram_parameter(k, sh, dt, isOutput=False)
    Y = nc.declare_dram_parameter("y", [ns_run, 1], f32, isOutput=True)
    DBG = None
    if debug:
        DBG = nc.declare_dram_parameter("dbg_x0", [128, 2, 2000], f32, isOutput=True)

    with tile.TileContext(nc) as tc:
        sing = tc.alloc_tile_pool(name="sing", bufs=1)
        sc = tc.alloc_tile_pool(name="sc", bufs=2)
        bigx = tc.alloc_tile_pool(name="bigx", bufs=1)
        wpool = tc.alloc_tile_pool(name="wpool", bufs=1)
        # PSUM pools: pmA rotating 1-bank x3, pmB 2-bank, pmP 1-bank,
        # pmS 1-bank (stats+esum), pmT small bf16 transpose staging
        pmA = tc.alloc_tile_pool(name="pmA", bufs=3, space="PSUM")
        pmB = tc.alloc_tile_pool(name="pmB", bufs=1, space="PSUM")
        pmP = tc.alloc_tile_pool(name="pmP", bufs=1, space="PSUM")
        pmS = tc.alloc_tile_pool(name="pmS", bufs=1, space="PSUM")

        fc = tc.alloc_tile_pool(name="fc", bufs=1)
        _FRONT = ["wmelT", "melb", "w1t", "c1b", "gng", "gnb", "w2t", "c2b",
                  "cng", "posT", "freqv", "epW1T", "epb1", "epW2T", "epb2row"]
        C = {}
        for k in _FRONT + ["poolq", "c1v", "indTc"]:
            pool = fc if k in _FRONT else sing
            C[k] = pool.tile(shapes[k][0], shapes[k][1], tag=k, name="c_" + k)
            nc.sync.dma_start(out=C[k][:], in_=P[k][:])
        # transformer weights stream through 2 SBUF slots (DMA is nearly
        # idle); slot for layer i+2 refills while layer i+1 runs
        WSPECS = [("wqkvT", [128, 2, 776], bf16),
                  ("woT", [128, 2, 256], bf16),
                  ("boW", [128, 2], f32),
                  ("w1TT", [128, 2, 1024], bf16),
                  ("b1v", [128, 8], f32),
                  ("w2TT", [128, 8, 256], bf16),
                  ("b2W", [128, 2], f32)]

        def load_layer_weights(slot, i):
            W = {}
            for k, sh, dt in WSPECS:
                W[k] = wpool.tile(sh, dt, tag=f"w{slot}_{k}",
                                  name=f"w{slot}_{k}")
                nc.sync.dma_start(out=W[k][:], in_=P[k][i])
            return W

        LWslot = None  # loaded after the frontend DMAs are queued

        identF = sing.tile([128, 128], f32, tag="identF")
        make_identity(nc, identF[:])
        onesPb = sing.tile([128, 1], bf16, tag="onesPb")      # plain ones bf16
        nc.vector.memset(onesPb[:], 1.0)
        onesP8 = sing.tile([128, 1], f8, tag="onesP8")        # ones fp8e4
        nc.vector.memset(onesP8[:], 1.0)
        onesMb = sing.tile([128, 1], bf16, tag="onesMb")      # 1/256 (stats lhsT)
        nc.vector.memset(onesMb[:], 1.0 / 256.0)
        onesMf = sing.tile([128, 1], f32r, tag="onesMf")      # 1/256 f32r
        _mfs = sing.tile([128, 1], f32, tag="_mfs")
        nc.vector.memset(_mfs[:], 1.0 / 256.0)
        nc.vector.tensor_copy(onesMf[:], _mfs[:])
        ones1b = sing.tile([1, 128], bf16, tag="ones1b")
        nc.vector.memset(ones1b[:], 1.0)
        ones1f = sing.tile([1, 128], f32, tag="ones1f")
        nc.vector.memset(ones1f[:], 1.0)
        ones1r = sing.tile([1, 128], f32r, tag="ones1r")
        _1rs = sing.tile([1, 128], f32, tag="_1rs")
        nc.vector.memset(_1rs[:], 1.0)
        nc.vector.tensor_copy(ones1r[:], _1rs[:])
        ones500b = sing.tile([1, 500], bf16, tag="ones500b")
        nc.vector.memset(ones500b[:], 1.0)
        onesPf = sing.tile([128, 1], f32, tag="onesPf")
        nc.vector.memset(onesPf[:], 1.0)
        zerov = fc.tile([128, 1], f32, tag="zerov")
        nc.vector.memset(zerov[:], 0.0)
        epsv = sing.tile([128, 1], f32, tag="epsv")
        nc.vector.memset(epsv[:], EPS)
        iotaB = fc.tile([128, 500], f32, tag="iotaB")
        nc.gpsimd.iota(iotaB[:], pattern=[[1, 500]], base=0, channel_multiplier=0,
                       allow_small_or_imprecise_dtypes=True)

        # f32r residual stream: mean/pool matmuls stream at 1 cyc/row
        x = bigx.tile([128, 2, 2000], f32r, tag="x_fm")

        def s2(t):  # step-2 view of [p, n] -> [p, n//2]
            return t.rearrange("p (t s) -> p s t", s=2)[:, 0, :]

        # ================= front end =================
        fr = tc.alloc_tile_pool(name="fr", bufs=2)
        for s in range(ns_run):
            cs = s * 500
            melp = fr.tile([80, 2006], bf16, tag="melp")
            nc.vector.memset(melp[:, 0:3], 0.0)
            nc.vector.memset(melp[:, 2003:2006], 0.0)
            nc.sync.dma_start(out=melp[:, 3:2003], in_=P["mel4"][s])
            xmelp = fr.tile([16, 2006], bf16, tag="xmelp")
            nc.vector.memset(xmelp[:, 0:3], 0.0)
            nc.vector.memset(xmelp[:, 2003:2006], 0.0)
            for nch in range(4):
                pcm = pmA.tile([128, 512], f32, tag="pmA")
                nc.tensor.matmul(out=pcm[:16, 0:500], lhsT=C["wmelT"][:],
                                 rhs=melp[:, 3 + nch * 500: 3 + nch * 500 + 500],
                                 start=True, stop=True)
                nc.scalar.activation(out=xmelp[:, 3 + nch * 500: 3 + nch * 500 + 500],
                                     in_=pcm[:16, 0:500], func=AF.Identity,
                                     bias=C["melb"][:, 0:1])
            pc1 = pmB.tile([128, 2, 512], f32, tag="pmB")
            for half in range(2):
                for k in range(7):
                    nc.tensor.matmul(
                        out=pc1[:, half, 0:500], lhsT=C["w1t"][:, k, :],
                        rhs=s2(xmelp[:, k + half * 1000: k + half * 1000 + 1000]),
                        start=(k == 0), stop=(k == 6))
            h1g = fr.tile([128, 2, 500], bf16, tag="h1g")
            stg = fr.tile([128, 2], f32, tag="stg")
            nc.scalar.activation(out=h1g[:], in_=pc1[:, :, 0:500], func=AF.Gelu,
                                 bias=C["c1b"][:, 0:1], accum_out=stg[:, 0:1])
            sqf = fr.tile([128, 2, 500], bf16, tag="sqf")
            nc.scalar.activation(out=sqf[:], in_=h1g[:], func=AF.Square,
                                 accum_out=stg[:, 1:2])
            pg = pmS.tile([128, 512], f32, tag="pmS")
            nc.tensor.matmul(out=pg[:1, 0:2], lhsT=onesPf[:], rhs=stg[:],
                             start=True, stop=True)
            sn = sc.tile([1, 8], f32, tag="sn")
            nc.vector.tensor_scalar(out=sn[:, 0:2], in0=pg[:1, 0:2],
                                    scalar1=1.0 / 128000.0, scalar2=None, op0=A.mult)
            nc.vector.tensor_tensor(out=sn[:, 2:3], in0=sn[:, 0:1], in1=sn[:, 0:1],
                                    op=A.mult)
            nc.vector.tensor_tensor(out=sn[:, 3:4], in0=sn[:, 1:2], in1=sn[:, 2:3],
                                    op=A.subtract)
            nc.scalar.activation(out=sn[:, 4:5], in_=sn[:, 3:4], func=AF.Ln,
                                 bias=epsv[0:1, :])
            nc.scalar.activation(out=sn[:, 1:2], in_=sn[:, 4:5], func=AF.Exp,
                                 scale=-0.5)
            pgb = pmA.tile([128, 512], f32, tag="pmA")
            nc.tensor.matmul(out=pgb[:, 0:2], lhsT=ones1f[:], rhs=sn[:, 0:2],
                             start=True, stop=True)
            sv = sc.tile([128, 2], f32, tag="sv")
            nc.vector.tensor_tensor(out=sv[:, 0:1], in0=pgb[:, 1:2], in1=C["gng"][:],
                                    op=A.mult)
            nc.vector.tensor_tensor(out=sv[:, 1:2], in0=pgb[:, 0:1], in1=sv[:, 0:1],
                                    op=A.mult)
            nc.vector.tensor_tensor(out=sv[:, 1:2], in0=C["gnb"][:], in1=sv[:, 1:2],
                                    op=A.subtract)
            x2p = fr.tile([128, 1006], bf16, tag="x2p")
            nc.vector.memset(x2p[:, 0:3], 0.0)
            nc.vector.memset(x2p[:, 1003:1006], 0.0)
            nc.scalar.activation(out=x2p[:, 3:1003],
                                 in_=h1g.rearrange("p a b -> p (a b)"),
                                 func=AF.Identity, scale=sv[:, 0:1], bias=sv[:, 1:2])
            pc2 = pmB.tile([128, 2, 512], f32, tag="pmB")
            for mb in range(2):
                for k in range(7):
                    nc.tensor.matmul(out=pc2[:, mb, 0:500],
                                     lhsT=C["w2t"][:, k, mb * 128:(mb + 1) * 128],
                                     rhs=s2(x2p[:, k:k + 1000]),
                                     start=(k == 0), stop=(k == 6))
            for mb in range(2):
                nc.scalar.activation(out=x[:, mb, cs:cs + 500], in_=pc2[:, mb, 0:500],
                                     func=AF.Gelu, bias=C["c2b"][:, mb:mb + 1])
            # CN layernorm (stats in bf16) + cng scale + starbias + pos
            sbv = fr.tile([128, 2], f32, tag="sbv")
            nc.sync.dma_start(out=sbv[:], in_=P["starbias"][s])
            sq2 = fr.tile([128, 2, 500], bf16, tag="sqf")
            nc.vector.tensor_tensor(out=sq2[:], in0=x[:, :, cs:cs + 500],
                                    in1=x[:, :, cs:cs + 500], op=A.mult)
            pstt = pmS.tile([128, 512], f32, tag="pmS")
            for blk in range(2):
                nc.tensor.matmul(out=pstt[0:1, 0:500], lhsT=onesMf[:],
                                 rhs=x[:, blk, cs:cs + 500],
                                 start=(blk == 0), stop=(blk == 1))
            for blk in range(2):
                nc.tensor.matmul(out=pstt[32:33, 0:500], lhsT=onesMb[:],
                                 rhs=sq2[:, blk, :], start=(blk == 0), stop=(blk == 1))
            ru = sc.tile([1, 2, 500], bf16, tag="ru")
            tmp = sc.tile([1, 2, 500], f32, tag="tmp1")
            nc.scalar.activation(out=tmp[:1, 0, :], in_=pstt[0:1, 0:500],
                                 func=AF.Square)
            nc.vector.tensor_tensor(out=tmp[:1, 1, :], in0=pstt[32:33, 0:500],
                                    in1=tmp[:1, 0, :], op=A.subtract)
            nc.scalar.activation(out=tmp[:1, 0, :], in_=tmp[:1, 1, :], func=AF.Ln,
                                 bias=epsv[0:1, :])
            nc.scalar.activation(out=ru[:1, 0, :], in_=tmp[:1, 0, :],
                                 func=AF.Exp, scale=-0.5)
            nc.vector.tensor_tensor(out=ru[:1, 1, :], in0=pstt[0:1, 0:500],
                                    in1=ru[:1, 0, :], op=A.mult)
            pbc = pmB.tile([128, 2, 512], f32, tag="pmB")
            for jj in range(2):
                nc.tensor.matmul(out=pbc[:, jj, 0:500], lhsT=ones1b[:],
                                 rhs=ru[:1, jj, :], start=True, stop=True)
            for blk in range(2):
                nc.vector.tensor_tensor(out=x[:, blk, cs:cs + 500],
                                        in0=x[:, blk, cs:cs + 500],
                                        in1=pbc[:, 0, 0:500], op=A.mult)
                nc.vector.tensor_tensor(out=x[:, blk, cs:cs + 500],
                                        in0=x[:, blk, cs:cs + 500],
                                        in1=pbc[:, 1, 0:500], op=A.subtract)
                nc.scalar.activation(out=x[:, blk, cs:cs + 500],
                                     in_=x[:, blk, cs:cs + 500], func=AF.Identity,
                                     scale=C["cng"][:, blk:blk + 1],
                                     bias=sbv[:, blk:blk + 1])
            nc.vector.tensor_tensor(out=x[:, :, cs:cs + 500], in0=x[:, :, cs:cs + 500],
                                    in1=C["posT"][:], op=A.add)

            # events
            tpv = fr.tile([128, 2], f32, tag="tpv")
            nc.sync.dma_start(out=tpv[:], in_=P["tposv"][s])
            kpv = fr.tile([128, 2], f32, tag="kpv")
            nc.sync.dma_start(out=kpv[:], in_=P["keepv"][s])
            comb = fr.tile([128, 6, 256], bf16, tag="comb")
            nc.sync.dma_start(out=comb[:], in_=P["combv"][s])
            hmid = fr.tile([128, 2, 256], bf16, tag="hmid")
            for mb in range(2):
                ph = pmA.tile([128, 512], f32, tag="pmA")
                for kc in range(6):
                    nc.tensor.matmul(out=ph[:, 0:256],
                                     lhsT=C["epW1T"][:, kc, mb * 128:(mb + 1) * 128],
                                     rhs=comb[:, kc, :], start=(kc == 0),
                                     stop=(kc == 5))
                nc.scalar.activation(out=hmid[:, mb, :], in_=ph[:, 0:256],
                                     func=AF.Gelu, bias=C["epb1"][:, mb:mb + 1])
            evt = fr.tile([128, 2, 256], bf16, tag="evt")
            for ec in range(2):
                pe = pmA.tile([128, 512], f32, tag="pmA")
                for kc in range(2):
                    nc.tensor.matmul(out=pe[:, 0:256],
                                     lhsT=hmid[:, kc, ec * 128:(ec + 1) * 128],
                                     rhs=C["epW2T"][:, kc, :], start=(kc == 0),
                                     stop=(kc == 1))
                nc.vector.tensor_tensor(out=evt[:, ec, :], in0=pe[:, 0:256],
                                        in1=C["epb2row"][:], op=A.add)
                nc.vector.tensor_scalar(out=evt[:, ec, :], in0=evt[:, ec, :],
                                        scalar1=kpv[:, ec:ec + 1], scalar2=None,
                                        op0=A.mult)
            oh = fr.tile([128, 2, 500], bf16, tag="oh")
            for ec in range(2):
                nc.vector.tensor_scalar(out=oh[:, ec, :], in0=iotaB[:],
                                        scalar1=tpv[:, ec:ec + 1], scalar2=None,
                                        op0=A.is_equal)
            for mb in range(2):
                px = pmA.tile([128, 512], f32, tag="pmA")
                for ec in range(2):
                    nc.tensor.matmul(out=px[:, 0:500],
                                     lhsT=evt[:, ec, mb * 128:(mb + 1) * 128],
                                     rhs=oh[:, ec, :], start=(ec == 0), stop=(ec == 1))
                nc.vector.tensor_tensor(out=x[:, mb, cs:cs + 500],
                                        in0=x[:, mb, cs:cs + 500], in1=px[:, 0:500],
                                        op=A.add)
        fr.release()
        fc.release()
        LWslot = [load_layer_weights(0, 0)]
        if nl_run > 1:
            LWslot.append(load_layer_weights(1, 1))
        big = tc.alloc_tile_pool(name="big", bufs=1)
        scr = tc.alloc_tile_pool(name="scr", bufs=1)
        scr2 = tc.alloc_tile_pool(name="scr2", bufs=1)

        if debug:
            nc.sync.dma_start(out=DBG[:], in_=x[:])

        # ================= transformer =================
        def emit_ln_stats(s):
            # Per-column mean / E[x^2] of x(s) into rows 0/32 of a pmB tile
            # (same tile later reused for the r/u broadcast).
            co = s * 500
            sq = scr2.tile([128, 2, 500], bf16, tag="sq")
            nc.vector.tensor_tensor(out=sq[:], in0=x[:, :, co:co + 500],
                                    in1=x[:, :, co:co + 500], op=A.mult)
            pbt = pmB.tile([128, 2, 512], f32, tag="pmB")
            for blk in range(2):
                nc.tensor.matmul(out=pbt[0:1, 0, 0:500], lhsT=onesMf[:],
                                 rhs=x[:, blk, co:co + 500],
                                 start=(blk == 0), stop=(blk == 1))
            for blk in range(2):
                nc.tensor.matmul(out=pbt[32:33, 0, 0:500], lhsT=onesMb[:],
                                 rhs=sq[:, blk, :], start=(blk == 0),
                                 stop=(blk == 1))
            return pbt

        def emit_ln_finish(xn, s, pbt):
            # 1/sigma = exp(-0.5*ln(var)) keeps ACT on the exp/ln table.
            co = s * 500
            ru = sc.tile([1, 2, 500], bf16, tag="ru")
            tmp = sc.tile([1, 2, 500], f32, tag="tmp1")
            nc.scalar.activation(out=tmp[:1, 0, :], in_=pbt[0:1, 0, 0:500],
                                 func=AF.Square)
            nc.vector.tensor_tensor(out=tmp[:1, 1, :], in0=pbt[32:33, 0, 0:500],
                                    in1=tmp[:1, 0, :], op=A.subtract)
            nc.scalar.activation(out=tmp[:1, 0, :], in_=tmp[:1, 1, :],
                                 func=AF.Ln, bias=epsv[0:1, :])
            nc.scalar.activation(out=ru[:1, 0, :], in_=tmp[:1, 0, :],
                                 func=AF.Exp, scale=-0.5)
            nc.vector.tensor_tensor(out=ru[:1, 1, :], in0=pbt[0:1, 0, 0:500],
                                    in1=ru[:1, 0, :], op=A.mult)
            for jj in range(2):
                nc.tensor.matmul(out=pbt[:, jj, 0:500], lhsT=ones1b[:],
                                 rhs=ru[:1, jj, :], start=True, stop=True)
            for blk in range(2):
                nc.vector.tensor_tensor(out=xn[:, blk, co:co + 500],
                                        in0=x[:, blk, co:co + 500],
                                        in1=pbt[:, 0, 0:500], op=A.mult)
                nc.vector.tensor_tensor(out=xn[:, blk, co:co + 500],
                                        in0=xn[:, blk, co:co + 500],
                                        in1=pbt[:, 1, 0:500], op=A.subtract)

        def emit_ln(xn, s):
            emit_ln_finish(xn, s, emit_ln_stats(s))

        def emit_ln_stats_sb(s, st4):
            # Stage (mean, var) at partition 0 so the sqrt/recip finish can
            # run after the whole gelu phase (one table swap per phase).
            # Square is in every act table, so no load here.
            pbt = emit_ln_stats(s)
            m2 = sc.tile([1, 512], f32, tag="m2sc")
            nc.vector.tensor_copy(st4[0:1, 0, s, :], pbt[0:1, 0, 0:500])
            nc.scalar.activation(out=m2[:1, 0:500], in_=pbt[0:1, 0, 0:500],
                                 func=AF.Square)
            nc.vector.tensor_tensor(out=st4[0:1, 1, s, :],
                                    in0=pbt[32:33, 0, 0:500],
                                    in1=m2[:1, 0:500], op=A.subtract)

        def emit_ln_ru4(st4):
            # Batched r/u for all 4 samples (Sqrt + DVE recip: one table load
            # at the phase boundary). Reading the full st4 var plane makes
            # this depend on sample 3's stats, keeping table-based ACT work
            # off the gelu phase.
            ru4 = scr2.tile([1, 2, 4, 500], bf16, tag="ru4")
            nc.scalar.activation(out=ru4[:1, 0, :, :], in_=st4[:1, 1, :, :],
                                 func=AF.Sqrt, bias=epsv[0:1, :])
            # r = 1/sigma; u = mean*r (bf16, same precision as the inline
            # path). Sample 0 first so apply(0) -> qkv(0) unblocks before the
            # remaining samples' rows are processed.
            with nc.allow_low_precision("ln r/u bf16 as inline path"):
                nc.vector.reciprocal(out=ru4[:1, 1, 0, :],
                                     in_=ru4[:1, 0, 0, :])
            nc.vector.tensor_tensor(out=ru4[:1, 0, 0, :],
                                    in0=st4[:1, 0, 0, :],
                                    in1=ru4[:1, 1, 0, :], op=A.mult)
            with nc.allow_low_precision("ln r/u bf16 as inline path"):
                nc.vector.reciprocal(out=ru4[:1, 1, 1:4, :],
                                     in_=ru4[:1, 0, 1:4, :])
            nc.vector.tensor_tensor(out=ru4[:1, 0, 1:4, :],
                                    in0=st4[:1, 0, 1:4, :],
                                    in1=ru4[:1, 1, 1:4, :], op=A.mult)
            return ru4

        def emit_ln_apply(xn, s, ru4):
            # Broadcast r/u for one sample and normalize. Emitted with one
            # sample of lookahead so the in-order PE queue never stalls on ru4.
            co = s * 500
            pbt = pmB.tile([128, 2, 512], f32, tag="pmB")
            nc.tensor.matmul(out=pbt[:, 0, 0:500], lhsT=ones1b[:],
                             rhs=ru4[:1, 1, s, :], start=True, stop=True)
            nc.tensor.matmul(out=pbt[:, 1, 0:500], lhsT=ones1b[:],
                             rhs=ru4[:1, 0, s, :], start=True, stop=True)
            for blk in range(2):
                nc.vector.tensor_tensor(out=xn[:, blk, co:co + 500],
                                        in0=x[:, blk, co:co + 500],
                                        in1=pbt[:, 0, 0:500], op=A.mult)
                nc.vector.tensor_tensor(out=xn[:, blk, co:co + 500],
                                        in0=xn[:, blk, co:co + 500],
                                        in1=pbt[:, 1, 0:500], op=A.subtract)

        UNITS = [(c4, j) for j in range(2) for c4 in range(4)]
        LAG = 4
        xn = big.tile([128, 2, 2000], bf16, tag="xnA")
        for s in range(ns_run):
            emit_ln(xn, s)
        pend_ln = None
        for i in range(nl_run):
            W = LWslot[i % 2]
            attn = big.tile([128, 2, 2000], bf16, tag="attn")
            xn2 = big.tile([128, 2, 2000], bf16, tag="xn2")
            lnst = {}
            for s in range(ns_run):
                cs = s * 500
                if pend_ln is not None and s + 1 < ns_run:
                    emit_ln_apply(pend_ln[0], s + 1, pend_ln[1])
                if s >= 1:
                    lnst[s - 1] = emit_ln_stats(s - 1)
                # double-buffered per sample so qkv(s+1) can run under the
                # units pipeline of sample s
                qkv = scr.tile([128, 6, 512], bf16, tag=f"qkv{s % 2}")
                cqS = scr.tile([8, 512], f32, tag=f"cqS{s % 2}")
                if i == 0 and s <= 1:
                    # pad keys 500..511: k''=0, v=0 -> score 0, av contrib 0
                    nc.vector.memset(qkv[:, 2:6, 500:512], 0.0)
                    # exp bias -30 at pad keys -> eT ~ 0 there
                    nc.vector.memset(cqS[:, 500:512], -30.0)
                for j in [0, 2, 4, 5, 1, 3]:
                    pq = pmA.tile([128, 512], f32, tag="pmA")
                    for kc in range(2):
                        nc.tensor.matmul(
                            out=pq[:, 0:500],
                            lhsT=W["wqkvT"][:, kc, j * 128:(j + 1) * 128],
                            rhs=xn[:, kc, cs:cs + 500],
                            start=(kc == 0), stop=(kc == 1))
                    if j >= 2:
                        nc.vector.tensor_copy(qkv[:, j, 0:500], pq[:, 0:500])
                    else:
                        nc.scalar.activation(out=qkv[:, j, 0:500],
                                             in_=pq[:, 0:500], func=AF.Copy)
                # per-key exp-bias rows c_h = wc_h . zhat (extra qkv outputs)
                pq8 = pmA.tile([128, 512], f32, tag="pmA")
                for kc in range(2):
                    nc.tensor.matmul(out=pq8[0:8, 0:500],
                                     lhsT=W["wqkvT"][:, kc, 768:776],
                                     rhs=xn[:, kc, cs:cs + 500],
                                     start=(kc == 0), stop=(kc == 1))
                nc.vector.tensor_copy(cqS[0:8, 0:500], pq8[0:8, 0:500])
                # V^T via SBUF->SBUF DMA transpose (128-key blocks)
                vts = []
                for j in range(2):
                    vt = scr2.tile([128, 4, 128], bf16, tag=f"vt{j}{s % 2}")
                    for skc in range(4):
                        nc.sync.dma_start_transpose(
                            out=vt[:, skc, :],
                            in_=qkv[:, 4 + j, 128 * skc:128 * skc + 128])
                    vts.append(vt)
                # c-bias transposed into per-key layout: esT cols 32..64
                esT = pmS.tile([128, 512], f32, tag="pmS")
                pot0 = pmP.tile([128, 512], f32, tag="pmP")
                pot1 = pmB.tile([128, 512], f32, tag="potB")
                pots = [pot0, pot1]
                for skc in range(4):
                    nc.tensor.transpose(out=esT[:, 32 + 8 * skc:40 + 8 * skc],
                                        in_=cqS[0:8, 128 * skc:128 * skc + 128],
                                        identity=identF[:8, 0:8])
                cbS = scr.tile([128, 4, 8], f32, tag="cbS")
                nc.vector.tensor_copy(cbS[:], esT[:, 32:64])
                def attn_tail(j):
                    rrT = sc.tile([125, 16], f32, tag=f"rrT{j}")
                    nc.vector.reciprocal(out=rrT[:], in_=esT[:125, 16 * j:16 * j + 16])
                    pcol = 64 + 128 * j
                    nc.tensor.transpose(out=esT[0:16, pcol:pcol + 125], in_=rrT[:],
                                        identity=identF[:125, 0:125])
                    rrTT = sc.tile([16, 125], bf16, tag=f"rrTT{j}")
                    with nc.allow_low_precision("softmax recip bcast bf16"):
                        nc.vector.tensor_copy(rrTT[:], esT[0:16, pcol:pcol + 125])
                    prb = pmA.tile([128, 512], f32, tag="pmA")
                    for qb in range(4):
                        nc.tensor.matmul(out=prb[:, 125 * qb:125 * qb + 125],
                                         lhsT=C["indTc"][:, qb, :], rhs=rrTT[:],
                                         start=True, stop=True)
                    prbS = scr2.tile([128, 500], bf16, tag=f"prbS{j}")
                    nc.vector.tensor_copy(prbS[:], prb[:, 0:500])
                    nc.vector.tensor_tensor(out=attn[:, j, cs:cs + 500],
                                            in0=pots[j][:, 0:500], in1=prbS[:],
                                            op=A.mult)
                eTs = {}
                for t in range(len(UNITS) + LAG):
                    if t < len(UNITS):
                        c4, j = UNITS[t]
                        poff = 32 * c4
                        eT = scr.tile([128, 4, 500], bf16, tag=f"eT{t % 5}")
                        eTs[t] = eT
                        for skc in range(4):
                            psc = pmA.tile([128, 512], f32, tag="pmA")
                            nc.tensor.matmul(
                                out=psc[:, 0:500],
                                lhsT=qkv[poff:poff + 32, 2 + j,
                                         128 * skc:128 * skc + 128],
                                rhs=qkv[poff:poff + 32, j, 0:500],
                                start=True, stop=True,
                                tile_position=(poff, 0))
                            nc.scalar.activation(
                                out=eT[:, skc, :], in_=psc[:, 0:500],
                                func=AF.Exp,
                                bias=cbS[:, skc, 4 * j + c4:4 * j + c4 + 1])
                    if t >= LAG:
                        c4, j = UNITS[t - LAG]
                        poff = 32 * c4
                        eT = eTs.pop(t - LAG)
                        for skc in range(4):
                            nc.tensor.matmul(out=pots[j][poff:poff + 32, 0:500],
                                             lhsT=vts[j][:, skc, poff:poff + 32],
                                             rhs=eT[:, skc, :],
                                             start=(skc == 0), stop=(skc == 3),
                                             tile_position=(0, poff))
                        # esum: accumulate the 4 key blocks directly in PSUM
                        # (out free size 1 -> ~free on PE) instead of a DVE
                        # tree-add of eT.
                        for qb in range(4):
                            for skc in range(4):
                                nc.tensor.matmul(
                                    out=esT[:125, 16 * j + 4 * c4 + qb:
                                            16 * j + 4 * c4 + qb + 1],
                                    lhsT=eT[:, skc, 125 * qb:125 * qb + 125],
                                    rhs=onesPb[:, 0:1],
                                    start=(skc == 0), stop=(skc == 3))
                        if t - LAG == 3:
                            attn_tail(0)
                        elif t - LAG == 7:
                            attn_tail(1)
                # Wo + residual for this sample (bias via 1-row matmul)
                for mb in range(2):
                    po = pmA.tile([128, 512], f32, tag="pmA")
                    for kc in range(2):
                        nc.tensor.matmul(
                            out=po[:, 0:500],
                            lhsT=W["woT"][:, kc, mb * 128:(mb + 1) * 128],
                            rhs=attn[:, kc, cs:cs + 500],
                            start=(kc == 0), stop=(kc == 1))
                    # bias folded into the residual add (saves a 500-col
                    # rank-1 matmul and shortens the psum group)
                    nc.vector.scalar_tensor_tensor(
                        out=x[:, mb, cs:cs + 500], in0=po[:, 0:500],
                        scalar=W["boW"][:, mb:mb + 1],
                        in1=x[:, mb, cs:cs + 500], op0=A.add, op1=A.add)
                if s >= 1:
                    emit_ln_finish(xn2, s - 1, lnst.pop(s - 1))
            emit_ln(xn2, ns_run - 1)
            xn_next = big.tile([128, 2, 2000], bf16,
                               tag=("xnA" if (i + 1) % 2 == 0 else "xnB"))
            st4 = None
            if i + 1 < nl_run:
                st4 = scr2.tile([1, 2, 4, 500], f32, tag="lnsb4",
                                name="lnsb4")
            for s in range(ns_run):
                cs = s * 500
                if i + 1 < nl_run and s >= 1:
                    emit_ln_stats_sb(s - 1, st4)
                fh = scr.tile([128, 8, 500], bf16, tag=f"fh{s % 2}")
                for hb in range(8):
                    phh = pmA.tile([128, 512], f32, tag="pmA")
                    for kc in range(2):
                        nc.tensor.matmul(
                            out=phh[:, 0:500],
                            lhsT=W["w1TT"][:, kc, hb * 128:(hb + 1) * 128],
                            rhs=xn2[:, kc, cs:cs + 500], start=(kc == 0),
                            stop=(kc == 1))
                    nc.scalar.activation(out=fh[:, hb, :], in_=phh[:, 0:500],
                                         func=AF.Gelu, bias=W["b1v"][:, hb:hb + 1])
                for mb in range(2):
                    pf = pmA.tile([128, 512], f32, tag="pmA")
                    for hb in range(8):
                        nc.tensor.matmul(
                            out=pf[:, 0:500],
                            lhsT=W["w2TT"][:, hb, mb * 128:(mb + 1) * 128],
                            rhs=fh[:, hb, :], start=(hb == 0), stop=(hb == 7))
                    nc.vector.scalar_tensor_tensor(
                        out=x[:, mb, cs:cs + 500], in0=pf[:, 0:500],
                        scalar=W["b2W"][:, mb:mb + 1],
                        in1=x[:, mb, cs:cs + 500], op0=A.add, op1=A.add)
            if i + 1 < nl_run:
                emit_ln_stats_sb(ns_run - 1, st4)
                ru4 = emit_ln_ru4(st4)
                emit_ln_apply(xn_next, 0, ru4)
                pend_ln = (xn_next, ru4)
            if i + 2 < nl_run:
                LWslot[i % 2] = load_layer_weights(i % 2, i + 2)
            xn = xn_next

        # ================= pooling + head =================
        # pass A: attention-pool weights + fused weighted-sum per sample;
        # pass B: the scalar head chain, overlapped across samples
        pooleds = []
        for s in range(ns_run):
            cs = s * 500
            plg = pmS.tile([128, 512], f32, tag="pmS")
            for blk in range(2):
                nc.tensor.matmul(out=plg[0:1, 0:500],
                                 lhsT=C["poolq"][:, blk:blk + 1],
                                 rhs=x[:, blk, cs:cs + 500], start=(blk == 0),
                                 stop=(blk == 1))
            wrow = sc.tile([1, 500], f32r, tag="wrow")
            nc.scalar.activation(out=wrow[:], in_=plg[0:1, 0:500], func=AF.Exp,
                                 scale=1.0 / 16.0)
            pwb = pmA.tile([128, 512], f32, tag="pmA")
            nc.tensor.matmul(out=pwb[:, 0:500], lhsT=ones1r[:], rhs=wrow[:],
                             start=True, stop=True)
            wx = scr2.tile([128, 2, 500], f32, tag="wx")
            pooled = sc.tile([128, 4], f32, tag=f"pooled{s}")
            pooleds.append(pooled)
            for blk in range(2):
                nc.vector.tensor_tensor(out=wx[:, blk, :],
                                        in0=x[:, blk, cs:cs + 500],
                                        in1=pwb[:, 0:500], op=A.mult)
            nc.vector.tensor_reduce(out=pooled[:, 0:2], in_=wx[:], axis=AX.X,
                                    op=A.add)
        for s in range(ns_run):
            pooled = pooleds[s]
            nc.scalar.activation(out=pooled[:, 2:4], in_=pooled[:, 0:2],
                                 func=AF.Square)
            pps = pmS.tile([128, 512], f32, tag="pmS")
            nc.tensor.matmul(out=pps[0:1, 0:4], lhsT=onesPf[:], rhs=pooled[:],
                             start=True, stop=True)
            z = sc.tile([1, 16], f32, tag="z")
            nc.vector.tensor_copy(z[:, 12:16], pps[0:1, 0:4])
            nc.vector.tensor_tensor(out=z[:, 0:1], in0=z[:, 12:13],
                                    in1=z[:, 13:14], op=A.add)
            nc.vector.tensor_tensor(out=z[:, 1:2], in0=z[:, 14:15],
                                    in1=z[:, 15:16], op=A.add)
            nc.vector.tensor_scalar(out=z[:, 2:3], in0=z[:, 0:1],
                                    scalar1=1.0 / 256.0, scalar2=None, op0=A.mult)
            nc.vector.tensor_tensor(out=z[:, 3:4], in0=z[:, 2:3], in1=z[:, 2:3],
                                    op=A.mult)
            nc.vector.tensor_scalar(out=z[:, 4:5], in0=z[:, 1:2],
                                    scalar1=1.0 / 256.0, scalar2=None, op0=A.mult)
            nc.vector.tensor_tensor(out=z[:, 4:5], in0=z[:, 4:5], in1=z[:, 3:4],
                                    op=A.subtract)
            nc.scalar.activation(out=z[:, 5:6], in_=z[:, 4:5], func=AF.Ln,
                                 bias=epsv[0:1, :])
            nc.scalar.activation(out=z[:, 6:7], in_=z[:, 5:6], func=AF.Exp,
                                 scale=-0.5)
            cp = sc.tile([128, 2], f32, tag="cp")
            nc.vector.tensor_tensor(out=cp[:], in0=pooled[:, 0:2], in1=C["c1v"][:],
                                    op=A.mult)
            pa = pmS.tile([128, 512], f32, tag="pmS")
            nc.tensor.matmul(out=pa[0:1, 0:2], lhsT=onesPf[:], rhs=cp[:],
                             start=True, stop=True)
            nc.vector.tensor_copy(z[:, 10:12], pa[0:1, 0:2])
            nc.vector.tensor_tensor(out=z[:, 7:8], in0=z[:, 10:11],
                                    in1=z[:, 11:12], op=A.add)
            nc.vector.tensor_scalar(out=z[:, 8:9], in0=z[:, 2:3], scalar1=sc1,
                                    scalar2=None, op0=A.mult)
            nc.vector.tensor_tensor(out=z[:, 8:9], in0=z[:, 7:8], in1=z[:, 8:9],
                                    op=A.subtract)
            nc.vector.tensor_tensor(out=z[:, 8:9], in0=z[:, 8:9], in1=z[:, 6:7],
                                    op=A.mult)
            nc.vector.tensor_scalar(out=z[:, 9:10], in0=z[:, 8:9], scalar1=c2s,
                                    scalar2=None, op0=A.add)
            nc.sync.dma_start(out=Y[s:s + 1, :], in_=z[:, 9:10])

        for p in [pmS, pmP, pmB, pmA, scr2, scr, big, wpool, bigx, sc, sing]:
            p.release()

    nc.compile()
    return nc


_BUILT = {}


def _get_nc(key, **kw):
    if key not in _BUILT:
        _BUILT[key] = _build(**kw)
    return _BUILT[key]


def _make_in_maps(prep, ns=NS, ncores=NCORES):
    in_maps = []
    for c in range(ncores):
        sl = slice(c * ns, (c + 1) * ns)
        m = {k: prep[k] for k in CONST_KEYS}
        m["mel4"] = prep["_mel"][sl]
        m["evrows"] = prep["_evrows"][sl]
        m["combv"] = prep["_combv"][sl]
        m["tposv"] = prep["_tposv"][sl]
        m["keepv"] = prep["_keepv"][sl]
        m["starbias"] = prep["_starbias"][sl]
        in_maps.append(m)
    return in_maps


def kernel(**inputs):
    from concourse.bass_utils import run_bass_kernel_spmd

    prep = _host_prep(inputs)
    nc = _get_nc("full", nl_run=NLAYERS, ns_run=NS, debug=False,
                 sc1=prep["sc1"], c2s=prep["c2s"])
    res = run_bass_kernel_spmd(nc, _make_in_maps(prep), list(range(NCORES)))
    y = np.concatenate([res.results[c]["y"].reshape(-1) for c in range(NCORES)])
    return y.astype(np.float32)

